# revision 29
# baseline (speedup 1.0000x reference)
"""Complex AttentionPool2d on 8 trn2 NeuronCores, data-parallel over batch.

Contract: kernel(**inputs) takes the FULL inputs from setup_inputs() and
returns the FULL [32, 512] complex64 output.

V2: all matmuls bf16 (fp32 PSUM accum); k^T eliminated algebraically.
Math (per batch):
  x = bf16(complex(x_real, x_imag)).reshape(E, 256)
  x_cat = [mean(x), x] + pos                       # [E, 257]
  q0 = x_cat[:, 0] @ wq^T + bq                     # only query pos 0 matters
  qk[h, e] = sum_d q0[h*64+d] wk[h*64+d, e]        # fold q into k-proj
  logits[h, s] = sum_e qk[h, e] x_cat[e, s]        # == q0 . k[s]
  w = softmax(logits.re) + i*softmax(logits.im)
  v = x_cat^T @ wv^T                               # [257, 512]
  attn0 = (w @ v) per-head masked; y = attn0 @ (w_p @ w_out)^T + b_c

Sharding: batch 32 -> 4 per core.
"""
import numpy as np

B, E, HW, S = 32, 512, 256, 257
SP = 258            # S padded even
NH, HD = 8, 64
OUT = 512
NCORES = 8
BPC = B // NCORES   # batches per core

_cached = {}


def _build():
    import concourse.bacc as bacc
    import concourse.tile as tile
    import concourse.mybir as mybir

    F32 = mybir.dt.float32
    BF16 = mybir.dt.bfloat16
    AX = mybir.AxisListType
    ACTF = mybir.ActivationFunctionType

    nc = bacc.Bacc("TRN2", target_bir_lowering=False, debug=False)

    # ---- DRAM I/O ----
    # x layout: [E, BPC, SP] so one DMA per e-tile covers all 4 batches;
    # col 0 reserved for the mean token, col 257 zero pad
    d_xr = nc.dram_tensor("xr", [E, BPC, SP], BF16, kind="ExternalInput")
    d_xi = nc.dram_tensor("xi", [E, BPC, SP], BF16, kind="ExternalInput")
    d_wqr = nc.dram_tensor("wqr", [128, 4, E], BF16, kind="ExternalInput")
    d_wqi = nc.dram_tensor("wqi", [128, 4, E], BF16, kind="ExternalInput")
    d_wkr = nc.dram_tensor("wkr", [128, 4, E], BF16, kind="ExternalInput")
    d_wki = nc.dram_tensor("wki", [128, 4, E], BF16, kind="ExternalInput")
    d_wvr = nc.dram_tensor("wvr", [128, 4, OUT], BF16, kind="ExternalInput")
    d_wvi = nc.dram_tensor("wvi", [128, 4, OUT], BF16, kind="ExternalInput")
    d_wcr = nc.dram_tensor("wcr", [128, 4, OUT], BF16, kind="ExternalInput")
    d_wci = nc.dram_tensor("wci", [128, 4, OUT], BF16, kind="ExternalInput")
    d_bqr = nc.dram_tensor("bqr", [128, 4], F32, kind="ExternalInput")
    d_bqi = nc.dram_tensor("bqi", [128, 4], F32, kind="ExternalInput")
    d_bcr = nc.dram_tensor("bcr", [BPC, OUT], F32, kind="ExternalInput")
    d_bci = nc.dram_tensor("bci", [BPC, OUT], F32, kind="ExternalInput")
    d_id = nc.dram_tensor("ident", [128, 128], F32, kind="ExternalInput")
    d_mask = nc.dram_tensor("mask8", [NH, OUT], F32, kind="ExternalInput")
    d_sel = nc.dram_tensor("sel32", [32, BPC], BF16, kind="ExternalInput")
    d_zbd = nc.dram_tensor("zbd", [128, 32], BF16, kind="ExternalInput")
    d_yr = nc.dram_tensor("yr", [BPC, OUT], F32, kind="ExternalOutput")
    d_yi = nc.dram_tensor("yi", [BPC, OUT], F32, kind="ExternalOutput")

    with tile.TileContext(nc) as tc:
        with tc.tile_pool(name="consts", bufs=1) as consts, \
             tc.tile_pool(name="vpool", bufs=1) as vpool:
            # ---- persistent weights / constants (bf16) ----
            wv_r = consts.tile([128, 4, OUT], BF16)
            wv_i = consts.tile([128, 4, OUT], BF16)
            wv_in = consts.tile([128, 4, OUT], BF16)
            wq_r = consts.tile([128, 4, E], BF16)
            wq_i = consts.tile([128, 4, E], BF16)
            wk_r = consts.tile([128, 4, E], BF16)
            wk_i = consts.tile([128, 4, E], BF16)
            wc_r = consts.tile([128, 4, OUT], BF16)
            wc_i = consts.tile([128, 4, OUT], BF16)
            wvr = [wv_r[:, e, :] for e in range(4)]
            wvi = [wv_i[:, e, :] for e in range(4)]
            wvin = [wv_in[:, e, :] for e in range(4)]
            wqr = [wq_r[:, e, :] for e in range(4)]
            wqi = [wq_i[:, e, :] for e in range(4)]
            wkr = [wk_r[:, e, :] for e in range(4)]
            wki = [wk_i[:, e, :] for e in range(4)]
            wcr = [wc_r[:, e, :] for e in range(4)]
            wci = [wc_i[:, e, :] for e in range(4)]
            bqr = consts.tile([128, 4], F32)
            bqi = consts.tile([128, 4], F32)
            bqin = consts.tile([128, 4], F32)
            bcr = consts.tile([BPC, OUT], F32)
            bci = consts.tile([BPC, OUT], F32)
            ident = consts.tile([128, 128], F32)
            mask8 = consts.tile([NH, OUT], F32)
            sel32 = consts.tile([32, BPC], BF16)

            # x tiles: [128e, BPC, SP]
            xbr = [vpool.tile([128, BPC, SP], BF16, name=f"xbr{e}") for e in range(4)]
            xbi = [vpool.tile([128, BPC, SP], BF16, name=f"xbi{e}") for e in range(4)]
            x0in = [vpool.tile([128, BPC], BF16, name=f"x0in{e}") for e in range(4)]
            # v tiles live until hv
            vr = [[vpool.tile([128, OUT], BF16, name=f"vr{b}_{s}")
                   for s in range(2)] for b in range(BPC)]
            vi = [[vpool.tile([128, OUT], BF16, name=f"vi{b}_{s}")
                   for s in range(2)] for b in range(BPC)]
            vCr_sb = vpool.tile([BPC, OUT], BF16)
            vCi_sb = vpool.tile([BPC, OUT], BF16)
            # bd: per-u zero-padded block-diag q0 [128, 32] (cols b*8+2u+p)
            bd_r = [vpool.tile([128, 32], BF16, name=f"bd_r{u}") for u in range(4)]
            bd_i = [vpool.tile([128, 32], BF16, name=f"bd_i{u}") for u in range(4)]
            bd_in = [vpool.tile([128, 32], BF16, name=f"bd_in{u}") for u in range(4)]
            q0r_sb = vpool.tile([BPC, E], F32)
            q0i_sb = vpool.tile([BPC, E], F32)
            qk_sb_r = vpool.tile([32, E], F32)
            qk_sb_i = vpool.tile([32, E], F32)
            qkT_r = [vpool.tile([128, 32], BF16, name=f"qkTr{e}") for e in range(4)]
            qkT_i = [vpool.tile([128, 32], BF16, name=f"qkTi{e}") for e in range(4)]
            qkT_in = [vpool.tile([128, 32], BF16, name=f"qkTin{e}") for e in range(4)]

            # ---- DMA issue order matters per queue ----
            # sync queue: small consts then x real
            nc.sync.dma_start(out=ident, in_=d_id.ap())
            nc.sync.dma_start(out=sel32, in_=d_sel.ap())
            nc.sync.dma_start(out=mask8, in_=d_mask.ap())
            nc.sync.dma_start(out=bqr, in_=d_bqr.ap())
            nc.sync.dma_start(out=bqi, in_=d_bqi.ap())
            nc.sync.dma_start(out=bcr, in_=d_bcr.ap())
            nc.sync.dma_start(out=bci, in_=d_bci.ap())
            for u in range(4):
                nc.sync.dma_start(out=bd_r[u], in_=d_zbd.ap())
                nc.sync.dma_start(out=bd_i[u], in_=d_zbd.ap())
                nc.sync.dma_start(out=bd_in[u], in_=d_zbd.ap())
            # Two hw DMA queues (scalar, gpsimd), interleaved in first-need
            # order; weights are pre-tiled [128, 4, X] on host so every DMA
            # is contiguous per partition.
            for e in range(4):
                sl = slice(e * 128, (e + 1) * 128)
                nc.scalar.dma_start(out=xbr[e][:], in_=d_xr.ap()[sl, :, :])
                nc.scalar.dma_start(out=wv_r[:, e, :], in_=d_wvr.ap()[:, e, :])
                nc.gpsimd.dma_start(out=xbi[e][:], in_=d_xi.ap()[sl, :, :])
                nc.gpsimd.dma_start(out=wv_i[:, e, :], in_=d_wvi.ap()[:, e, :])
            nc.scalar.dma_start(out=wq_r[:], in_=d_wqr.ap())
            nc.scalar.dma_start(out=wq_i[:], in_=d_wqi.ap())
            nc.gpsimd.dma_start(out=wk_r[:], in_=d_wkr.ap())
            nc.gpsimd.dma_start(out=wk_i[:], in_=d_wki.ap())
            nc.gpsimd.dma_start(out=wc_r[:], in_=d_wcr.ap())
            nc.gpsimd.dma_start(out=wc_i[:], in_=d_wci.ap())

            nc.vector.tensor_scalar_mul(bqin, bqi, -1.0)
            # negate wv imag on device (keeps it off the DMA critical path)
            for e in range(4):
                nc.vector.tensor_scalar_mul(wv_in[:, e, :], wv_i[:, e, :], -1.0)
            # x arrives fully prepped from host (mean in col 0, pos added);
            # only the negated imag of token 0 is built on device
            for e in range(4):
                nc.scalar.activation(x0in[e][:], xbi[e][:, :, 0], ACTF.Copy,
                                     bias=0.0, scale=-1.0)

            mm = nc.tensor.matmul

            with tc.tile_pool(name="psA", bufs=2, space="PSUM") as psA:
                # v rows s in [sb*128, (sb+1)*128) = x cols (col 0 = mean tok)
                def emit_v(b):
                    for sb in range(2):
                        cs = slice(sb * 128, (sb + 1) * 128)
                        p1 = psA.tile([128, OUT], F32, tag="pv1", name=f"pv1_{b}_{sb}")
                        pi = psA.tile([128, OUT], F32, tag="pvi", name=f"pvi_{b}_{sb}")
                        for j, (x, w) in enumerate(
                                [(xbr[e][:, b, cs], wvr[e]) for e in range(4)]
                                + [(xbi[e][:, b, cs], wvin[e]) for e in range(4)]):
                            mm(p1[:], x, w[:], start=(j == 0), stop=(j == 7))
                        for j, (x, w) in enumerate(
                                [(xbr[e][:, b, cs], wvi[e]) for e in range(4)]
                                + [(xbi[e][:, b, cs], wvr[e]) for e in range(4)]):
                            mm(pi[:], x, w[:], start=(j == 0), stop=(j == 7))
                        nc.vector.tensor_copy(vr[b][sb][:], p1[:])
                        nc.scalar.copy(vi[b][sb][:], pi[:])

                emit_v(0)

                # ============ q0 -> bd ============
                with tc.tile_pool(name="psB1", bufs=1, space="PSUM") as psB1:
                    pqr = psB1.tile([BPC, E], F32, tag="pqr")
                    pqi = psB1.tile([BPC, E], F32, tag="pqi")
                    for j, (x, w) in enumerate(
                            [(xbr[e][:, :, 0], wqr[e][:]) for e in range(4)]
                            + [(x0in[e][:], wqi[e][:]) for e in range(4)]):
                        mm(pqr[:], x, w, start=(j == 0), stop=(j == 7))
                    for j, (x, w) in enumerate(
                            [(xbr[e][:, :, 0], wqi[e][:]) for e in range(4)]
                            + [(xbi[e][:, :, 0], wqr[e][:]) for e in range(4)]):
                        mm(pqi[:], x, w, start=(j == 0), stop=(j == 7))
                    nc.scalar.copy(q0r_sb[:], pqr[:])
                    nc.scalar.copy(q0i_sb[:], pqi[:])

                    # transpose q0 -> bd block-diag [128, 4u, 8]
                    # bd[p*64+d, u, 2b+p] = q0[b, u*128+p*64+d] + bq bias
                    for u in range(4):
                        fs = slice(u * 128, (u + 1) * 128)
                        ptr = psB1.tile([128, 4], F32, tag="ptq", bufs=1, name=f"ptq{u}")
                        pti = psB1.tile([128, 4], F32, tag="ptj", bufs=1, name=f"ptj{u}")
                        nc.tensor.transpose(ptr[:], q0r_sb[:, fs], ident[0:BPC, 0:BPC])
                        nc.tensor.transpose(pti[:], q0i_sb[:, fs], ident[0:BPC, 0:BPC])
                        for p in range(2):
                            rows = slice(p * 64, (p + 1) * 64)
                            cols = slice(2 * u + p, 32, 8)
                            nc.scalar.activation(bd_r[u][rows, cols], ptr[rows, :],
                                                 ACTF.Identity,
                                                 bias=bqr[rows, u:u + 1], scale=1.0)
                            nc.scalar.activation(bd_i[u][rows, cols], pti[rows, :],
                                                 ACTF.Identity,
                                                 bias=bqi[rows, u:u + 1], scale=1.0)
                            nc.scalar.activation(bd_in[u][rows, cols], pti[rows, :],
                                                 ACTF.Identity,
                                                 bias=bqin[rows, u:u + 1], scale=-1.0)

                # next v batch fills PE while bd copies run
                emit_v(1)

                # ============ qk = bd^T @ wk  [rows b*8 + 2u+p, 512e] ============
                with tc.tile_pool(name="psQK", bufs=1, space="PSUM") as psQK:
                    pkr = psQK.tile([32, E], F32, tag="pkr")
                    pki = psQK.tile([32, E], F32, tag="pki")
                    for j, (bdt, w) in enumerate(
                            [(bd_r[u], wkr[u]) for u in range(4)]
                            + [(bd_in[u], wki[u]) for u in range(4)]):
                        mm(pkr[:], bdt[:], w[:], start=(j == 0), stop=(j == 7))
                    for j, (bdt, w) in enumerate(
                            [(bd_r[u], wki[u]) for u in range(4)]
                            + [(bd_i[u], wkr[u]) for u in range(4)]):
                        mm(pki[:], bdt[:], w[:], start=(j == 0), stop=(j == 7))
                    nc.vector.tensor_copy(qk_sb_r[:], pkr[:])
                    nc.scalar.copy(qk_sb_i[:], pki[:])

                # v b2 + vC fill PE during qk copies / qkT transposes
                emit_v(2)

                # vC: token-256 v row for all batches
                with tc.tile_pool(name="psVC", bufs=1, space="PSUM") as psVC:
                    p1 = psVC.tile([BPC, OUT], F32, tag="pc1")
                    pi = psVC.tile([BPC, OUT], F32, tag="pci")
                    for j, (x, w) in enumerate(
                            [(xbr[e][:, :, 256], wvr[e]) for e in range(4)]
                            + [(xbi[e][:, :, 256], wvin[e]) for e in range(4)]):
                        mm(p1[:], x, w[:], start=(j == 0), stop=(j == 7))
                    for j, (x, w) in enumerate(
                            [(xbr[e][:, :, 256], wvi[e]) for e in range(4)]
                            + [(xbi[e][:, :, 256], wvr[e]) for e in range(4)]):
                        mm(pi[:], x, w[:], start=(j == 0), stop=(j == 7))
                    nc.vector.tensor_copy(vCr_sb[:], p1[:])
                    nc.scalar.copy(vCi_sb[:], pi[:])

                # transpose qk -> qkT [128e, 4u, 8] (+ negated imag)
                with tc.tile_pool(name="psQT", bufs=2, space="PSUM") as psQT:
                    for e in range(4):
                        es = slice(e * 128, (e + 1) * 128)
                        ptr = psQT.tile([128, 32], F32, tag="qtr", name=f"qtr{e}")
                        pti = psQT.tile([128, 32], F32, tag="qti", name=f"qti{e}")
                        nc.tensor.transpose(ptr[:], qk_sb_r[:, es], ident[0:32, 0:32])
                        nc.tensor.transpose(pti[:], qk_sb_i[:, es], ident[0:32, 0:32])
                        nc.scalar.copy(qkT_r[e][:], ptr[:])
                        nc.scalar.copy(qkT_i[e][:], pti[:])
                        nc.vector.tensor_scalar_mul(qkT_in[e][:], pti[:], -1.0)

                # last v batch fills PE while qkT copies land
                emit_v(3)

            # ============ per-batch pipeline: logits -> softmax -> wT -> hv
            # (hv of batch b overlaps logits of batch b+1 on PE) ============
            with tc.tile_pool(name="miscB2", bufs=1) as mb:
                # vC2m[p, b, :]: rows (re, -im); vC2s rows (im, re) -- with
                # the negation on the vC side, both hv tail matmuls can use
                # wtc_b directly (no per-batch wtc_a assembly DMAs)
                vCin_sb = mb.tile([BPC, OUT], BF16)
                nc.vector.tensor_scalar_mul(vCin_sb[:], vCi_sb[:], -1.0)
                vC2 = mb.tile([2, BPC, OUT], BF16)
                vC2s = mb.tile([2, BPC, OUT], BF16)
                nc.gpsimd.dma_start(out=vC2[0:1, :, :], in_=vCr_sb[:])
                nc.gpsimd.dma_start(out=vC2[1:2, :, :], in_=vCin_sb[:])
                nc.gpsimd.dma_start(out=vC2s[0:1, :, :], in_=vCi_sb[:])
                nc.gpsimd.dma_start(out=vC2s[1:2, :, :], in_=vCr_sb[:])

                w_b = [mb.tile([8, 2, S], F32, name=f"w_b{b}") for b in range(BPC)]
                wTr = [mb.tile([128, 32], BF16, name=f"wTr{a}") for a in range(2)]
                wTi = [mb.tile([128, 32], BF16, name=f"wTi{a}") for a in range(2)]
                wTin = [mb.tile([128, 32], BF16, name=f"wTin{a}") for a in range(2)]
                wtc_b = mb.tile([2, 32], BF16)   # rows: wTr_c, wTi_c
                hvm_r = [mb.tile([NH, OUT], BF16, name=f"hvm_r{b}") for b in range(BPC)]
                hvm_i = [mb.tile([NH, OUT], BF16, name=f"hvm_i{b}") for b in range(BPC)]
                hvm_all_r = mb.tile([32, OUT], BF16)
                hvm_all_i = mb.tile([32, OUT], BF16)

                with tc.tile_pool(name="psB2", bufs=2, space="PSUM") as psB2, \
                     tc.tile_pool(name="psB3", bufs=1, space="PSUM") as psB3, \
                     tc.tile_pool(name="psB4", bufs=2, space="PSUM") as psB4:
                    # pw[:, 0:2, :] = wT re s-halves; [:, 2:4, :] = im; ptc sep
                    pw = psB3.tile([128, 4, 32], F32, tag="pw")
                    ptc = psB3.tile([2, 32], F32, tag="ptc")

                    def softmax(b, ri, psum):
                        # logits are O(+-8): exp safe in f32 without max-shift
                        sm = mb.tile([8, 1], F32, tag="ssm", name=f"sm{b}_{ri}")
                        rs = mb.tile([8, 1], F32, tag="srs", name=f"rs{b}_{ri}")
                        nc.scalar.activation(w_b[b][:, ri, :], psum[:, 0:S],
                                             ACTF.Exp, bias=0.0, scale=1.0,
                                             accum_out=sm[:])
                        nc.vector.reciprocal(rs[:], sm[:])
                        nc.vector.tensor_scalar_mul(w_b[b][:, ri, :],
                                                    w_b[b][:, ri, :], rs[:])

                    def emit_logits(b):
                        bcols = slice(b * 8, b * 8 + 8)
                        pr = psB2.tile([8, SP], F32, tag="plg", name=f"plgr{b}")
                        for j, (q, x) in enumerate(
                                [(qkT_r[e][:, bcols], xbr[e][:, b, :])
                                 for e in range(4)]
                                + [(qkT_in[e][:, bcols], xbi[e][:, b, :])
                                   for e in range(4)]):
                            mm(pr[:], q, x, start=(j == 0), stop=(j == 7))
                        softmax(b, 0, pr)
                        pq = psB2.tile([8, SP], F32, tag="plg", name=f"plgi{b}")
                        for j, (q, x) in enumerate(
                                [(qkT_r[e][:, bcols], xbi[e][:, b, :])
                                 for e in range(4)]
                                + [(qkT_i[e][:, bcols], xbr[e][:, b, :])
                                   for e in range(4)]):
                            mm(pq[:], q, x, start=(j == 0), stop=(j == 7))
                        softmax(b, 1, pq)

                    def emit_attn(b):
                        bcols = slice(b * 8, b * 8 + 8)
                        # -- transpose w -> wT columns for this batch --
                        for a in range(2):
                            cs = slice(a * 128, (a + 1) * 128)
                            for ri in range(2):
                                nc.tensor.matmul(pw[:, 2 * ri + a, bcols],
                                                 w_b[b][:, ri, cs],
                                                 ident[0:8, 0:8],
                                                 is_transpose=True,
                                                 skip_group_check=True)
                        nc.tensor.matmul(ptc[:, bcols], w_b[b][:, :, 256],
                                         ident[0:8, 0:8], is_transpose=True,
                                         skip_group_check=True)
                        for a in range(2):
                            nc.scalar.copy(wTr[a][:, bcols], pw[:, a, bcols])
                            nc.scalar.copy(wTi[a][:, bcols], pw[:, 2 + a, bcols])
                            nc.scalar.activation(wTin[a][:, bcols],
                                                 pw[:, 2 + a, bcols],
                                                 ACTF.Copy, bias=0.0, scale=-1.0)
                        nc.scalar.copy(wtc_b[:, bcols], ptc[:, bcols])
                        # -- hv --
                        ph_r = psB4.tile([NH, OUT], F32, tag="phr", name=f"phr{b}")
                        ph_i = psB4.tile([NH, OUT], F32, tag="phi", name=f"phi{b}")
                        mm(ph_r[:], wTr[0][:, bcols], vr[b][0][:], start=True, stop=False)
                        mm(ph_r[:], wTr[1][:, bcols], vr[b][1][:], start=False, stop=False)
                        mm(ph_r[:], wTin[0][:, bcols], vi[b][0][:], start=False, stop=False)
                        mm(ph_r[:], wTin[1][:, bcols], vi[b][1][:], start=False, stop=False)
                        mm(ph_r[:], wtc_b[:, bcols], vC2[:, b, :], start=False, stop=True)
                        mm(ph_i[:], wTi[0][:, bcols], vr[b][0][:], start=True, stop=False)
                        mm(ph_i[:], wTi[1][:, bcols], vr[b][1][:], start=False, stop=False)
                        mm(ph_i[:], wTr[0][:, bcols], vi[b][0][:], start=False, stop=False)
                        mm(ph_i[:], wTr[1][:, bcols], vi[b][1][:], start=False, stop=False)
                        mm(ph_i[:], wtc_b[:, bcols], vC2s[:, b, :], start=False, stop=True)
                        nc.vector.tensor_mul(hvm_r[b][:], ph_r[:], mask8[:])
                        nc.vector.tensor_mul(hvm_i[b][:], ph_i[:], mask8[:])
                        nc.gpsimd.dma_start(out=hvm_all_r[b * 8:b * 8 + 8, :],
                                          in_=hvm_r[b][:])
                        nc.gpsimd.dma_start(out=hvm_all_i[b * 8:b * 8 + 8, :],
                                          in_=hvm_i[b][:])

                    # software-pipelined: logits of b+1 issue before attn of b
                    # so PE never waits on softmax
                    emit_logits(0)
                    emit_logits(1)
                    emit_attn(0)
                    emit_logits(2)
                    emit_attn(1)
                    emit_logits(3)
                    emit_attn(2)
                    emit_attn(3)

                # ---- extract attn0^T [128, 4] per f-tile via selection matmul ----
                att_r = [mb.tile([128, 4], BF16, name=f"att_r{u}") for u in range(4)]
                att_i = [mb.tile([128, 4], BF16, name=f"att_i{u}") for u in range(4)]
                att_in = [mb.tile([128, 4], BF16, name=f"att_in{u}") for u in range(4)]
                with tc.tile_pool(name="psB5", bufs=2, space="PSUM") as psB5:
                    for u in range(4):
                        fs = slice(u * 128, (u + 1) * 128)
                        par = psB5.tile([128, 4], F32, tag="par", name=f"par{u}")
                        pai = psB5.tile([128, 4], F32, tag="pai", name=f"pai{u}")
                        mm(par[:], hvm_all_r[:, fs], sel32[:], start=True, stop=True)
                        mm(pai[:], hvm_all_i[:, fs], sel32[:], start=True, stop=True)
                        nc.scalar.copy(att_r[u][:], par[:])
                        nc.scalar.copy(att_i[u][:], pai[:])
                        nc.scalar.activation(att_in[u][:], pai[:], ACTF.Copy,
                                             bias=0.0, scale=-1.0)

                # ---- y = attn0 @ Wc^T + b_c ----
                yr_sb = mb.tile([BPC, OUT], F32)
                yi_sb = mb.tile([BPC, OUT], F32)
                with tc.tile_pool(name="psB6", bufs=1, space="PSUM") as psB6:
                    py_r = psB6.tile([BPC, OUT], F32, tag="pyr")
                    py_i = psB6.tile([BPC, OUT], F32, tag="pyi")
                    for j, u in enumerate(range(4)):
                        mm(py_r[:], att_r[u][:], wcr[u][:], start=(j == 0), stop=False)
                        mm(py_r[:], att_in[u][:], wci[u][:], start=False, stop=(j == 3))
                        mm(py_i[:], att_r[u][:], wci[u][:], start=(j == 0), stop=False)
                        mm(py_i[:], att_i[u][:], wcr[u][:], start=False, stop=(j == 3))
                    nc.vector.tensor_add(yr_sb[:], py_r[:], bcr[:])
                    nc.vector.tensor_add(yi_sb[:], py_i[:], bci[:])
                    nc.sync.dma_start(out=d_yr.ap(), in_=yr_sb[:])
                    nc.sync.dma_start(out=d_yi.ap(), in_=yi_sb[:])

    nc.compile()
    return nc


def _host_prep(inputs):
    """Build per-core in_maps from the full inputs."""
    import ml_dtypes
    f32 = np.float32
    bf16 = ml_dtypes.bfloat16
    xr = np.ascontiguousarray(inputs["x_real"], dtype=f32).reshape(B, E, HW)
    xi = np.ascontiguousarray(inputs["x_imag"], dtype=f32).reshape(B, E, HW)
    pos = np.asarray(inputs["pos_r"], dtype=f32) + 1j * np.asarray(inputs["pos_i"], dtype=f32)
    w_in_r = np.asarray(inputs["w_in_r"], dtype=f32)
    w_in_i = np.asarray(inputs["w_in_i"], dtype=f32)
    b_in_r = np.asarray(inputs["b_in_r"], dtype=f32)
    b_in_i = np.asarray(inputs["b_in_i"], dtype=f32)
    w_out = np.asarray(inputs["w_out_r"], dtype=f32) + 1j * np.asarray(inputs["w_out_i"], dtype=f32)
    b_out = np.asarray(inputs["b_out_r"], dtype=f32) + 1j * np.asarray(inputs["b_out_i"], dtype=f32)
    w_p = np.asarray(inputs["w_p_r"], dtype=f32) + 1j * np.asarray(inputs["w_p_i"], dtype=f32)
    b_p = np.asarray(inputs["b_p_r"], dtype=f32) + 1j * np.asarray(inputs["b_p_i"], dtype=f32)

    w_in = w_in_r + 1j * w_in_i
    wq, wk, wv = w_in[:E], w_in[E:2 * E], w_in[2 * E:]
    qs = f32(1.0 / np.sqrt(HD))

    posb = np.zeros((E, SP), np.complex64)
    posb[:, :S] = pos

    wc = w_p @ w_out                                        # [OUT, E] complex
    bq = qs * (b_in_r[:E] + 1j * b_in_i[:E])                # [E]

    b_v = b_in_r[2 * E:] + 1j * b_in_i[2 * E:]
    b_c = (1 + 1j) * (b_v @ wc.T) + b_out @ w_p.T + b_p     # [OUT] complex

    mask8 = np.zeros((NH, OUT), f32)
    for h in range(NH):
        mask8[h, h * HD:(h + 1) * HD] = 1.0
    sel32 = np.zeros((32, BPC), f32)
    for b in range(BPC):
        sel32[b * 8:(b + 1) * 8, b] = 1.0

    tile4 = lambda a: np.ascontiguousarray(
        np.asarray(a, f32).reshape(4, 128, -1).transpose(1, 0, 2)).astype(bf16)
    shared = dict(
        wqr=tile4(wq.real.T * qs),
        wqi=tile4(wq.imag.T * qs),
        wkr=tile4(wk.real),
        wki=tile4(wk.imag),
        wvr=tile4(wv.real.T),
        wvi=tile4(wv.imag.T),
        wcr=tile4(wc.real.T),
        wci=tile4(wc.imag.T),
        bqr=bq.real.astype(f32).reshape(4, 128).T.copy(),
        bqi=bq.imag.astype(f32).reshape(4, 128).T.copy(),
        bcr=np.broadcast_to(b_c.real.astype(f32), (BPC, OUT)).copy(),
        bci=np.broadcast_to(b_c.imag.astype(f32), (BPC, OUT)).copy(),
        ident=np.eye(128, dtype=f32),
        mask8=mask8,
        sel32=sel32.astype(bf16),
        zbd=np.zeros((128, 32), bf16),
    )
    # x_cat fully prepped on host: col 0 = mean, then + pos; col 257 zero
    xrp = np.zeros((B, E, SP), f32)
    xip = np.zeros((B, E, SP), f32)
    xrp[:, :, 1:1 + HW] = xr
    xip[:, :, 1:1 + HW] = xi
    xrp[:, :, 0] = xr.mean(-1)
    xip[:, :, 0] = xi.mean(-1)
    xrp[:, :, :S] += posb.real[None, :, :S]
    xip[:, :, :S] += posb.imag[None, :, :S]
    in_maps = []
    for c in range(NCORES):
        m = dict(shared)
        m["xr"] = np.ascontiguousarray(
            xrp[c * BPC:(c + 1) * BPC].transpose(1, 0, 2)).astype(bf16)
        m["xi"] = np.ascontiguousarray(
            xip[c * BPC:(c + 1) * BPC].transpose(1, 0, 2)).astype(bf16)
        in_maps.append(m)
    return in_maps


def _run(inputs, trace=False, **kw):
    from concourse.bass_utils import run_bass_kernel_spmd
    if "nc" not in _cached:
        _cached["nc"] = _build()
    nc = _cached["nc"]
    in_maps = _host_prep(inputs)
    res = run_bass_kernel_spmd(nc, in_maps, core_ids=list(range(NCORES)),
                               trace=trace, **kw)
    out = np.empty((B, OUT), np.complex64)
    for c in range(NCORES):
        out[c * BPC:(c + 1) * BPC] = (res.results[c]["yr"]
                                      + 1j * res.results[c]["yi"])
    return out, res


def kernel(**inputs) -> np.ndarray:
    out, _ = _run(inputs)
    return out


# revision 30
# speedup vs baseline: 1.0036x; 1.0036x over previous
"""Complex AttentionPool2d on 8 trn2 NeuronCores, data-parallel over batch.

Contract: kernel(**inputs) takes the FULL inputs from setup_inputs() and
returns the FULL [32, 512] complex64 output.

V2: all matmuls bf16 (fp32 PSUM accum); k^T eliminated algebraically.
Math (per batch):
  x = bf16(complex(x_real, x_imag)).reshape(E, 256)
  x_cat = [mean(x), x] + pos                       # [E, 257]
  q0 = x_cat[:, 0] @ wq^T + bq                     # only query pos 0 matters
  qk[h, e] = sum_d q0[h*64+d] wk[h*64+d, e]        # fold q into k-proj
  logits[h, s] = sum_e qk[h, e] x_cat[e, s]        # == q0 . k[s]
  w = softmax(logits.re) + i*softmax(logits.im)
  v = x_cat^T @ wv^T                               # [257, 512]
  attn0 = (w @ v) per-head masked; y = attn0 @ (w_p @ w_out)^T + b_c

Sharding: batch 32 -> 4 per core.
"""
import numpy as np

B, E, HW, S = 32, 512, 256, 257
SP = 258            # S padded even
NH, HD = 8, 64
OUT = 512
NCORES = 8
BPC = B // NCORES   # batches per core

_cached = {}


def _build():
    import concourse.bacc as bacc
    import concourse.tile as tile
    import concourse.mybir as mybir

    F32 = mybir.dt.float32
    BF16 = mybir.dt.bfloat16
    AX = mybir.AxisListType
    ACTF = mybir.ActivationFunctionType

    nc = bacc.Bacc("TRN2", target_bir_lowering=False, debug=False)

    # ---- DRAM I/O ----
    # x layout: [E, BPC, SP] so one DMA per e-tile covers all 4 batches;
    # col 0 reserved for the mean token, col 257 zero pad
    d_xr = nc.dram_tensor("xr", [E, BPC, SP], BF16, kind="ExternalInput")
    d_xi = nc.dram_tensor("xi", [E, BPC, SP], BF16, kind="ExternalInput")
    d_wqr = nc.dram_tensor("wqr", [128, 4, E], BF16, kind="ExternalInput")
    d_wqi = nc.dram_tensor("wqi", [128, 4, E], BF16, kind="ExternalInput")
    d_wkr = nc.dram_tensor("wkr", [128, 4, E], BF16, kind="ExternalInput")
    d_wki = nc.dram_tensor("wki", [128, 4, E], BF16, kind="ExternalInput")
    d_wvr = nc.dram_tensor("wvr", [128, 4, OUT], BF16, kind="ExternalInput")
    d_wvi = nc.dram_tensor("wvi", [128, 4, OUT], BF16, kind="ExternalInput")
    d_wcr = nc.dram_tensor("wcr", [128, 4, OUT], BF16, kind="ExternalInput")
    d_wci = nc.dram_tensor("wci", [128, 4, OUT], BF16, kind="ExternalInput")
    d_bqr = nc.dram_tensor("bqr", [128, 4], F32, kind="ExternalInput")
    d_bqi = nc.dram_tensor("bqi", [128, 4], F32, kind="ExternalInput")
    d_bcr = nc.dram_tensor("bcr", [BPC, OUT], F32, kind="ExternalInput")
    d_bci = nc.dram_tensor("bci", [BPC, OUT], F32, kind="ExternalInput")
    d_id = nc.dram_tensor("ident", [128, 128], F32, kind="ExternalInput")
    d_mask = nc.dram_tensor("mask8", [NH, OUT], F32, kind="ExternalInput")
    d_sel = nc.dram_tensor("sel32", [32, BPC], BF16, kind="ExternalInput")
    d_zbd = nc.dram_tensor("zbd", [128, 32], BF16, kind="ExternalInput")
    d_yr = nc.dram_tensor("yr", [BPC, OUT], F32, kind="ExternalOutput")
    d_yi = nc.dram_tensor("yi", [BPC, OUT], F32, kind="ExternalOutput")

    with tile.TileContext(nc) as tc:
        with tc.tile_pool(name="consts", bufs=1) as consts, \
             tc.tile_pool(name="vpool", bufs=1) as vpool:
            # ---- persistent weights / constants (bf16) ----
            wv_r = consts.tile([128, 4, OUT], BF16)
            wv_i = consts.tile([128, 4, OUT], BF16)
            wv_in = consts.tile([128, 4, OUT], BF16)
            wq_r = consts.tile([128, 4, E], BF16)
            wq_i = consts.tile([128, 4, E], BF16)
            wk_r = consts.tile([128, 4, E], BF16)
            wk_i = consts.tile([128, 4, E], BF16)
            wc_r = consts.tile([128, 4, OUT], BF16)
            wc_i = consts.tile([128, 4, OUT], BF16)
            wvr = [wv_r[:, e, :] for e in range(4)]
            wvi = [wv_i[:, e, :] for e in range(4)]
            wvin = [wv_in[:, e, :] for e in range(4)]
            wqr = [wq_r[:, e, :] for e in range(4)]
            wqi = [wq_i[:, e, :] for e in range(4)]
            wkr = [wk_r[:, e, :] for e in range(4)]
            wki = [wk_i[:, e, :] for e in range(4)]
            wcr = [wc_r[:, e, :] for e in range(4)]
            wci = [wc_i[:, e, :] for e in range(4)]
            bqr = consts.tile([128, 4], F32)
            bqi = consts.tile([128, 4], F32)
            bqin = consts.tile([128, 4], F32)
            bcr = consts.tile([BPC, OUT], F32)
            bci = consts.tile([BPC, OUT], F32)
            ident = consts.tile([128, 128], F32)
            mask8 = consts.tile([NH, OUT], F32)
            sel32 = consts.tile([32, BPC], BF16)

            # x tiles: [128e, BPC, SP]
            xbr = [vpool.tile([128, BPC, SP], BF16, name=f"xbr{e}") for e in range(4)]
            xbi = [vpool.tile([128, BPC, SP], BF16, name=f"xbi{e}") for e in range(4)]
            x0in = [vpool.tile([128, BPC], BF16, name=f"x0in{e}") for e in range(4)]
            # v tiles live until hv
            vr = [[vpool.tile([128, OUT], BF16, name=f"vr{b}_{s}")
                   for s in range(2)] for b in range(BPC)]
            vi = [[vpool.tile([128, OUT], BF16, name=f"vi{b}_{s}")
                   for s in range(2)] for b in range(BPC)]
            vCr_sb = vpool.tile([BPC, OUT], BF16)
            vCi_sb = vpool.tile([BPC, OUT], BF16)
            # bd: per-u zero-padded block-diag q0 [128, 32] (cols b*8+2u+p)
            bd_r = [vpool.tile([128, 32], BF16, name=f"bd_r{u}") for u in range(4)]
            bd_i = [vpool.tile([128, 32], BF16, name=f"bd_i{u}") for u in range(4)]
            bd_in = [vpool.tile([128, 32], BF16, name=f"bd_in{u}") for u in range(4)]
            q0r_sb = vpool.tile([BPC, E], F32)
            q0i_sb = vpool.tile([BPC, E], F32)
            qk_sb_r = vpool.tile([32, E], F32)
            qk_sb_i = vpool.tile([32, E], F32)
            qkT_r = [vpool.tile([128, 32], BF16, name=f"qkTr{e}") for e in range(4)]
            qkT_i = [vpool.tile([128, 32], BF16, name=f"qkTi{e}") for e in range(4)]
            qkT_in = [vpool.tile([128, 32], BF16, name=f"qkTin{e}") for e in range(4)]

            # ---- DMA issue order matters per queue ----
            # sync queue: small consts then x real
            nc.sync.dma_start(out=ident, in_=d_id.ap())
            nc.sync.dma_start(out=sel32, in_=d_sel.ap())
            nc.sync.dma_start(out=mask8, in_=d_mask.ap())
            nc.sync.dma_start(out=bqr, in_=d_bqr.ap())
            nc.sync.dma_start(out=bqi, in_=d_bqi.ap())
            nc.sync.dma_start(out=bcr, in_=d_bcr.ap())
            nc.sync.dma_start(out=bci, in_=d_bci.ap())
            for u in range(4):
                nc.sync.dma_start(out=bd_r[u], in_=d_zbd.ap())
                nc.sync.dma_start(out=bd_i[u], in_=d_zbd.ap())
                nc.sync.dma_start(out=bd_in[u], in_=d_zbd.ap())
            # Two hw DMA queues (scalar, gpsimd), interleaved in first-need
            # order; weights are pre-tiled [128, 4, X] on host so every DMA
            # is contiguous per partition.
            for e in range(4):
                sl = slice(e * 128, (e + 1) * 128)
                nc.scalar.dma_start(out=xbr[e][:], in_=d_xr.ap()[sl, :, :])
                nc.scalar.dma_start(out=wv_r[:, e, :], in_=d_wvr.ap()[:, e, :])
                nc.gpsimd.dma_start(out=xbi[e][:], in_=d_xi.ap()[sl, :, :])
                nc.gpsimd.dma_start(out=wv_i[:, e, :], in_=d_wvi.ap()[:, e, :])
            nc.scalar.dma_start(out=wq_r[:], in_=d_wqr.ap())
            nc.scalar.dma_start(out=wq_i[:], in_=d_wqi.ap())
            nc.gpsimd.dma_start(out=wk_r[:], in_=d_wkr.ap())
            nc.gpsimd.dma_start(out=wk_i[:], in_=d_wki.ap())
            nc.gpsimd.dma_start(out=wc_r[:], in_=d_wcr.ap())
            nc.gpsimd.dma_start(out=wc_i[:], in_=d_wci.ap())

            nc.vector.tensor_scalar_mul(bqin, bqi, -1.0)
            # negate wv imag on device (keeps it off the DMA critical path)
            for e in range(4):
                nc.vector.tensor_scalar_mul(wv_in[:, e, :], wv_i[:, e, :], -1.0)
            # x arrives fully prepped from host (mean in col 0, pos added);
            # only the negated imag of token 0 is built on device
            for e in range(4):
                nc.scalar.activation(x0in[e][:], xbi[e][:, :, 0], ACTF.Copy,
                                     bias=0.0, scale=-1.0)

            mm = nc.tensor.matmul

            with tc.tile_pool(name="psA", bufs=2, space="PSUM") as psA:
                # v rows s in [sb*128, (sb+1)*128) = x cols (col 0 = mean tok)
                def emit_v(b):
                    for sb in range(2):
                        cs = slice(sb * 128, (sb + 1) * 128)
                        p1 = psA.tile([128, OUT], F32, tag="pv1", name=f"pv1_{b}_{sb}")
                        pi = psA.tile([128, OUT], F32, tag="pvi", name=f"pvi_{b}_{sb}")
                        for j, (x, w) in enumerate(
                                [(xbr[e][:, b, cs], wvr[e]) for e in range(4)]
                                + [(xbi[e][:, b, cs], wvin[e]) for e in range(4)]):
                            mm(p1[:], x, w[:], start=(j == 0), stop=(j == 7))
                        for j, (x, w) in enumerate(
                                [(xbr[e][:, b, cs], wvi[e]) for e in range(4)]
                                + [(xbi[e][:, b, cs], wvr[e]) for e in range(4)]):
                            mm(pi[:], x, w[:], start=(j == 0), stop=(j == 7))
                        nc.vector.tensor_copy(vr[b][sb][:], p1[:])
                        nc.scalar.copy(vi[b][sb][:], pi[:])

                emit_v(0)

                # ============ q0 -> bd ============
                with tc.tile_pool(name="psB1", bufs=1, space="PSUM") as psB1:
                    pqr = psB1.tile([BPC, E], F32, tag="pqr")
                    pqi = psB1.tile([BPC, E], F32, tag="pqi")
                    for j, (x, w) in enumerate(
                            [(xbr[e][:, :, 0], wqr[e][:]) for e in range(4)]
                            + [(x0in[e][:], wqi[e][:]) for e in range(4)]):
                        mm(pqr[:], x, w, start=(j == 0), stop=(j == 7))
                    for j, (x, w) in enumerate(
                            [(xbr[e][:, :, 0], wqi[e][:]) for e in range(4)]
                            + [(xbi[e][:, :, 0], wqr[e][:]) for e in range(4)]):
                        mm(pqi[:], x, w, start=(j == 0), stop=(j == 7))
                    nc.scalar.copy(q0r_sb[:], pqr[:])
                    nc.scalar.copy(q0i_sb[:], pqi[:])

                    # transpose q0 -> bd block-diag [128, 4u, 8]
                    # bd[p*64+d, u, 2b+p] = q0[b, u*128+p*64+d] + bq bias
                    for u in range(4):
                        fs = slice(u * 128, (u + 1) * 128)
                        ptr = psB1.tile([128, 4], F32, tag="ptq", bufs=1, name=f"ptq{u}")
                        pti = psB1.tile([128, 4], F32, tag="ptj", bufs=1, name=f"ptj{u}")
                        nc.tensor.transpose(ptr[:], q0r_sb[:, fs], ident[0:BPC, 0:BPC])
                        nc.tensor.transpose(pti[:], q0i_sb[:, fs], ident[0:BPC, 0:BPC])
                        for p in range(2):
                            rows = slice(p * 64, (p + 1) * 64)
                            cols = slice(2 * u + p, 32, 8)
                            nc.scalar.activation(bd_r[u][rows, cols], ptr[rows, :],
                                                 ACTF.Identity,
                                                 bias=bqr[rows, u:u + 1], scale=1.0)
                            nc.scalar.activation(bd_i[u][rows, cols], pti[rows, :],
                                                 ACTF.Identity,
                                                 bias=bqi[rows, u:u + 1], scale=1.0)
                            nc.scalar.activation(bd_in[u][rows, cols], pti[rows, :],
                                                 ACTF.Identity,
                                                 bias=bqin[rows, u:u + 1], scale=-1.0)

                # next v batch fills PE while bd copies run
                emit_v(1)

                # ============ qk = bd^T @ wk  [rows b*8 + 2u+p, 512e] ============
                with tc.tile_pool(name="psQK", bufs=1, space="PSUM") as psQK:
                    pkr = psQK.tile([32, E], F32, tag="pkr")
                    pki = psQK.tile([32, E], F32, tag="pki")
                    for j, (bdt, w) in enumerate(
                            [(bd_r[u], wkr[u]) for u in range(4)]
                            + [(bd_in[u], wki[u]) for u in range(4)]):
                        mm(pkr[:], bdt[:], w[:], start=(j == 0), stop=(j == 7))
                    for j, (bdt, w) in enumerate(
                            [(bd_r[u], wki[u]) for u in range(4)]
                            + [(bd_i[u], wkr[u]) for u in range(4)]):
                        mm(pki[:], bdt[:], w[:], start=(j == 0), stop=(j == 7))
                    nc.vector.tensor_copy(qk_sb_r[:], pkr[:])
                    nc.scalar.copy(qk_sb_i[:], pki[:])

                # v b2 + vC fill PE during qk copies / qkT transposes
                emit_v(2)

                # vC: token-256 v row for all batches
                with tc.tile_pool(name="psVC", bufs=1, space="PSUM") as psVC:
                    p1 = psVC.tile([BPC, OUT], F32, tag="pc1")
                    pi = psVC.tile([BPC, OUT], F32, tag="pci")
                    for j, (x, w) in enumerate(
                            [(xbr[e][:, :, 256], wvr[e]) for e in range(4)]
                            + [(xbi[e][:, :, 256], wvin[e]) for e in range(4)]):
                        mm(p1[:], x, w[:], start=(j == 0), stop=(j == 7))
                    for j, (x, w) in enumerate(
                            [(xbr[e][:, :, 256], wvi[e]) for e in range(4)]
                            + [(xbi[e][:, :, 256], wvr[e]) for e in range(4)]):
                        mm(pi[:], x, w[:], start=(j == 0), stop=(j == 7))
                    nc.vector.tensor_copy(vCr_sb[:], p1[:])
                    nc.scalar.copy(vCi_sb[:], pi[:])

                # transpose qk -> qkT [128e, 4u, 8] (+ negated imag)
                with tc.tile_pool(name="psQT", bufs=2, space="PSUM") as psQT:
                    for e in range(4):
                        es = slice(e * 128, (e + 1) * 128)
                        ptr = psQT.tile([128, 32], F32, tag="qtr", name=f"qtr{e}")
                        pti = psQT.tile([128, 32], F32, tag="qti", name=f"qti{e}")
                        nc.tensor.transpose(ptr[:], qk_sb_r[:, es], ident[0:32, 0:32])
                        nc.tensor.transpose(pti[:], qk_sb_i[:, es], ident[0:32, 0:32])
                        nc.scalar.copy(qkT_r[e][:], ptr[:])
                        nc.scalar.copy(qkT_i[e][:], pti[:])
                        nc.vector.tensor_scalar_mul(qkT_in[e][:], pti[:], -1.0)

                # last v batch fills PE while qkT copies land
                emit_v(3)

            # ============ per-batch pipeline: logits -> softmax -> wT -> hv
            # (hv of batch b overlaps logits of batch b+1 on PE) ============
            with tc.tile_pool(name="miscB2", bufs=1) as mb:
                # vC2m[p, b, :]: rows (re, -im); vC2s rows (im, re) -- with
                # the negation on the vC side, both hv tail matmuls can use
                # wtc_b directly (no per-batch wtc_a assembly DMAs)
                vCin_sb = mb.tile([BPC, OUT], BF16)
                nc.vector.tensor_scalar_mul(vCin_sb[:], vCi_sb[:], -1.0)
                vC2 = mb.tile([2, BPC, OUT], BF16)
                vC2s = mb.tile([2, BPC, OUT], BF16)
                nc.sync.dma_start(out=vC2[0:1, :, :], in_=vCr_sb[:])
                nc.sync.dma_start(out=vC2[1:2, :, :], in_=vCin_sb[:])
                nc.sync.dma_start(out=vC2s[0:1, :, :], in_=vCi_sb[:])
                nc.sync.dma_start(out=vC2s[1:2, :, :], in_=vCr_sb[:])

                w_b = [mb.tile([8, 2, S], F32, name=f"w_b{b}") for b in range(BPC)]
                wTr = [mb.tile([128, 32], BF16, name=f"wTr{a}") for a in range(2)]
                wTi = [mb.tile([128, 32], BF16, name=f"wTi{a}") for a in range(2)]
                wTin = [mb.tile([128, 32], BF16, name=f"wTin{a}") for a in range(2)]
                wtc_b = mb.tile([2, 32], BF16)   # rows: wTr_c, wTi_c
                hvm_r = [mb.tile([NH, OUT], BF16, name=f"hvm_r{b}") for b in range(BPC)]
                hvm_i = [mb.tile([NH, OUT], BF16, name=f"hvm_i{b}") for b in range(BPC)]
                hvm_all_r = mb.tile([32, OUT], BF16)
                hvm_all_i = mb.tile([32, OUT], BF16)

                with tc.tile_pool(name="psB2", bufs=2, space="PSUM") as psB2, \
                     tc.tile_pool(name="psB3", bufs=1, space="PSUM") as psB3, \
                     tc.tile_pool(name="psB4", bufs=2, space="PSUM") as psB4:
                    # pw[:, 0:2, :] = wT re s-halves; [:, 2:4, :] = im; ptc sep
                    pw = psB3.tile([128, 4, 32], F32, tag="pw")
                    ptc = psB3.tile([2, 32], F32, tag="ptc")

                    def softmax(b, ri, psum):
                        # logits are O(+-8): exp safe in f32 without max-shift
                        sm = mb.tile([8, 1], F32, tag="ssm", name=f"sm{b}_{ri}")
                        rs = mb.tile([8, 1], F32, tag="srs", name=f"rs{b}_{ri}")
                        nc.scalar.activation(w_b[b][:, ri, :], psum[:, 0:S],
                                             ACTF.Exp, bias=0.0, scale=1.0,
                                             accum_out=sm[:])
                        nc.vector.reciprocal(rs[:], sm[:])
                        nc.vector.tensor_scalar_mul(w_b[b][:, ri, :],
                                                    w_b[b][:, ri, :], rs[:])

                    def emit_logits(b):
                        bcols = slice(b * 8, b * 8 + 8)
                        pr = psB2.tile([8, SP], F32, tag="plg", name=f"plgr{b}")
                        for j, (q, x) in enumerate(
                                [(qkT_r[e][:, bcols], xbr[e][:, b, :])
                                 for e in range(4)]
                                + [(qkT_in[e][:, bcols], xbi[e][:, b, :])
                                   for e in range(4)]):
                            mm(pr[:], q, x, start=(j == 0), stop=(j == 7))
                        softmax(b, 0, pr)
                        pq = psB2.tile([8, SP], F32, tag="plg", name=f"plgi{b}")
                        for j, (q, x) in enumerate(
                                [(qkT_r[e][:, bcols], xbi[e][:, b, :])
                                 for e in range(4)]
                                + [(qkT_i[e][:, bcols], xbr[e][:, b, :])
                                   for e in range(4)]):
                            mm(pq[:], q, x, start=(j == 0), stop=(j == 7))
                        softmax(b, 1, pq)

                    def emit_attn(b):
                        bcols = slice(b * 8, b * 8 + 8)
                        # -- transpose w -> wT columns for this batch --
                        for a in range(2):
                            cs = slice(a * 128, (a + 1) * 128)
                            for ri in range(2):
                                nc.tensor.matmul(pw[:, 2 * ri + a, bcols],
                                                 w_b[b][:, ri, cs],
                                                 ident[0:8, 0:8],
                                                 is_transpose=True,
                                                 skip_group_check=True)
                        nc.tensor.matmul(ptc[:, bcols], w_b[b][:, :, 256],
                                         ident[0:8, 0:8], is_transpose=True,
                                         skip_group_check=True)
                        for a in range(2):
                            nc.scalar.copy(wTr[a][:, bcols], pw[:, a, bcols])
                            nc.scalar.copy(wTi[a][:, bcols], pw[:, 2 + a, bcols])
                            nc.scalar.activation(wTin[a][:, bcols],
                                                 pw[:, 2 + a, bcols],
                                                 ACTF.Copy, bias=0.0, scale=-1.0)
                        nc.scalar.copy(wtc_b[:, bcols], ptc[:, bcols])
                        # -- hv --
                        ph_r = psB4.tile([NH, OUT], F32, tag="phr", name=f"phr{b}")
                        ph_i = psB4.tile([NH, OUT], F32, tag="phi", name=f"phi{b}")
                        mm(ph_r[:], wTr[0][:, bcols], vr[b][0][:], start=True, stop=False)
                        mm(ph_r[:], wTr[1][:, bcols], vr[b][1][:], start=False, stop=False)
                        mm(ph_r[:], wTin[0][:, bcols], vi[b][0][:], start=False, stop=False)
                        mm(ph_r[:], wTin[1][:, bcols], vi[b][1][:], start=False, stop=False)
                        mm(ph_r[:], wtc_b[:, bcols], vC2[:, b, :], start=False, stop=True)
                        mm(ph_i[:], wTi[0][:, bcols], vr[b][0][:], start=True, stop=False)
                        mm(ph_i[:], wTi[1][:, bcols], vr[b][1][:], start=False, stop=False)
                        mm(ph_i[:], wTr[0][:, bcols], vi[b][0][:], start=False, stop=False)
                        mm(ph_i[:], wTr[1][:, bcols], vi[b][1][:], start=False, stop=False)
                        mm(ph_i[:], wtc_b[:, bcols], vC2s[:, b, :], start=False, stop=True)
                        nc.vector.tensor_mul(hvm_r[b][:], ph_r[:], mask8[:])
                        nc.vector.tensor_mul(hvm_i[b][:], ph_i[:], mask8[:])
                        nc.sync.dma_start(out=hvm_all_r[b * 8:b * 8 + 8, :],
                                          in_=hvm_r[b][:])
                        nc.sync.dma_start(out=hvm_all_i[b * 8:b * 8 + 8, :],
                                          in_=hvm_i[b][:])

                    # software-pipelined: logits of b+1 issue before attn of b
                    # so PE never waits on softmax
                    emit_logits(0)
                    emit_logits(1)
                    emit_attn(0)
                    emit_logits(2)
                    emit_attn(1)
                    emit_logits(3)
                    emit_attn(2)
                    emit_attn(3)

                # ---- extract attn0^T [128, 4] per f-tile via selection matmul ----
                att_r = [mb.tile([128, 4], BF16, name=f"att_r{u}") for u in range(4)]
                att_i = [mb.tile([128, 4], BF16, name=f"att_i{u}") for u in range(4)]
                att_in = [mb.tile([128, 4], BF16, name=f"att_in{u}") for u in range(4)]
                with tc.tile_pool(name="psB5", bufs=2, space="PSUM") as psB5:
                    for u in range(4):
                        fs = slice(u * 128, (u + 1) * 128)
                        par = psB5.tile([128, 4], F32, tag="par", name=f"par{u}")
                        pai = psB5.tile([128, 4], F32, tag="pai", name=f"pai{u}")
                        mm(par[:], hvm_all_r[:, fs], sel32[:], start=True, stop=True)
                        mm(pai[:], hvm_all_i[:, fs], sel32[:], start=True, stop=True)
                        nc.scalar.copy(att_r[u][:], par[:])
                        nc.scalar.copy(att_i[u][:], pai[:])
                        nc.scalar.activation(att_in[u][:], pai[:], ACTF.Copy,
                                             bias=0.0, scale=-1.0)

                # ---- y = attn0 @ Wc^T + b_c ----
                yr_sb = mb.tile([BPC, OUT], F32)
                yi_sb = mb.tile([BPC, OUT], F32)
                with tc.tile_pool(name="psB6", bufs=1, space="PSUM") as psB6:
                    py_r = psB6.tile([BPC, OUT], F32, tag="pyr")
                    py_i = psB6.tile([BPC, OUT], F32, tag="pyi")
                    for j, u in enumerate(range(4)):
                        mm(py_r[:], att_r[u][:], wcr[u][:], start=(j == 0), stop=False)
                        mm(py_r[:], att_in[u][:], wci[u][:], start=False, stop=(j == 3))
                        mm(py_i[:], att_r[u][:], wci[u][:], start=(j == 0), stop=False)
                        mm(py_i[:], att_i[u][:], wcr[u][:], start=False, stop=(j == 3))
                    nc.vector.tensor_add(yr_sb[:], py_r[:], bcr[:])
                    nc.vector.tensor_add(yi_sb[:], py_i[:], bci[:])
                    nc.sync.dma_start(out=d_yr.ap(), in_=yr_sb[:])
                    nc.sync.dma_start(out=d_yi.ap(), in_=yi_sb[:])

    nc.compile()
    return nc


def _host_prep(inputs):
    """Build per-core in_maps from the full inputs."""
    import ml_dtypes
    f32 = np.float32
    bf16 = ml_dtypes.bfloat16
    xr = np.ascontiguousarray(inputs["x_real"], dtype=f32).reshape(B, E, HW)
    xi = np.ascontiguousarray(inputs["x_imag"], dtype=f32).reshape(B, E, HW)
    pos = np.asarray(inputs["pos_r"], dtype=f32) + 1j * np.asarray(inputs["pos_i"], dtype=f32)
    w_in_r = np.asarray(inputs["w_in_r"], dtype=f32)
    w_in_i = np.asarray(inputs["w_in_i"], dtype=f32)
    b_in_r = np.asarray(inputs["b_in_r"], dtype=f32)
    b_in_i = np.asarray(inputs["b_in_i"], dtype=f32)
    w_out = np.asarray(inputs["w_out_r"], dtype=f32) + 1j * np.asarray(inputs["w_out_i"], dtype=f32)
    b_out = np.asarray(inputs["b_out_r"], dtype=f32) + 1j * np.asarray(inputs["b_out_i"], dtype=f32)
    w_p = np.asarray(inputs["w_p_r"], dtype=f32) + 1j * np.asarray(inputs["w_p_i"], dtype=f32)
    b_p = np.asarray(inputs["b_p_r"], dtype=f32) + 1j * np.asarray(inputs["b_p_i"], dtype=f32)

    w_in = w_in_r + 1j * w_in_i
    wq, wk, wv = w_in[:E], w_in[E:2 * E], w_in[2 * E:]
    qs = f32(1.0 / np.sqrt(HD))

    posb = np.zeros((E, SP), np.complex64)
    posb[:, :S] = pos

    wc = w_p @ w_out                                        # [OUT, E] complex
    bq = qs * (b_in_r[:E] + 1j * b_in_i[:E])                # [E]

    b_v = b_in_r[2 * E:] + 1j * b_in_i[2 * E:]
    b_c = (1 + 1j) * (b_v @ wc.T) + b_out @ w_p.T + b_p     # [OUT] complex

    mask8 = np.zeros((NH, OUT), f32)
    for h in range(NH):
        mask8[h, h * HD:(h + 1) * HD] = 1.0
    sel32 = np.zeros((32, BPC), f32)
    for b in range(BPC):
        sel32[b * 8:(b + 1) * 8, b] = 1.0

    tile4 = lambda a: np.ascontiguousarray(
        np.asarray(a, f32).reshape(4, 128, -1).transpose(1, 0, 2)).astype(bf16)
    shared = dict(
        wqr=tile4(wq.real.T * qs),
        wqi=tile4(wq.imag.T * qs),
        wkr=tile4(wk.real),
        wki=tile4(wk.imag),
        wvr=tile4(wv.real.T),
        wvi=tile4(wv.imag.T),
        wcr=tile4(wc.real.T),
        wci=tile4(wc.imag.T),
        bqr=bq.real.astype(f32).reshape(4, 128).T.copy(),
        bqi=bq.imag.astype(f32).reshape(4, 128).T.copy(),
        bcr=np.broadcast_to(b_c.real.astype(f32), (BPC, OUT)).copy(),
        bci=np.broadcast_to(b_c.imag.astype(f32), (BPC, OUT)).copy(),
        ident=np.eye(128, dtype=f32),
        mask8=mask8,
        sel32=sel32.astype(bf16),
        zbd=np.zeros((128, 32), bf16),
    )
    # x_cat fully prepped on host: col 0 = mean, then + pos; col 257 zero
    xrp = np.zeros((B, E, SP), f32)
    xip = np.zeros((B, E, SP), f32)
    xrp[:, :, 1:1 + HW] = xr
    xip[:, :, 1:1 + HW] = xi
    xrp[:, :, 0] = xr.mean(-1)
    xip[:, :, 0] = xi.mean(-1)
    xrp[:, :, :S] += posb.real[None, :, :S]
    xip[:, :, :S] += posb.imag[None, :, :S]
    in_maps = []
    for c in range(NCORES):
        m = dict(shared)
        m["xr"] = np.ascontiguousarray(
            xrp[c * BPC:(c + 1) * BPC].transpose(1, 0, 2)).astype(bf16)
        m["xi"] = np.ascontiguousarray(
            xip[c * BPC:(c + 1) * BPC].transpose(1, 0, 2)).astype(bf16)
        in_maps.append(m)
    return in_maps


def _run(inputs, trace=False, **kw):
    from concourse.bass_utils import run_bass_kernel_spmd
    if "nc" not in _cached:
        _cached["nc"] = _build()
    nc = _cached["nc"]
    in_maps = _host_prep(inputs)
    res = run_bass_kernel_spmd(nc, in_maps, core_ids=list(range(NCORES)),
                               trace=trace, **kw)
    out = np.empty((B, OUT), np.complex64)
    for c in range(NCORES):
        out[c * BPC:(c + 1) * BPC] = (res.results[c]["yr"]
                                      + 1j * res.results[c]["yi"])
    return out, res


def kernel(**inputs) -> np.ndarray:
    out, _ = _run(inputs)
    return out


# revision 31
# speedup vs baseline: 1.0163x; 1.0127x over previous
"""Complex AttentionPool2d on 8 trn2 NeuronCores, data-parallel over batch.

Contract: kernel(**inputs) takes the FULL inputs from setup_inputs() and
returns the FULL [32, 512] complex64 output.

V2: all matmuls bf16 (fp32 PSUM accum); k^T eliminated algebraically.
Math (per batch):
  x = bf16(complex(x_real, x_imag)).reshape(E, 256)
  x_cat = [mean(x), x] + pos                       # [E, 257]
  q0 = x_cat[:, 0] @ wq^T + bq                     # only query pos 0 matters
  qk[h, e] = sum_d q0[h*64+d] wk[h*64+d, e]        # fold q into k-proj
  logits[h, s] = sum_e qk[h, e] x_cat[e, s]        # == q0 . k[s]
  w = softmax(logits.re) + i*softmax(logits.im)
  v = x_cat^T @ wv^T                               # [257, 512]
  attn0 = (w @ v) per-head masked; y = attn0 @ (w_p @ w_out)^T + b_c

Sharding: batch 32 -> 4 per core.
"""
import numpy as np

B, E, HW, S = 32, 512, 256, 257
SP = 258            # S padded even
NH, HD = 8, 64
OUT = 512
NCORES = 8
BPC = B // NCORES   # batches per core

_cached = {}


def _build():
    import concourse.bacc as bacc
    import concourse.tile as tile
    import concourse.mybir as mybir

    F32 = mybir.dt.float32
    BF16 = mybir.dt.bfloat16
    AX = mybir.AxisListType
    ACTF = mybir.ActivationFunctionType

    nc = bacc.Bacc("TRN2", target_bir_lowering=False, debug=False)

    # ---- DRAM I/O ----
    # x layout: [E, BPC, SP] so one DMA per e-tile covers all 4 batches;
    # col 0 reserved for the mean token, col 257 zero pad
    d_xr = nc.dram_tensor("xr", [E, BPC, SP], BF16, kind="ExternalInput")
    d_xi = nc.dram_tensor("xi", [E, BPC, SP], BF16, kind="ExternalInput")
    d_wqr = nc.dram_tensor("wqr", [128, 4, E], BF16, kind="ExternalInput")
    d_wqi = nc.dram_tensor("wqi", [128, 4, E], BF16, kind="ExternalInput")
    d_wkr = nc.dram_tensor("wkr", [128, 4, E], BF16, kind="ExternalInput")
    d_wki = nc.dram_tensor("wki", [128, 4, E], BF16, kind="ExternalInput")
    d_wvr = nc.dram_tensor("wvr", [128, 4, OUT], BF16, kind="ExternalInput")
    d_wvi = nc.dram_tensor("wvi", [128, 4, OUT], BF16, kind="ExternalInput")
    d_wcr = nc.dram_tensor("wcr", [128, 4, OUT], BF16, kind="ExternalInput")
    d_wci = nc.dram_tensor("wci", [128, 4, OUT], BF16, kind="ExternalInput")
    d_bqr = nc.dram_tensor("bqr", [128, 4], F32, kind="ExternalInput")
    d_bqi = nc.dram_tensor("bqi", [128, 4], F32, kind="ExternalInput")
    d_bcr = nc.dram_tensor("bcr", [BPC, OUT], F32, kind="ExternalInput")
    d_bci = nc.dram_tensor("bci", [BPC, OUT], F32, kind="ExternalInput")
    d_id = nc.dram_tensor("ident", [128, 128], F32, kind="ExternalInput")
    d_mask = nc.dram_tensor("mask8", [NH, OUT], F32, kind="ExternalInput")
    d_sel = nc.dram_tensor("sel32", [32, BPC], BF16, kind="ExternalInput")
    d_zbd = nc.dram_tensor("zbd", [128, 32], BF16, kind="ExternalInput")
    d_yr = nc.dram_tensor("yr", [BPC, OUT], F32, kind="ExternalOutput")
    d_yi = nc.dram_tensor("yi", [BPC, OUT], F32, kind="ExternalOutput")

    with tile.TileContext(nc) as tc:
        with tc.tile_pool(name="consts", bufs=1) as consts, \
             tc.tile_pool(name="vpool", bufs=1) as vpool:
            # ---- persistent weights / constants (bf16) ----
            wv_r = consts.tile([128, 4, OUT], BF16)
            wv_i = consts.tile([128, 4, OUT], BF16)
            wv_in = consts.tile([128, 4, OUT], BF16)
            wq_r = consts.tile([128, 4, E], BF16)
            wq_i = consts.tile([128, 4, E], BF16)
            wk_r = consts.tile([128, 4, E], BF16)
            wk_i = consts.tile([128, 4, E], BF16)
            wc_r = consts.tile([128, 4, OUT], BF16)
            wc_i = consts.tile([128, 4, OUT], BF16)
            wvr = [wv_r[:, e, :] for e in range(4)]
            wvi = [wv_i[:, e, :] for e in range(4)]
            wvin = [wv_in[:, e, :] for e in range(4)]
            wqr = [wq_r[:, e, :] for e in range(4)]
            wqi = [wq_i[:, e, :] for e in range(4)]
            wkr = [wk_r[:, e, :] for e in range(4)]
            wki = [wk_i[:, e, :] for e in range(4)]
            wcr = [wc_r[:, e, :] for e in range(4)]
            wci = [wc_i[:, e, :] for e in range(4)]
            bqr = consts.tile([128, 4], F32)
            bqi = consts.tile([128, 4], F32)
            bqin = consts.tile([128, 4], F32)
            bcr = consts.tile([BPC, OUT], F32)
            bci = consts.tile([BPC, OUT], F32)
            ident = consts.tile([128, 128], F32)
            mask8 = consts.tile([NH, OUT], F32)
            sel32 = consts.tile([32, BPC], BF16)

            # x tiles: [128e, BPC, SP]
            xbr = [vpool.tile([128, BPC, SP], BF16, name=f"xbr{e}") for e in range(4)]
            xbi = [vpool.tile([128, BPC, SP], BF16, name=f"xbi{e}") for e in range(4)]
            x0in = [vpool.tile([128, BPC], BF16, name=f"x0in{e}") for e in range(4)]
            # v tiles live until hv
            vr = [[vpool.tile([128, OUT], BF16, name=f"vr{b}_{s}")
                   for s in range(2)] for b in range(BPC)]
            vi = [[vpool.tile([128, OUT], BF16, name=f"vi{b}_{s}")
                   for s in range(2)] for b in range(BPC)]
            vCr_sb = vpool.tile([BPC, OUT], BF16)
            vCi_sb = vpool.tile([BPC, OUT], BF16)
            # bd: per-u zero-padded block-diag q0 [128, 32] (cols b*8+2u+p)
            bd_r = [vpool.tile([128, 32], BF16, name=f"bd_r{u}") for u in range(4)]
            bd_i = [vpool.tile([128, 32], BF16, name=f"bd_i{u}") for u in range(4)]
            bd_in = [vpool.tile([128, 32], BF16, name=f"bd_in{u}") for u in range(4)]
            q0r_sb = vpool.tile([BPC, E], F32)
            q0i_sb = vpool.tile([BPC, E], F32)
            qk_sb_r = vpool.tile([32, E], F32)
            qk_sb_i = vpool.tile([32, E], F32)
            qkT_r = [vpool.tile([128, 32], BF16, name=f"qkTr{e}") for e in range(4)]
            qkT_i = [vpool.tile([128, 32], BF16, name=f"qkTi{e}") for e in range(4)]
            qkT_in = [vpool.tile([128, 32], BF16, name=f"qkTin{e}") for e in range(4)]

            # ---- DMA issue order matters per queue ----
            # sync queue: small consts then x real
            nc.sync.dma_start(out=ident, in_=d_id.ap())
            nc.sync.dma_start(out=sel32, in_=d_sel.ap())
            nc.sync.dma_start(out=mask8, in_=d_mask.ap())
            nc.sync.dma_start(out=bqr, in_=d_bqr.ap())
            nc.sync.dma_start(out=bqi, in_=d_bqi.ap())
            nc.sync.dma_start(out=bcr, in_=d_bcr.ap())
            nc.sync.dma_start(out=bci, in_=d_bci.ap())
            for u in range(4):
                nc.sync.dma_start(out=bd_r[u], in_=d_zbd.ap())
                nc.sync.dma_start(out=bd_i[u], in_=d_zbd.ap())
                nc.sync.dma_start(out=bd_in[u], in_=d_zbd.ap())
            # Two hw DMA queues (scalar, gpsimd), interleaved in first-need
            # order; weights are pre-tiled [128, 4, X] on host so every DMA
            # is contiguous per partition.
            for e in range(4):
                sl = slice(e * 128, (e + 1) * 128)
                nc.scalar.dma_start(out=xbr[e][:], in_=d_xr.ap()[sl, :, :])
                nc.scalar.dma_start(out=wv_r[:, e, :], in_=d_wvr.ap()[:, e, :])
                if e == 0:
                    nc.scalar.dma_start(out=wq_r[:, 0, :], in_=d_wqr.ap()[:, 0, :])
                nc.gpsimd.dma_start(out=xbi[e][:], in_=d_xi.ap()[sl, :, :])
                nc.gpsimd.dma_start(out=wv_i[:, e, :], in_=d_wvi.ap()[:, e, :])
            for e in range(4):
                if e > 0:
                    nc.scalar.dma_start(out=wq_r[:, e, :], in_=d_wqr.ap()[:, e, :])
                nc.scalar.dma_start(out=wq_i[:, e, :], in_=d_wqi.ap()[:, e, :])
            for e in range(4):
                nc.gpsimd.dma_start(out=wk_r[:, e, :], in_=d_wkr.ap()[:, e, :])
                nc.gpsimd.dma_start(out=wk_i[:, e, :], in_=d_wki.ap()[:, e, :])
            for e in range(4):
                nc.scalar.dma_start(out=wc_r[:, e, :], in_=d_wcr.ap()[:, e, :])
                nc.gpsimd.dma_start(out=wc_i[:, e, :], in_=d_wci.ap()[:, e, :])

            nc.vector.tensor_scalar_mul(bqin, bqi, -1.0)
            # negate wv imag on device (keeps it off the DMA critical path)
            for e in range(4):
                nc.vector.tensor_scalar_mul(wv_in[:, e, :], wv_i[:, e, :], -1.0)
            # x arrives fully prepped from host (mean in col 0, pos added);
            # only the negated imag of token 0 is built on device
            for e in range(4):
                nc.scalar.activation(x0in[e][:], xbi[e][:, :, 0], ACTF.Copy,
                                     bias=0.0, scale=-1.0)

            mm = nc.tensor.matmul

            with tc.tile_pool(name="psA", bufs=2, space="PSUM") as psA:
                # v rows s in [sb*128, (sb+1)*128) = x cols (col 0 = mean tok)
                def emit_v(b):
                    for sb in range(2):
                        cs = slice(sb * 128, (sb + 1) * 128)
                        p1 = psA.tile([128, OUT], F32, tag="pv1", name=f"pv1_{b}_{sb}")
                        pi = psA.tile([128, OUT], F32, tag="pvi", name=f"pvi_{b}_{sb}")
                        for j, (x, w) in enumerate(
                                [(xbr[e][:, b, cs], wvr[e]) for e in range(4)]
                                + [(xbi[e][:, b, cs], wvin[e]) for e in range(4)]):
                            mm(p1[:], x, w[:], start=(j == 0), stop=(j == 7))
                        for j, (x, w) in enumerate(
                                [(xbr[e][:, b, cs], wvi[e]) for e in range(4)]
                                + [(xbi[e][:, b, cs], wvr[e]) for e in range(4)]):
                            mm(pi[:], x, w[:], start=(j == 0), stop=(j == 7))
                        nc.vector.tensor_copy(vr[b][sb][:], p1[:])
                        nc.scalar.copy(vi[b][sb][:], pi[:])

                emit_v(0)

                # ============ q0 -> bd ============
                with tc.tile_pool(name="psB1", bufs=1, space="PSUM") as psB1:
                    pqr = psB1.tile([BPC, E], F32, tag="pqr")
                    pqi = psB1.tile([BPC, E], F32, tag="pqi")
                    for j, (x, w) in enumerate(
                            [(xbr[e][:, :, 0], wqr[e][:]) for e in range(4)]
                            + [(x0in[e][:], wqi[e][:]) for e in range(4)]):
                        mm(pqr[:], x, w, start=(j == 0), stop=(j == 7))
                    for j, (x, w) in enumerate(
                            [(xbr[e][:, :, 0], wqi[e][:]) for e in range(4)]
                            + [(xbi[e][:, :, 0], wqr[e][:]) for e in range(4)]):
                        mm(pqi[:], x, w, start=(j == 0), stop=(j == 7))
                    nc.scalar.copy(q0r_sb[:], pqr[:])
                    nc.scalar.copy(q0i_sb[:], pqi[:])

                    # transpose q0 -> bd block-diag [128, 4u, 8]
                    # bd[p*64+d, u, 2b+p] = q0[b, u*128+p*64+d] + bq bias
                    for u in range(4):
                        fs = slice(u * 128, (u + 1) * 128)
                        ptr = psB1.tile([128, 4], F32, tag="ptq", bufs=1, name=f"ptq{u}")
                        pti = psB1.tile([128, 4], F32, tag="ptj", bufs=1, name=f"ptj{u}")
                        nc.tensor.transpose(ptr[:], q0r_sb[:, fs], ident[0:BPC, 0:BPC])
                        nc.tensor.transpose(pti[:], q0i_sb[:, fs], ident[0:BPC, 0:BPC])
                        for p in range(2):
                            rows = slice(p * 64, (p + 1) * 64)
                            cols = slice(2 * u + p, 32, 8)
                            nc.scalar.activation(bd_r[u][rows, cols], ptr[rows, :],
                                                 ACTF.Identity,
                                                 bias=bqr[rows, u:u + 1], scale=1.0)
                            nc.scalar.activation(bd_i[u][rows, cols], pti[rows, :],
                                                 ACTF.Identity,
                                                 bias=bqi[rows, u:u + 1], scale=1.0)
                            nc.scalar.activation(bd_in[u][rows, cols], pti[rows, :],
                                                 ACTF.Identity,
                                                 bias=bqin[rows, u:u + 1], scale=-1.0)

                # next v batch fills PE while bd copies run
                emit_v(1)

                # ============ qk = bd^T @ wk  [rows b*8 + 2u+p, 512e] ============
                with tc.tile_pool(name="psQK", bufs=1, space="PSUM") as psQK:
                    pkr = psQK.tile([32, E], F32, tag="pkr")
                    pki = psQK.tile([32, E], F32, tag="pki")
                    for j, (bdt, w) in enumerate(
                            [(bd_r[u], wkr[u]) for u in range(4)]
                            + [(bd_in[u], wki[u]) for u in range(4)]):
                        mm(pkr[:], bdt[:], w[:], start=(j == 0), stop=(j == 7))
                    for j, (bdt, w) in enumerate(
                            [(bd_r[u], wki[u]) for u in range(4)]
                            + [(bd_i[u], wkr[u]) for u in range(4)]):
                        mm(pki[:], bdt[:], w[:], start=(j == 0), stop=(j == 7))
                    nc.vector.tensor_copy(qk_sb_r[:], pkr[:])
                    nc.scalar.copy(qk_sb_i[:], pki[:])

                # v b2 + vC fill PE during qk copies / qkT transposes
                emit_v(2)

                # vC: token-256 v row for all batches
                with tc.tile_pool(name="psVC", bufs=1, space="PSUM") as psVC:
                    p1 = psVC.tile([BPC, OUT], F32, tag="pc1")
                    pi = psVC.tile([BPC, OUT], F32, tag="pci")
                    for j, (x, w) in enumerate(
                            [(xbr[e][:, :, 256], wvr[e]) for e in range(4)]
                            + [(xbi[e][:, :, 256], wvin[e]) for e in range(4)]):
                        mm(p1[:], x, w[:], start=(j == 0), stop=(j == 7))
                    for j, (x, w) in enumerate(
                            [(xbr[e][:, :, 256], wvi[e]) for e in range(4)]
                            + [(xbi[e][:, :, 256], wvr[e]) for e in range(4)]):
                        mm(pi[:], x, w[:], start=(j == 0), stop=(j == 7))
                    nc.vector.tensor_copy(vCr_sb[:], p1[:])
                    nc.scalar.copy(vCi_sb[:], pi[:])

                # transpose qk -> qkT [128e, 4u, 8] (+ negated imag)
                with tc.tile_pool(name="psQT", bufs=2, space="PSUM") as psQT:
                    for e in range(4):
                        es = slice(e * 128, (e + 1) * 128)
                        ptr = psQT.tile([128, 32], F32, tag="qtr", name=f"qtr{e}")
                        pti = psQT.tile([128, 32], F32, tag="qti", name=f"qti{e}")
                        nc.tensor.transpose(ptr[:], qk_sb_r[:, es], ident[0:32, 0:32])
                        nc.tensor.transpose(pti[:], qk_sb_i[:, es], ident[0:32, 0:32])
                        nc.scalar.copy(qkT_r[e][:], ptr[:])
                        nc.scalar.copy(qkT_i[e][:], pti[:])
                        nc.vector.tensor_scalar_mul(qkT_in[e][:], pti[:], -1.0)

                # last v batch fills PE while qkT copies land
                emit_v(3)

            # ============ per-batch pipeline: logits -> softmax -> wT -> hv
            # (hv of batch b overlaps logits of batch b+1 on PE) ============
            with tc.tile_pool(name="miscB2", bufs=1) as mb:
                # vC2m[p, b, :]: rows (re, -im); vC2s rows (im, re) -- with
                # the negation on the vC side, both hv tail matmuls can use
                # wtc_b directly (no per-batch wtc_a assembly DMAs)
                vCin_sb = mb.tile([BPC, OUT], BF16)
                nc.vector.tensor_scalar_mul(vCin_sb[:], vCi_sb[:], -1.0)
                vC2 = mb.tile([2, BPC, OUT], BF16)
                vC2s = mb.tile([2, BPC, OUT], BF16)
                nc.sync.dma_start(out=vC2[0:1, :, :], in_=vCr_sb[:])
                nc.sync.dma_start(out=vC2[1:2, :, :], in_=vCin_sb[:])
                nc.sync.dma_start(out=vC2s[0:1, :, :], in_=vCi_sb[:])
                nc.sync.dma_start(out=vC2s[1:2, :, :], in_=vCr_sb[:])

                w_b = [mb.tile([8, 2, S], F32, name=f"w_b{b}") for b in range(BPC)]
                wTr = [mb.tile([128, 32], BF16, name=f"wTr{a}") for a in range(2)]
                wTi = [mb.tile([128, 32], BF16, name=f"wTi{a}") for a in range(2)]
                wTin = [mb.tile([128, 32], BF16, name=f"wTin{a}") for a in range(2)]
                wtc_b = mb.tile([2, 32], BF16)   # rows: wTr_c, wTi_c
                hvm_r = [mb.tile([NH, OUT], BF16, name=f"hvm_r{b}") for b in range(BPC)]
                hvm_i = [mb.tile([NH, OUT], BF16, name=f"hvm_i{b}") for b in range(BPC)]
                hvm_all_r = mb.tile([32, OUT], BF16)
                hvm_all_i = mb.tile([32, OUT], BF16)

                with tc.tile_pool(name="psB2", bufs=2, space="PSUM") as psB2, \
                     tc.tile_pool(name="psB3", bufs=1, space="PSUM") as psB3, \
                     tc.tile_pool(name="psB4", bufs=2, space="PSUM") as psB4:
                    # pw[:, 0:2, :] = wT re s-halves; [:, 2:4, :] = im; ptc sep
                    pw = psB3.tile([128, 4, 32], F32, tag="pw")
                    ptc = psB3.tile([2, 32], F32, tag="ptc")

                    def softmax(b, ri, psum):
                        # logits are O(+-8): exp safe in f32 without max-shift
                        sm = mb.tile([8, 1], F32, tag="ssm", name=f"sm{b}_{ri}")
                        rs = mb.tile([8, 1], F32, tag="srs", name=f"rs{b}_{ri}")
                        nc.scalar.activation(w_b[b][:, ri, :], psum[:, 0:S],
                                             ACTF.Exp, bias=0.0, scale=1.0,
                                             accum_out=sm[:])
                        nc.vector.reciprocal(rs[:], sm[:])
                        nc.vector.tensor_scalar_mul(w_b[b][:, ri, :],
                                                    w_b[b][:, ri, :], rs[:])

                    def emit_logits(b):
                        bcols = slice(b * 8, b * 8 + 8)
                        pr = psB2.tile([8, SP], F32, tag="plg", name=f"plgr{b}")
                        for j, (q, x) in enumerate(
                                [(qkT_r[e][:, bcols], xbr[e][:, b, :])
                                 for e in range(4)]
                                + [(qkT_in[e][:, bcols], xbi[e][:, b, :])
                                   for e in range(4)]):
                            mm(pr[:], q, x, start=(j == 0), stop=(j == 7))
                        softmax(b, 0, pr)
                        pq = psB2.tile([8, SP], F32, tag="plg", name=f"plgi{b}")
                        for j, (q, x) in enumerate(
                                [(qkT_r[e][:, bcols], xbi[e][:, b, :])
                                 for e in range(4)]
                                + [(qkT_i[e][:, bcols], xbr[e][:, b, :])
                                   for e in range(4)]):
                            mm(pq[:], q, x, start=(j == 0), stop=(j == 7))
                        softmax(b, 1, pq)

                    def emit_attn(b):
                        bcols = slice(b * 8, b * 8 + 8)
                        # -- transpose w -> wT columns for this batch --
                        for a in range(2):
                            cs = slice(a * 128, (a + 1) * 128)
                            for ri in range(2):
                                nc.tensor.matmul(pw[:, 2 * ri + a, bcols],
                                                 w_b[b][:, ri, cs],
                                                 ident[0:8, 0:8],
                                                 is_transpose=True,
                                                 skip_group_check=True)
                        nc.tensor.matmul(ptc[:, bcols], w_b[b][:, :, 256],
                                         ident[0:8, 0:8], is_transpose=True,
                                         skip_group_check=True)
                        for a in range(2):
                            nc.scalar.copy(wTr[a][:, bcols], pw[:, a, bcols])
                            nc.scalar.copy(wTi[a][:, bcols], pw[:, 2 + a, bcols])
                            nc.scalar.activation(wTin[a][:, bcols],
                                                 pw[:, 2 + a, bcols],
                                                 ACTF.Copy, bias=0.0, scale=-1.0)
                        nc.scalar.copy(wtc_b[:, bcols], ptc[:, bcols])
                        # -- hv --
                        ph_r = psB4.tile([NH, OUT], F32, tag="phr", name=f"phr{b}")
                        ph_i = psB4.tile([NH, OUT], F32, tag="phi", name=f"phi{b}")
                        mm(ph_r[:], wTr[0][:, bcols], vr[b][0][:], start=True, stop=False)
                        mm(ph_r[:], wTr[1][:, bcols], vr[b][1][:], start=False, stop=False)
                        mm(ph_r[:], wTin[0][:, bcols], vi[b][0][:], start=False, stop=False)
                        mm(ph_r[:], wTin[1][:, bcols], vi[b][1][:], start=False, stop=False)
                        mm(ph_r[:], wtc_b[:, bcols], vC2[:, b, :], start=False, stop=True)
                        mm(ph_i[:], wTi[0][:, bcols], vr[b][0][:], start=True, stop=False)
                        mm(ph_i[:], wTi[1][:, bcols], vr[b][1][:], start=False, stop=False)
                        mm(ph_i[:], wTr[0][:, bcols], vi[b][0][:], start=False, stop=False)
                        mm(ph_i[:], wTr[1][:, bcols], vi[b][1][:], start=False, stop=False)
                        mm(ph_i[:], wtc_b[:, bcols], vC2s[:, b, :], start=False, stop=True)
                        nc.vector.tensor_mul(hvm_r[b][:], ph_r[:], mask8[:])
                        nc.vector.tensor_mul(hvm_i[b][:], ph_i[:], mask8[:])
                        nc.sync.dma_start(out=hvm_all_r[b * 8:b * 8 + 8, :],
                                          in_=hvm_r[b][:])
                        nc.sync.dma_start(out=hvm_all_i[b * 8:b * 8 + 8, :],
                                          in_=hvm_i[b][:])

                    # software-pipelined: logits of b+1 issue before attn of b
                    # so PE never waits on softmax
                    emit_logits(0)
                    emit_logits(1)
                    emit_attn(0)
                    emit_logits(2)
                    emit_attn(1)
                    emit_logits(3)
                    emit_attn(2)
                    emit_attn(3)

                # ---- extract attn0^T [128, 4] per f-tile via selection matmul ----
                att_r = [mb.tile([128, 4], BF16, name=f"att_r{u}") for u in range(4)]
                att_i = [mb.tile([128, 4], BF16, name=f"att_i{u}") for u in range(4)]
                att_in = [mb.tile([128, 4], BF16, name=f"att_in{u}") for u in range(4)]
                with tc.tile_pool(name="psB5", bufs=2, space="PSUM") as psB5:
                    for u in range(4):
                        fs = slice(u * 128, (u + 1) * 128)
                        par = psB5.tile([128, 4], F32, tag="par", name=f"par{u}")
                        pai = psB5.tile([128, 4], F32, tag="pai", name=f"pai{u}")
                        mm(par[:], hvm_all_r[:, fs], sel32[:], start=True, stop=True)
                        mm(pai[:], hvm_all_i[:, fs], sel32[:], start=True, stop=True)
                        nc.scalar.copy(att_r[u][:], par[:])
                        nc.scalar.copy(att_i[u][:], pai[:])
                        nc.scalar.activation(att_in[u][:], pai[:], ACTF.Copy,
                                             bias=0.0, scale=-1.0)

                # ---- y = attn0 @ Wc^T + b_c ----
                yr_sb = mb.tile([BPC, OUT], F32)
                yi_sb = mb.tile([BPC, OUT], F32)
                with tc.tile_pool(name="psB6", bufs=1, space="PSUM") as psB6:
                    py_r = psB6.tile([BPC, OUT], F32, tag="pyr")
                    py_i = psB6.tile([BPC, OUT], F32, tag="pyi")
                    for j, u in enumerate(range(4)):
                        mm(py_r[:], att_r[u][:], wcr[u][:], start=(j == 0), stop=False)
                        mm(py_r[:], att_in[u][:], wci[u][:], start=False, stop=(j == 3))
                        mm(py_i[:], att_r[u][:], wci[u][:], start=(j == 0), stop=False)
                        mm(py_i[:], att_i[u][:], wcr[u][:], start=False, stop=(j == 3))
                    nc.vector.tensor_add(yr_sb[:], py_r[:], bcr[:])
                    nc.vector.tensor_add(yi_sb[:], py_i[:], bci[:])
                    nc.sync.dma_start(out=d_yr.ap(), in_=yr_sb[:])
                    nc.sync.dma_start(out=d_yi.ap(), in_=yi_sb[:])

    nc.compile()
    return nc


def _host_prep(inputs):
    """Build per-core in_maps from the full inputs."""
    import ml_dtypes
    f32 = np.float32
    bf16 = ml_dtypes.bfloat16
    xr = np.ascontiguousarray(inputs["x_real"], dtype=f32).reshape(B, E, HW)
    xi = np.ascontiguousarray(inputs["x_imag"], dtype=f32).reshape(B, E, HW)
    pos = np.asarray(inputs["pos_r"], dtype=f32) + 1j * np.asarray(inputs["pos_i"], dtype=f32)
    w_in_r = np.asarray(inputs["w_in_r"], dtype=f32)
    w_in_i = np.asarray(inputs["w_in_i"], dtype=f32)
    b_in_r = np.asarray(inputs["b_in_r"], dtype=f32)
    b_in_i = np.asarray(inputs["b_in_i"], dtype=f32)
    w_out = np.asarray(inputs["w_out_r"], dtype=f32) + 1j * np.asarray(inputs["w_out_i"], dtype=f32)
    b_out = np.asarray(inputs["b_out_r"], dtype=f32) + 1j * np.asarray(inputs["b_out_i"], dtype=f32)
    w_p = np.asarray(inputs["w_p_r"], dtype=f32) + 1j * np.asarray(inputs["w_p_i"], dtype=f32)
    b_p = np.asarray(inputs["b_p_r"], dtype=f32) + 1j * np.asarray(inputs["b_p_i"], dtype=f32)

    w_in = w_in_r + 1j * w_in_i
    wq, wk, wv = w_in[:E], w_in[E:2 * E], w_in[2 * E:]
    qs = f32(1.0 / np.sqrt(HD))

    posb = np.zeros((E, SP), np.complex64)
    posb[:, :S] = pos

    wc = w_p @ w_out                                        # [OUT, E] complex
    bq = qs * (b_in_r[:E] + 1j * b_in_i[:E])                # [E]

    b_v = b_in_r[2 * E:] + 1j * b_in_i[2 * E:]
    b_c = (1 + 1j) * (b_v @ wc.T) + b_out @ w_p.T + b_p     # [OUT] complex

    mask8 = np.zeros((NH, OUT), f32)
    for h in range(NH):
        mask8[h, h * HD:(h + 1) * HD] = 1.0
    sel32 = np.zeros((32, BPC), f32)
    for b in range(BPC):
        sel32[b * 8:(b + 1) * 8, b] = 1.0

    tile4 = lambda a: np.ascontiguousarray(
        np.asarray(a, f32).reshape(4, 128, -1).transpose(1, 0, 2)).astype(bf16)
    shared = dict(
        wqr=tile4(wq.real.T * qs),
        wqi=tile4(wq.imag.T * qs),
        wkr=tile4(wk.real),
        wki=tile4(wk.imag),
        wvr=tile4(wv.real.T),
        wvi=tile4(wv.imag.T),
        wcr=tile4(wc.real.T),
        wci=tile4(wc.imag.T),
        bqr=bq.real.astype(f32).reshape(4, 128).T.copy(),
        bqi=bq.imag.astype(f32).reshape(4, 128).T.copy(),
        bcr=np.broadcast_to(b_c.real.astype(f32), (BPC, OUT)).copy(),
        bci=np.broadcast_to(b_c.imag.astype(f32), (BPC, OUT)).copy(),
        ident=np.eye(128, dtype=f32),
        mask8=mask8,
        sel32=sel32.astype(bf16),
        zbd=np.zeros((128, 32), bf16),
    )
    # x_cat fully prepped on host: col 0 = mean, then + pos; col 257 zero
    xrp = np.zeros((B, E, SP), f32)
    xip = np.zeros((B, E, SP), f32)
    xrp[:, :, 1:1 + HW] = xr
    xip[:, :, 1:1 + HW] = xi
    xrp[:, :, 0] = xr.mean(-1)
    xip[:, :, 0] = xi.mean(-1)
    xrp[:, :, :S] += posb.real[None, :, :S]
    xip[:, :, :S] += posb.imag[None, :, :S]
    in_maps = []
    for c in range(NCORES):
        m = dict(shared)
        m["xr"] = np.ascontiguousarray(
            xrp[c * BPC:(c + 1) * BPC].transpose(1, 0, 2)).astype(bf16)
        m["xi"] = np.ascontiguousarray(
            xip[c * BPC:(c + 1) * BPC].transpose(1, 0, 2)).astype(bf16)
        in_maps.append(m)
    return in_maps


def _run(inputs, trace=False, **kw):
    from concourse.bass_utils import run_bass_kernel_spmd
    if "nc" not in _cached:
        _cached["nc"] = _build()
    nc = _cached["nc"]
    in_maps = _host_prep(inputs)
    res = run_bass_kernel_spmd(nc, in_maps, core_ids=list(range(NCORES)),
                               trace=trace, **kw)
    out = np.empty((B, OUT), np.complex64)
    for c in range(NCORES):
        out[c * BPC:(c + 1) * BPC] = (res.results[c]["yr"]
                                      + 1j * res.results[c]["yi"])
    return out, res


def kernel(**inputs) -> np.ndarray:
    out, _ = _run(inputs)
    return out


# revision 32
# speedup vs baseline: 1.0163x; 1.0000x over previous
"""Complex AttentionPool2d on 8 trn2 NeuronCores, data-parallel over batch.

Contract: kernel(**inputs) takes the FULL inputs from setup_inputs() and
returns the FULL [32, 512] complex64 output.

V2: all matmuls bf16 (fp32 PSUM accum); k^T eliminated algebraically.
Math (per batch):
  x = bf16(complex(x_real, x_imag)).reshape(E, 256)
  x_cat = [mean(x), x] + pos                       # [E, 257]
  q0 = x_cat[:, 0] @ wq^T + bq                     # only query pos 0 matters
  qk[h, e] = sum_d q0[h*64+d] wk[h*64+d, e]        # fold q into k-proj
  logits[h, s] = sum_e qk[h, e] x_cat[e, s]        # == q0 . k[s]
  w = softmax(logits.re) + i*softmax(logits.im)
  v = x_cat^T @ wv^T                               # [257, 512]
  attn0 = (w @ v) per-head masked; y = attn0 @ (w_p @ w_out)^T + b_c

Sharding: batch 32 -> 4 per core.
"""
import numpy as np

B, E, HW, S = 32, 512, 256, 257
SP = 258            # S padded even
NH, HD = 8, 64
OUT = 512
NCORES = 8
BPC = B // NCORES   # batches per core

_cached = {}


def _build():
    import concourse.bacc as bacc
    import concourse.tile as tile
    import concourse.mybir as mybir

    F32 = mybir.dt.float32
    BF16 = mybir.dt.bfloat16
    AX = mybir.AxisListType
    ACTF = mybir.ActivationFunctionType

    nc = bacc.Bacc("TRN2", target_bir_lowering=False, debug=False)

    # ---- DRAM I/O ----
    # x layout: [E, BPC, SP] so one DMA per e-tile covers all 4 batches;
    # col 0 reserved for the mean token, col 257 zero pad
    d_xr = nc.dram_tensor("xr", [E, BPC, SP], BF16, kind="ExternalInput")
    d_xi = nc.dram_tensor("xi", [E, BPC, SP], BF16, kind="ExternalInput")
    d_wqr = nc.dram_tensor("wqr", [128, 4, E], BF16, kind="ExternalInput")
    d_wqi = nc.dram_tensor("wqi", [128, 4, E], BF16, kind="ExternalInput")
    d_wkr = nc.dram_tensor("wkr", [128, 4, E], BF16, kind="ExternalInput")
    d_wki = nc.dram_tensor("wki", [128, 4, E], BF16, kind="ExternalInput")
    d_wvr = nc.dram_tensor("wvr", [128, 4, OUT], BF16, kind="ExternalInput")
    d_wvi = nc.dram_tensor("wvi", [128, 4, OUT], BF16, kind="ExternalInput")
    d_wcr = nc.dram_tensor("wcr", [128, 4, OUT], BF16, kind="ExternalInput")
    d_wci = nc.dram_tensor("wci", [128, 4, OUT], BF16, kind="ExternalInput")
    d_bqr = nc.dram_tensor("bqr", [128, 4], F32, kind="ExternalInput")
    d_bqi = nc.dram_tensor("bqi", [128, 4], F32, kind="ExternalInput")
    d_bcr = nc.dram_tensor("bcr", [BPC, OUT], F32, kind="ExternalInput")
    d_bci = nc.dram_tensor("bci", [BPC, OUT], F32, kind="ExternalInput")
    d_id = nc.dram_tensor("ident", [128, 128], F32, kind="ExternalInput")
    d_mask = nc.dram_tensor("mask8", [NH, OUT], F32, kind="ExternalInput")
    d_sel = nc.dram_tensor("sel32", [32, BPC], BF16, kind="ExternalInput")
    d_zbd = nc.dram_tensor("zbd", [128, 32], BF16, kind="ExternalInput")
    d_yr = nc.dram_tensor("yr", [BPC, OUT], F32, kind="ExternalOutput")
    d_yi = nc.dram_tensor("yi", [BPC, OUT], F32, kind="ExternalOutput")

    with tile.TileContext(nc) as tc:
        with tc.tile_pool(name="consts", bufs=1) as consts, \
             tc.tile_pool(name="vpool", bufs=1) as vpool:
            # ---- persistent weights / constants (bf16) ----
            wvr = [consts.tile([128, OUT], BF16, name=f"wvr{e}") for e in range(4)]
            wvi = [consts.tile([128, OUT], BF16, name=f"wvi{e}") for e in range(4)]
            wvin = [consts.tile([128, OUT], BF16, name=f"wvin{e}") for e in range(4)]
            wqr = [consts.tile([128, E], BF16, name=f"wqr{e}") for e in range(4)]
            wqi = [consts.tile([128, E], BF16, name=f"wqi{e}") for e in range(4)]
            wkr = [consts.tile([128, E], BF16, name=f"wkr{e}") for e in range(4)]
            wki = [consts.tile([128, E], BF16, name=f"wki{e}") for e in range(4)]
            wcr = [consts.tile([128, OUT], BF16, name=f"wcr{e}") for e in range(4)]
            wci = [consts.tile([128, OUT], BF16, name=f"wci{e}") for e in range(4)]
            bqr = consts.tile([128, 4], F32)
            bqi = consts.tile([128, 4], F32)
            bqin = consts.tile([128, 4], F32)
            bcr = consts.tile([BPC, OUT], F32)
            bci = consts.tile([BPC, OUT], F32)
            ident = consts.tile([128, 128], F32)
            mask8 = consts.tile([NH, OUT], F32)
            sel32 = consts.tile([32, BPC], BF16)

            # x tiles: [128e, BPC, SP]
            xbr = [vpool.tile([128, BPC, SP], BF16, name=f"xbr{e}") for e in range(4)]
            xbi = [vpool.tile([128, BPC, SP], BF16, name=f"xbi{e}") for e in range(4)]
            x0in = [vpool.tile([128, BPC], BF16, name=f"x0in{e}") for e in range(4)]
            # v tiles live until hv
            vr = [[vpool.tile([128, OUT], BF16, name=f"vr{b}_{s}")
                   for s in range(2)] for b in range(BPC)]
            vi = [[vpool.tile([128, OUT], BF16, name=f"vi{b}_{s}")
                   for s in range(2)] for b in range(BPC)]
            vCr_sb = vpool.tile([BPC, OUT], BF16)
            vCi_sb = vpool.tile([BPC, OUT], BF16)
            # bd: per-u zero-padded block-diag q0 [128, 32] (cols b*8+2u+p)
            bd_r = [vpool.tile([128, 32], BF16, name=f"bd_r{u}") for u in range(4)]
            bd_i = [vpool.tile([128, 32], BF16, name=f"bd_i{u}") for u in range(4)]
            bd_in = [vpool.tile([128, 32], BF16, name=f"bd_in{u}") for u in range(4)]
            q0r_sb = vpool.tile([BPC, E], F32)
            q0i_sb = vpool.tile([BPC, E], F32)
            qk_sb_r = vpool.tile([32, E], F32)
            qk_sb_i = vpool.tile([32, E], F32)
            qkT_r = [vpool.tile([128, 32], BF16, name=f"qkTr{e}") for e in range(4)]
            qkT_i = [vpool.tile([128, 32], BF16, name=f"qkTi{e}") for e in range(4)]
            qkT_in = [vpool.tile([128, 32], BF16, name=f"qkTin{e}") for e in range(4)]

            # ---- DMA issue order matters per queue ----
            # sync queue: small consts then x real
            nc.sync.dma_start(out=ident, in_=d_id.ap())
            nc.sync.dma_start(out=sel32, in_=d_sel.ap())
            nc.sync.dma_start(out=mask8, in_=d_mask.ap())
            nc.sync.dma_start(out=bqr, in_=d_bqr.ap())
            nc.sync.dma_start(out=bqi, in_=d_bqi.ap())
            nc.sync.dma_start(out=bcr, in_=d_bcr.ap())
            nc.sync.dma_start(out=bci, in_=d_bci.ap())
            for u in range(4):
                nc.sync.dma_start(out=bd_r[u], in_=d_zbd.ap())
                nc.sync.dma_start(out=bd_i[u], in_=d_zbd.ap())
                nc.sync.dma_start(out=bd_in[u], in_=d_zbd.ap())
            # Two hw DMA queues (scalar, gpsimd), interleaved in first-need
            # order; weights are pre-tiled [128, 4, X] on host so every DMA
            # is contiguous per partition.
            for e in range(4):
                sl = slice(e * 128, (e + 1) * 128)
                nc.scalar.dma_start(out=xbr[e][:], in_=d_xr.ap()[sl, :, :])
                nc.scalar.dma_start(out=wvr[e], in_=d_wvr.ap()[:, e, :])
                if e == 0:
                    nc.scalar.dma_start(out=wqr[0], in_=d_wqr.ap()[:, 0, :])
                nc.gpsimd.dma_start(out=xbi[e][:], in_=d_xi.ap()[sl, :, :])
                nc.gpsimd.dma_start(out=wvi[e], in_=d_wvi.ap()[:, e, :])
            for e in range(4):
                if e > 0:
                    nc.scalar.dma_start(out=wqr[e], in_=d_wqr.ap()[:, e, :])
                nc.scalar.dma_start(out=wqi[e], in_=d_wqi.ap()[:, e, :])
            for e in range(4):
                nc.gpsimd.dma_start(out=wkr[e], in_=d_wkr.ap()[:, e, :])
                nc.gpsimd.dma_start(out=wki[e], in_=d_wki.ap()[:, e, :])
            for e in range(4):
                nc.scalar.dma_start(out=wcr[e], in_=d_wcr.ap()[:, e, :])
                nc.gpsimd.dma_start(out=wci[e], in_=d_wci.ap()[:, e, :])

            nc.vector.tensor_scalar_mul(bqin, bqi, -1.0)
            # negate wv imag on device (keeps it off the DMA critical path)
            for e in range(4):
                nc.vector.tensor_scalar_mul(wvin[e][:], wvi[e][:], -1.0)
            # x arrives fully prepped from host (mean in col 0, pos added);
            # only the negated imag of token 0 is built on device
            for e in range(4):
                nc.scalar.activation(x0in[e][:], xbi[e][:, :, 0], ACTF.Copy,
                                     bias=0.0, scale=-1.0)

            mm = nc.tensor.matmul

            with tc.tile_pool(name="psA", bufs=2, space="PSUM") as psA:
                # v rows s in [sb*128, (sb+1)*128) = x cols (col 0 = mean tok)
                def emit_v(b):
                    for sb in range(2):
                        cs = slice(sb * 128, (sb + 1) * 128)
                        p1 = psA.tile([128, OUT], F32, tag="pv1", name=f"pv1_{b}_{sb}")
                        pi = psA.tile([128, OUT], F32, tag="pvi", name=f"pvi_{b}_{sb}")
                        for j, (x, w) in enumerate(
                                [(xbr[e][:, b, cs], wvr[e]) for e in range(4)]
                                + [(xbi[e][:, b, cs], wvin[e]) for e in range(4)]):
                            mm(p1[:], x, w[:], start=(j == 0), stop=(j == 7))
                        for j, (x, w) in enumerate(
                                [(xbr[e][:, b, cs], wvi[e]) for e in range(4)]
                                + [(xbi[e][:, b, cs], wvr[e]) for e in range(4)]):
                            mm(pi[:], x, w[:], start=(j == 0), stop=(j == 7))
                        nc.vector.tensor_copy(vr[b][sb][:], p1[:])
                        nc.scalar.copy(vi[b][sb][:], pi[:])

                emit_v(0)

                # ============ q0 -> bd ============
                with tc.tile_pool(name="psB1", bufs=1, space="PSUM") as psB1:
                    pqr = psB1.tile([BPC, E], F32, tag="pqr")
                    pqi = psB1.tile([BPC, E], F32, tag="pqi")
                    for j, (x, w) in enumerate(
                            [(xbr[e][:, :, 0], wqr[e][:]) for e in range(4)]
                            + [(x0in[e][:], wqi[e][:]) for e in range(4)]):
                        mm(pqr[:], x, w, start=(j == 0), stop=(j == 7))
                    for j, (x, w) in enumerate(
                            [(xbr[e][:, :, 0], wqi[e][:]) for e in range(4)]
                            + [(xbi[e][:, :, 0], wqr[e][:]) for e in range(4)]):
                        mm(pqi[:], x, w, start=(j == 0), stop=(j == 7))
                    nc.scalar.copy(q0r_sb[:], pqr[:])
                    nc.scalar.copy(q0i_sb[:], pqi[:])

                    # transpose q0 -> bd block-diag [128, 4u, 8]
                    # bd[p*64+d, u, 2b+p] = q0[b, u*128+p*64+d] + bq bias
                    for u in range(4):
                        fs = slice(u * 128, (u + 1) * 128)
                        ptr = psB1.tile([128, 4], F32, tag="ptq", bufs=1, name=f"ptq{u}")
                        pti = psB1.tile([128, 4], F32, tag="ptj", bufs=1, name=f"ptj{u}")
                        nc.tensor.transpose(ptr[:], q0r_sb[:, fs], ident[0:BPC, 0:BPC])
                        nc.tensor.transpose(pti[:], q0i_sb[:, fs], ident[0:BPC, 0:BPC])
                        for p in range(2):
                            rows = slice(p * 64, (p + 1) * 64)
                            cols = slice(2 * u + p, 32, 8)
                            nc.scalar.activation(bd_r[u][rows, cols], ptr[rows, :],
                                                 ACTF.Identity,
                                                 bias=bqr[rows, u:u + 1], scale=1.0)
                            nc.scalar.activation(bd_i[u][rows, cols], pti[rows, :],
                                                 ACTF.Identity,
                                                 bias=bqi[rows, u:u + 1], scale=1.0)
                            nc.scalar.activation(bd_in[u][rows, cols], pti[rows, :],
                                                 ACTF.Identity,
                                                 bias=bqin[rows, u:u + 1], scale=-1.0)

                # next v batch fills PE while bd copies run
                emit_v(1)

                # ============ qk = bd^T @ wk  [rows b*8 + 2u+p, 512e] ============
                with tc.tile_pool(name="psQK", bufs=1, space="PSUM") as psQK:
                    pkr = psQK.tile([32, E], F32, tag="pkr")
                    pki = psQK.tile([32, E], F32, tag="pki")
                    for j, (bdt, w) in enumerate(
                            [(bd_r[u], wkr[u]) for u in range(4)]
                            + [(bd_in[u], wki[u]) for u in range(4)]):
                        mm(pkr[:], bdt[:], w[:], start=(j == 0), stop=(j == 7))
                    for j, (bdt, w) in enumerate(
                            [(bd_r[u], wki[u]) for u in range(4)]
                            + [(bd_i[u], wkr[u]) for u in range(4)]):
                        mm(pki[:], bdt[:], w[:], start=(j == 0), stop=(j == 7))
                    nc.vector.tensor_copy(qk_sb_r[:], pkr[:])
                    nc.scalar.copy(qk_sb_i[:], pki[:])

                # v b2 + vC fill PE during qk copies / qkT transposes
                emit_v(2)

                # vC: token-256 v row for all batches
                with tc.tile_pool(name="psVC", bufs=1, space="PSUM") as psVC:
                    p1 = psVC.tile([BPC, OUT], F32, tag="pc1")
                    pi = psVC.tile([BPC, OUT], F32, tag="pci")
                    for j, (x, w) in enumerate(
                            [(xbr[e][:, :, 256], wvr[e]) for e in range(4)]
                            + [(xbi[e][:, :, 256], wvin[e]) for e in range(4)]):
                        mm(p1[:], x, w[:], start=(j == 0), stop=(j == 7))
                    for j, (x, w) in enumerate(
                            [(xbr[e][:, :, 256], wvi[e]) for e in range(4)]
                            + [(xbi[e][:, :, 256], wvr[e]) for e in range(4)]):
                        mm(pi[:], x, w[:], start=(j == 0), stop=(j == 7))
                    nc.vector.tensor_copy(vCr_sb[:], p1[:])
                    nc.scalar.copy(vCi_sb[:], pi[:])

                # transpose qk -> qkT [128e, 4u, 8] (+ negated imag)
                with tc.tile_pool(name="psQT", bufs=2, space="PSUM") as psQT:
                    for e in range(4):
                        es = slice(e * 128, (e + 1) * 128)
                        ptr = psQT.tile([128, 32], F32, tag="qtr", name=f"qtr{e}")
                        pti = psQT.tile([128, 32], F32, tag="qti", name=f"qti{e}")
                        nc.tensor.transpose(ptr[:], qk_sb_r[:, es], ident[0:32, 0:32])
                        nc.tensor.transpose(pti[:], qk_sb_i[:, es], ident[0:32, 0:32])
                        nc.scalar.copy(qkT_r[e][:], ptr[:])
                        nc.scalar.copy(qkT_i[e][:], pti[:])
                        nc.vector.tensor_scalar_mul(qkT_in[e][:], pti[:], -1.0)

                # last v batch fills PE while qkT copies land
                emit_v(3)

            # ============ per-batch pipeline: logits -> softmax -> wT -> hv
            # (hv of batch b overlaps logits of batch b+1 on PE) ============
            with tc.tile_pool(name="miscB2", bufs=1) as mb:
                # vC2m[p, b, :]: rows (re, -im); vC2s rows (im, re) -- with
                # the negation on the vC side, both hv tail matmuls can use
                # wtc_b directly (no per-batch wtc_a assembly DMAs)
                vCin_sb = mb.tile([BPC, OUT], BF16)
                nc.vector.tensor_scalar_mul(vCin_sb[:], vCi_sb[:], -1.0)
                vC2 = mb.tile([2, BPC, OUT], BF16)
                vC2s = mb.tile([2, BPC, OUT], BF16)
                nc.sync.dma_start(out=vC2[0:1, :, :], in_=vCr_sb[:])
                nc.sync.dma_start(out=vC2[1:2, :, :], in_=vCin_sb[:])
                nc.sync.dma_start(out=vC2s[0:1, :, :], in_=vCi_sb[:])
                nc.sync.dma_start(out=vC2s[1:2, :, :], in_=vCr_sb[:])

                w_b = [mb.tile([8, 2, S], F32, name=f"w_b{b}") for b in range(BPC)]
                wTr = [mb.tile([128, 32], BF16, name=f"wTr{a}") for a in range(2)]
                wTi = [mb.tile([128, 32], BF16, name=f"wTi{a}") for a in range(2)]
                wTin = [mb.tile([128, 32], BF16, name=f"wTin{a}") for a in range(2)]
                wtc_b = mb.tile([2, 32], BF16)   # rows: wTr_c, wTi_c
                hvm_r = [mb.tile([NH, OUT], BF16, name=f"hvm_r{b}") for b in range(BPC)]
                hvm_i = [mb.tile([NH, OUT], BF16, name=f"hvm_i{b}") for b in range(BPC)]
                hvm_all_r = mb.tile([32, OUT], BF16)
                hvm_all_i = mb.tile([32, OUT], BF16)

                with tc.tile_pool(name="psB2", bufs=2, space="PSUM") as psB2, \
                     tc.tile_pool(name="psB3", bufs=1, space="PSUM") as psB3, \
                     tc.tile_pool(name="psB4", bufs=2, space="PSUM") as psB4:
                    # pw[:, 0:2, :] = wT re s-halves; [:, 2:4, :] = im; ptc sep
                    pw = psB3.tile([128, 4, 32], F32, tag="pw")
                    ptc = psB3.tile([2, 32], F32, tag="ptc")

                    def softmax(b, ri, psum):
                        # logits are O(+-8): exp safe in f32 without max-shift
                        sm = mb.tile([8, 1], F32, tag="ssm", name=f"sm{b}_{ri}")
                        rs = mb.tile([8, 1], F32, tag="srs", name=f"rs{b}_{ri}")
                        nc.scalar.activation(w_b[b][:, ri, :], psum[:, 0:S],
                                             ACTF.Exp, bias=0.0, scale=1.0,
                                             accum_out=sm[:])
                        nc.vector.reciprocal(rs[:], sm[:])
                        nc.vector.tensor_scalar_mul(w_b[b][:, ri, :],
                                                    w_b[b][:, ri, :], rs[:])

                    def emit_logits(b):
                        bcols = slice(b * 8, b * 8 + 8)
                        pr = psB2.tile([8, SP], F32, tag="plg", name=f"plgr{b}")
                        for j, (q, x) in enumerate(
                                [(qkT_r[e][:, bcols], xbr[e][:, b, :])
                                 for e in range(4)]
                                + [(qkT_in[e][:, bcols], xbi[e][:, b, :])
                                   for e in range(4)]):
                            mm(pr[:], q, x, start=(j == 0), stop=(j == 7))
                        softmax(b, 0, pr)
                        pq = psB2.tile([8, SP], F32, tag="plg", name=f"plgi{b}")
                        for j, (q, x) in enumerate(
                                [(qkT_r[e][:, bcols], xbi[e][:, b, :])
                                 for e in range(4)]
                                + [(qkT_i[e][:, bcols], xbr[e][:, b, :])
                                   for e in range(4)]):
                            mm(pq[:], q, x, start=(j == 0), stop=(j == 7))
                        softmax(b, 1, pq)

                    def emit_attn(b):
                        bcols = slice(b * 8, b * 8 + 8)
                        # -- transpose w -> wT columns for this batch --
                        for a in range(2):
                            cs = slice(a * 128, (a + 1) * 128)
                            for ri in range(2):
                                nc.tensor.matmul(pw[:, 2 * ri + a, bcols],
                                                 w_b[b][:, ri, cs],
                                                 ident[0:8, 0:8],
                                                 is_transpose=True,
                                                 skip_group_check=True)
                        nc.tensor.matmul(ptc[:, bcols], w_b[b][:, :, 256],
                                         ident[0:8, 0:8], is_transpose=True,
                                         skip_group_check=True)
                        for a in range(2):
                            nc.scalar.copy(wTr[a][:, bcols], pw[:, a, bcols])
                            nc.scalar.copy(wTi[a][:, bcols], pw[:, 2 + a, bcols])
                            nc.scalar.activation(wTin[a][:, bcols],
                                                 pw[:, 2 + a, bcols],
                                                 ACTF.Copy, bias=0.0, scale=-1.0)
                        nc.scalar.copy(wtc_b[:, bcols], ptc[:, bcols])
                        # -- hv --
                        ph_r = psB4.tile([NH, OUT], F32, tag="phr", name=f"phr{b}")
                        ph_i = psB4.tile([NH, OUT], F32, tag="phi", name=f"phi{b}")
                        mm(ph_r[:], wTr[0][:, bcols], vr[b][0][:], start=True, stop=False)
                        mm(ph_r[:], wTr[1][:, bcols], vr[b][1][:], start=False, stop=False)
                        mm(ph_r[:], wTin[0][:, bcols], vi[b][0][:], start=False, stop=False)
                        mm(ph_r[:], wTin[1][:, bcols], vi[b][1][:], start=False, stop=False)
                        mm(ph_r[:], wtc_b[:, bcols], vC2[:, b, :], start=False, stop=True)
                        mm(ph_i[:], wTi[0][:, bcols], vr[b][0][:], start=True, stop=False)
                        mm(ph_i[:], wTi[1][:, bcols], vr[b][1][:], start=False, stop=False)
                        mm(ph_i[:], wTr[0][:, bcols], vi[b][0][:], start=False, stop=False)
                        mm(ph_i[:], wTr[1][:, bcols], vi[b][1][:], start=False, stop=False)
                        mm(ph_i[:], wtc_b[:, bcols], vC2s[:, b, :], start=False, stop=True)
                        nc.vector.tensor_mul(hvm_r[b][:], ph_r[:], mask8[:])
                        nc.vector.tensor_mul(hvm_i[b][:], ph_i[:], mask8[:])
                        nc.sync.dma_start(out=hvm_all_r[b * 8:b * 8 + 8, :],
                                          in_=hvm_r[b][:])
                        nc.sync.dma_start(out=hvm_all_i[b * 8:b * 8 + 8, :],
                                          in_=hvm_i[b][:])

                    # software-pipelined: logits of b+1 issue before attn of b
                    # so PE never waits on softmax
                    emit_logits(0)
                    emit_logits(1)
                    emit_attn(0)
                    emit_logits(2)
                    emit_attn(1)
                    emit_logits(3)
                    emit_attn(2)
                    emit_attn(3)

                # ---- extract attn0^T [128, 4] per f-tile via selection matmul ----
                att_r = [mb.tile([128, 4], BF16, name=f"att_r{u}") for u in range(4)]
                att_i = [mb.tile([128, 4], BF16, name=f"att_i{u}") for u in range(4)]
                att_in = [mb.tile([128, 4], BF16, name=f"att_in{u}") for u in range(4)]
                with tc.tile_pool(name="psB5", bufs=2, space="PSUM") as psB5:
                    for u in range(4):
                        fs = slice(u * 128, (u + 1) * 128)
                        par = psB5.tile([128, 4], F32, tag="par", name=f"par{u}")
                        pai = psB5.tile([128, 4], F32, tag="pai", name=f"pai{u}")
                        mm(par[:], hvm_all_r[:, fs], sel32[:], start=True, stop=True)
                        mm(pai[:], hvm_all_i[:, fs], sel32[:], start=True, stop=True)
                        nc.scalar.copy(att_r[u][:], par[:])
                        nc.scalar.copy(att_i[u][:], pai[:])
                        nc.scalar.activation(att_in[u][:], pai[:], ACTF.Copy,
                                             bias=0.0, scale=-1.0)

                # ---- y = attn0 @ Wc^T + b_c ----
                yr_sb = mb.tile([BPC, OUT], F32)
                yi_sb = mb.tile([BPC, OUT], F32)
                with tc.tile_pool(name="psB6", bufs=1, space="PSUM") as psB6:
                    py_r = psB6.tile([BPC, OUT], F32, tag="pyr")
                    py_i = psB6.tile([BPC, OUT], F32, tag="pyi")
                    for j, u in enumerate(range(4)):
                        mm(py_r[:], att_r[u][:], wcr[u][:], start=(j == 0), stop=False)
                        mm(py_r[:], att_in[u][:], wci[u][:], start=False, stop=(j == 3))
                        mm(py_i[:], att_r[u][:], wci[u][:], start=(j == 0), stop=False)
                        mm(py_i[:], att_i[u][:], wcr[u][:], start=False, stop=(j == 3))
                    nc.vector.tensor_add(yr_sb[:], py_r[:], bcr[:])
                    nc.vector.tensor_add(yi_sb[:], py_i[:], bci[:])
                    nc.sync.dma_start(out=d_yr.ap(), in_=yr_sb[:])
                    nc.sync.dma_start(out=d_yi.ap(), in_=yi_sb[:])

    nc.compile()
    return nc


def _host_prep(inputs):
    """Build per-core in_maps from the full inputs."""
    import ml_dtypes
    f32 = np.float32
    bf16 = ml_dtypes.bfloat16
    xr = np.ascontiguousarray(inputs["x_real"], dtype=f32).reshape(B, E, HW)
    xi = np.ascontiguousarray(inputs["x_imag"], dtype=f32).reshape(B, E, HW)
    pos = np.asarray(inputs["pos_r"], dtype=f32) + 1j * np.asarray(inputs["pos_i"], dtype=f32)
    w_in_r = np.asarray(inputs["w_in_r"], dtype=f32)
    w_in_i = np.asarray(inputs["w_in_i"], dtype=f32)
    b_in_r = np.asarray(inputs["b_in_r"], dtype=f32)
    b_in_i = np.asarray(inputs["b_in_i"], dtype=f32)
    w_out = np.asarray(inputs["w_out_r"], dtype=f32) + 1j * np.asarray(inputs["w_out_i"], dtype=f32)
    b_out = np.asarray(inputs["b_out_r"], dtype=f32) + 1j * np.asarray(inputs["b_out_i"], dtype=f32)
    w_p = np.asarray(inputs["w_p_r"], dtype=f32) + 1j * np.asarray(inputs["w_p_i"], dtype=f32)
    b_p = np.asarray(inputs["b_p_r"], dtype=f32) + 1j * np.asarray(inputs["b_p_i"], dtype=f32)

    w_in = w_in_r + 1j * w_in_i
    wq, wk, wv = w_in[:E], w_in[E:2 * E], w_in[2 * E:]
    qs = f32(1.0 / np.sqrt(HD))

    posb = np.zeros((E, SP), np.complex64)
    posb[:, :S] = pos

    wc = w_p @ w_out                                        # [OUT, E] complex
    bq = qs * (b_in_r[:E] + 1j * b_in_i[:E])                # [E]

    b_v = b_in_r[2 * E:] + 1j * b_in_i[2 * E:]
    b_c = (1 + 1j) * (b_v @ wc.T) + b_out @ w_p.T + b_p     # [OUT] complex

    mask8 = np.zeros((NH, OUT), f32)
    for h in range(NH):
        mask8[h, h * HD:(h + 1) * HD] = 1.0
    sel32 = np.zeros((32, BPC), f32)
    for b in range(BPC):
        sel32[b * 8:(b + 1) * 8, b] = 1.0

    tile4 = lambda a: np.ascontiguousarray(
        np.asarray(a, f32).reshape(4, 128, -1).transpose(1, 0, 2)).astype(bf16)
    shared = dict(
        wqr=tile4(wq.real.T * qs),
        wqi=tile4(wq.imag.T * qs),
        wkr=tile4(wk.real),
        wki=tile4(wk.imag),
        wvr=tile4(wv.real.T),
        wvi=tile4(wv.imag.T),
        wcr=tile4(wc.real.T),
        wci=tile4(wc.imag.T),
        bqr=bq.real.astype(f32).reshape(4, 128).T.copy(),
        bqi=bq.imag.astype(f32).reshape(4, 128).T.copy(),
        bcr=np.broadcast_to(b_c.real.astype(f32), (BPC, OUT)).copy(),
        bci=np.broadcast_to(b_c.imag.astype(f32), (BPC, OUT)).copy(),
        ident=np.eye(128, dtype=f32),
        mask8=mask8,
        sel32=sel32.astype(bf16),
        zbd=np.zeros((128, 32), bf16),
    )
    # x_cat fully prepped on host: col 0 = mean, then + pos; col 257 zero
    xrp = np.zeros((B, E, SP), f32)
    xip = np.zeros((B, E, SP), f32)
    xrp[:, :, 1:1 + HW] = xr
    xip[:, :, 1:1 + HW] = xi
    xrp[:, :, 0] = xr.mean(-1)
    xip[:, :, 0] = xi.mean(-1)
    xrp[:, :, :S] += posb.real[None, :, :S]
    xip[:, :, :S] += posb.imag[None, :, :S]
    in_maps = []
    for c in range(NCORES):
        m = dict(shared)
        m["xr"] = np.ascontiguousarray(
            xrp[c * BPC:(c + 1) * BPC].transpose(1, 0, 2)).astype(bf16)
        m["xi"] = np.ascontiguousarray(
            xip[c * BPC:(c + 1) * BPC].transpose(1, 0, 2)).astype(bf16)
        in_maps.append(m)
    return in_maps


def _run(inputs, trace=False, **kw):
    from concourse.bass_utils import run_bass_kernel_spmd
    if "nc" not in _cached:
        _cached["nc"] = _build()
    nc = _cached["nc"]
    in_maps = _host_prep(inputs)
    res = run_bass_kernel_spmd(nc, in_maps, core_ids=list(range(NCORES)),
                               trace=trace, **kw)
    out = np.empty((B, OUT), np.complex64)
    for c in range(NCORES):
        out[c * BPC:(c + 1) * BPC] = (res.results[c]["yr"]
                                      + 1j * res.results[c]["yi"])
    return out, res


def kernel(**inputs) -> np.ndarray:
    out, _ = _run(inputs)
    return out


# revision 33
# speedup vs baseline: 1.0231x; 1.0067x over previous
"""Complex AttentionPool2d on 8 trn2 NeuronCores, data-parallel over batch.

Contract: kernel(**inputs) takes the FULL inputs from setup_inputs() and
returns the FULL [32, 512] complex64 output.

V2: all matmuls bf16 (fp32 PSUM accum); k^T eliminated algebraically.
Math (per batch):
  x = bf16(complex(x_real, x_imag)).reshape(E, 256)
  x_cat = [mean(x), x] + pos                       # [E, 257]
  q0 = x_cat[:, 0] @ wq^T + bq                     # only query pos 0 matters
  qk[h, e] = sum_d q0[h*64+d] wk[h*64+d, e]        # fold q into k-proj
  logits[h, s] = sum_e qk[h, e] x_cat[e, s]        # == q0 . k[s]
  w = softmax(logits.re) + i*softmax(logits.im)
  v = x_cat^T @ wv^T                               # [257, 512]
  attn0 = (w @ v) per-head masked; y = attn0 @ (w_p @ w_out)^T + b_c

Sharding: batch 32 -> 4 per core.
"""
import numpy as np

B, E, HW, S = 32, 512, 256, 257
SP = 258            # S padded even
NH, HD = 8, 64
OUT = 512
NCORES = 8
BPC = B // NCORES   # batches per core

_cached = {}


def _build():
    import concourse.bacc as bacc
    import concourse.tile as tile
    import concourse.mybir as mybir

    F32 = mybir.dt.float32
    BF16 = mybir.dt.bfloat16
    AX = mybir.AxisListType
    ACTF = mybir.ActivationFunctionType

    nc = bacc.Bacc("TRN2", target_bir_lowering=False, debug=False)

    # ---- DRAM I/O ----
    # x layout: [E, BPC, SP] so one DMA per e-tile covers all 4 batches;
    # col 0 reserved for the mean token, col 257 zero pad
    d_xr = nc.dram_tensor("xr", [E, BPC, SP], BF16, kind="ExternalInput")
    d_xi = nc.dram_tensor("xi", [E, BPC, SP], BF16, kind="ExternalInput")
    d_wqr = nc.dram_tensor("wqr", [128, 4, E], BF16, kind="ExternalInput")
    d_wqi = nc.dram_tensor("wqi", [128, 4, E], BF16, kind="ExternalInput")
    d_wkr = nc.dram_tensor("wkr", [128, 4, E], BF16, kind="ExternalInput")
    d_wki = nc.dram_tensor("wki", [128, 4, E], BF16, kind="ExternalInput")
    d_wvr = nc.dram_tensor("wvr", [128, 4, OUT], BF16, kind="ExternalInput")
    d_wvi = nc.dram_tensor("wvi", [128, 4, OUT], BF16, kind="ExternalInput")
    d_wcr = nc.dram_tensor("wcr", [128, 4, OUT], BF16, kind="ExternalInput")
    d_wci = nc.dram_tensor("wci", [128, 4, OUT], BF16, kind="ExternalInput")
    d_bqr = nc.dram_tensor("bqr", [128, 4], F32, kind="ExternalInput")
    d_bqi = nc.dram_tensor("bqi", [128, 4], F32, kind="ExternalInput")
    d_bcr = nc.dram_tensor("bcr", [BPC, OUT], F32, kind="ExternalInput")
    d_bci = nc.dram_tensor("bci", [BPC, OUT], F32, kind="ExternalInput")
    d_id = nc.dram_tensor("ident", [128, 128], F32, kind="ExternalInput")
    d_mask = nc.dram_tensor("mask8", [NH, OUT], F32, kind="ExternalInput")
    d_sel = nc.dram_tensor("sel32", [32, BPC], BF16, kind="ExternalInput")
    d_zbd = nc.dram_tensor("zbd", [128, 32], BF16, kind="ExternalInput")
    d_yr = nc.dram_tensor("yr", [BPC, OUT], F32, kind="ExternalOutput")
    d_yi = nc.dram_tensor("yi", [BPC, OUT], F32, kind="ExternalOutput")

    with tile.TileContext(nc) as tc:
        with tc.tile_pool(name="consts", bufs=1) as consts, \
             tc.tile_pool(name="vpool", bufs=1) as vpool:
            # ---- persistent weights / constants (bf16) ----
            wvr = [consts.tile([128, OUT], BF16, name=f"wvr{e}") for e in range(4)]
            wvi = [consts.tile([128, OUT], BF16, name=f"wvi{e}") for e in range(4)]
            wvin = [consts.tile([128, OUT], BF16, name=f"wvin{e}") for e in range(4)]
            wqr = [consts.tile([128, E], BF16, name=f"wqr{e}") for e in range(4)]
            wqi = [consts.tile([128, E], BF16, name=f"wqi{e}") for e in range(4)]
            wkr = [consts.tile([128, E], BF16, name=f"wkr{e}") for e in range(4)]
            wki = [consts.tile([128, E], BF16, name=f"wki{e}") for e in range(4)]
            wcr = [consts.tile([128, OUT], BF16, name=f"wcr{e}") for e in range(4)]
            wci = [consts.tile([128, OUT], BF16, name=f"wci{e}") for e in range(4)]
            bqr = consts.tile([128, 4], F32)
            bqi = consts.tile([128, 4], F32)
            bqin = consts.tile([128, 4], F32)
            bcr = consts.tile([BPC, OUT], F32)
            bci = consts.tile([BPC, OUT], F32)
            ident = consts.tile([128, 128], F32)
            mask8 = consts.tile([NH, OUT], F32)
            sel32 = consts.tile([32, BPC], BF16)

            # x tiles: [128e, BPC, SP]
            xbr = [vpool.tile([128, BPC, SP], BF16, name=f"xbr{e}") for e in range(4)]
            xbi = [vpool.tile([128, BPC, SP], BF16, name=f"xbi{e}") for e in range(4)]
            x0in = [vpool.tile([128, BPC], BF16, name=f"x0in{e}") for e in range(4)]
            # v tiles live until hv
            vr = [[vpool.tile([128, OUT], BF16, name=f"vr{b}_{s}")
                   for s in range(2)] for b in range(BPC)]
            vi = [[vpool.tile([128, OUT], BF16, name=f"vi{b}_{s}")
                   for s in range(2)] for b in range(BPC)]
            vCr_sb = vpool.tile([BPC, OUT], BF16)
            vCi_sb = vpool.tile([BPC, OUT], BF16)
            # bd: per-u zero-padded block-diag q0 [128, 32] (cols b*8+2u+p)
            bd_r = [vpool.tile([128, 32], BF16, name=f"bd_r{u}") for u in range(4)]
            bd_i = [vpool.tile([128, 32], BF16, name=f"bd_i{u}") for u in range(4)]
            bd_in = [vpool.tile([128, 32], BF16, name=f"bd_in{u}") for u in range(4)]
            q0r_sb = vpool.tile([BPC, E], F32)
            q0i_sb = vpool.tile([BPC, E], F32)
            qk_sb_r = vpool.tile([32, E], F32)
            qk_sb_i = vpool.tile([32, E], F32)
            qkT_r = [vpool.tile([128, 32], BF16, name=f"qkTr{e}") for e in range(4)]
            qkT_i = [vpool.tile([128, 32], BF16, name=f"qkTi{e}") for e in range(4)]
            qkT_in = [vpool.tile([128, 32], BF16, name=f"qkTin{e}") for e in range(4)]

            # ---- DMA issue order matters per queue ----
            # sync queue: small consts then x real
            nc.sync.dma_start(out=ident, in_=d_id.ap())
            nc.sync.dma_start(out=sel32, in_=d_sel.ap())
            nc.sync.dma_start(out=mask8, in_=d_mask.ap())
            nc.sync.dma_start(out=bqr, in_=d_bqr.ap())
            nc.sync.dma_start(out=bqi, in_=d_bqi.ap())
            nc.sync.dma_start(out=bcr, in_=d_bcr.ap())
            nc.sync.dma_start(out=bci, in_=d_bci.ap())
            for u in range(4):
                nc.sync.dma_start(out=bd_r[u], in_=d_zbd.ap())
                nc.sync.dma_start(out=bd_i[u], in_=d_zbd.ap())
                nc.sync.dma_start(out=bd_in[u], in_=d_zbd.ap())
            # Two hw DMA queues (scalar, gpsimd), interleaved in first-need
            # order; weights are pre-tiled [128, 4, X] on host so every DMA
            # is contiguous per partition.
            for e in range(4):
                sl = slice(e * 128, (e + 1) * 128)
                nc.scalar.dma_start(out=xbr[e][:], in_=d_xr.ap()[sl, :, :])
                nc.scalar.dma_start(out=wvr[e], in_=d_wvr.ap()[:, e, :])
                if e == 0:
                    nc.scalar.dma_start(out=wqr[0], in_=d_wqr.ap()[:, 0, :])
                nc.gpsimd.dma_start(out=xbi[e][:], in_=d_xi.ap()[sl, :, :])
                nc.gpsimd.dma_start(out=wvi[e], in_=d_wvi.ap()[:, e, :])
            for e in range(4):
                if e > 0:
                    nc.scalar.dma_start(out=wqr[e], in_=d_wqr.ap()[:, e, :])
                nc.scalar.dma_start(out=wqi[e], in_=d_wqi.ap()[:, e, :])
            for e in range(4):
                nc.gpsimd.dma_start(out=wkr[e], in_=d_wkr.ap()[:, e, :])
                nc.gpsimd.dma_start(out=wki[e], in_=d_wki.ap()[:, e, :])
            for e in range(4):
                nc.scalar.dma_start(out=wcr[e], in_=d_wcr.ap()[:, e, :])
                nc.gpsimd.dma_start(out=wci[e], in_=d_wci.ap()[:, e, :])

            nc.vector.tensor_scalar_mul(bqin, bqi, -1.0)
            # negate wv imag on device (keeps it off the DMA critical path)
            for e in range(4):
                nc.vector.tensor_scalar_mul(wvin[e][:], wvi[e][:], -1.0)
            # x arrives fully prepped from host (mean in col 0, pos added);
            # only the negated imag of token 0 is built on device
            for e in range(4):
                nc.scalar.activation(x0in[e][:], xbi[e][:, :, 0], ACTF.Copy,
                                     bias=0.0, scale=-1.0)

            mm = nc.tensor.matmul

            with tc.tile_pool(name="psA", bufs=2, space="PSUM") as psA:
                # v rows s in [sb*128, (sb+1)*128) = x cols (col 0 = mean tok)
                def emit_v(b):
                    for sb in range(2):
                        cs = slice(sb * 128, (sb + 1) * 128)
                        p1 = psA.tile([128, OUT], F32, tag="pv1", name=f"pv1_{b}_{sb}")
                        pi = psA.tile([128, OUT], F32, tag="pvi", name=f"pvi_{b}_{sb}")
                        for j, (x, w) in enumerate(
                                [(xbr[e][:, b, cs], wvr[e]) for e in range(4)]
                                + [(xbi[e][:, b, cs], wvin[e]) for e in range(4)]):
                            mm(p1[:], x, w[:], start=(j == 0), stop=(j == 7))
                        for j, (x, w) in enumerate(
                                [(xbr[e][:, b, cs], wvi[e]) for e in range(4)]
                                + [(xbi[e][:, b, cs], wvr[e]) for e in range(4)]):
                            mm(pi[:], x, w[:], start=(j == 0), stop=(j == 7))
                        nc.vector.tensor_copy(vr[b][sb][:], p1[:])
                        nc.scalar.copy(vi[b][sb][:], pi[:])

                emit_v(0)

                # ============ q0 -> bd ============
                with tc.tile_pool(name="psB1", bufs=1, space="PSUM") as psB1:
                    pqr = psB1.tile([BPC, E], F32, tag="pqr")
                    pqi = psB1.tile([BPC, E], F32, tag="pqi")
                    for j, (x, w) in enumerate(
                            [(xbr[e][:, :, 0], wqr[e][:]) for e in range(4)]
                            + [(x0in[e][:], wqi[e][:]) for e in range(4)]):
                        mm(pqr[:], x, w, start=(j == 0), stop=(j == 7))
                    for j, (x, w) in enumerate(
                            [(xbr[e][:, :, 0], wqi[e][:]) for e in range(4)]
                            + [(xbi[e][:, :, 0], wqr[e][:]) for e in range(4)]):
                        mm(pqi[:], x, w, start=(j == 0), stop=(j == 7))
                    nc.scalar.copy(q0r_sb[:], pqr[:])
                    nc.scalar.copy(q0i_sb[:], pqi[:])

                    # transpose q0 -> bd block-diag [128, 4u, 8]
                    # bd[p*64+d, u, 2b+p] = q0[b, u*128+p*64+d] + bq bias
                    for u in range(4):
                        fs = slice(u * 128, (u + 1) * 128)
                        ptr = psB1.tile([128, 4], F32, tag="ptq", bufs=1, name=f"ptq{u}")
                        pti = psB1.tile([128, 4], F32, tag="ptj", bufs=1, name=f"ptj{u}")
                        nc.tensor.transpose(ptr[:], q0r_sb[:, fs], ident[0:BPC, 0:BPC])
                        nc.tensor.transpose(pti[:], q0i_sb[:, fs], ident[0:BPC, 0:BPC])
                        for p in range(2):
                            rows = slice(p * 64, (p + 1) * 64)
                            cols = slice(2 * u + p, 32, 8)
                            nc.scalar.activation(bd_r[u][rows, cols], ptr[rows, :],
                                                 ACTF.Identity,
                                                 bias=bqr[rows, u:u + 1], scale=1.0)
                            nc.scalar.activation(bd_i[u][rows, cols], pti[rows, :],
                                                 ACTF.Identity,
                                                 bias=bqi[rows, u:u + 1], scale=1.0)
                            nc.scalar.activation(bd_in[u][rows, cols], pti[rows, :],
                                                 ACTF.Identity,
                                                 bias=bqin[rows, u:u + 1], scale=-1.0)

                # next v batch fills PE while bd copies run
                emit_v(1)

                # ============ qk = bd^T @ wk  [rows b*8 + 2u+p, 512e] ============
                with tc.tile_pool(name="psQK", bufs=1, space="PSUM") as psQK:
                    pkr = psQK.tile([32, E], F32, tag="pkr")
                    pki = psQK.tile([32, E], F32, tag="pki")
                    for j, (bdt, w) in enumerate(
                            [(bd_r[u], wkr[u]) for u in range(4)]
                            + [(bd_in[u], wki[u]) for u in range(4)]):
                        mm(pkr[:], bdt[:], w[:], start=(j == 0), stop=(j == 7))
                    for j, (bdt, w) in enumerate(
                            [(bd_r[u], wki[u]) for u in range(4)]
                            + [(bd_i[u], wkr[u]) for u in range(4)]):
                        mm(pki[:], bdt[:], w[:], start=(j == 0), stop=(j == 7))
                    nc.vector.tensor_copy(qk_sb_r[:], pkr[:])
                    nc.scalar.copy(qk_sb_i[:], pki[:])

                # v b2 + vC fill PE during qk copies / qkT transposes
                emit_v(2)

                # vC: token-256 v row for all batches
                with tc.tile_pool(name="psVC", bufs=1, space="PSUM") as psVC:
                    p1 = psVC.tile([BPC, OUT], F32, tag="pc1")
                    pi = psVC.tile([BPC, OUT], F32, tag="pci")
                    for j, (x, w) in enumerate(
                            [(xbr[e][:, :, 256], wvr[e]) for e in range(4)]
                            + [(xbi[e][:, :, 256], wvin[e]) for e in range(4)]):
                        mm(p1[:], x, w[:], start=(j == 0), stop=(j == 7))
                    for j, (x, w) in enumerate(
                            [(xbr[e][:, :, 256], wvi[e]) for e in range(4)]
                            + [(xbi[e][:, :, 256], wvr[e]) for e in range(4)]):
                        mm(pi[:], x, w[:], start=(j == 0), stop=(j == 7))
                    nc.vector.tensor_copy(vCr_sb[:], p1[:])
                    nc.scalar.copy(vCi_sb[:], pi[:])

                # transpose qk -> qkT [128e, 4u, 8] (+ negated imag)
                with tc.tile_pool(name="psQT", bufs=2, space="PSUM") as psQT:
                    for e in range(4):
                        es = slice(e * 128, (e + 1) * 128)
                        ptr = psQT.tile([128, 32], F32, tag="qtr", name=f"qtr{e}")
                        pti = psQT.tile([128, 32], F32, tag="qti", name=f"qti{e}")
                        nc.tensor.transpose(ptr[:], qk_sb_r[:, es], ident[0:32, 0:32])
                        nc.tensor.transpose(pti[:], qk_sb_i[:, es], ident[0:32, 0:32])
                        nc.scalar.copy(qkT_r[e][:], ptr[:])
                        nc.scalar.copy(qkT_i[e][:], pti[:])
                        nc.vector.tensor_scalar_mul(qkT_in[e][:], pti[:], -1.0)

                # last v batch fills PE while qkT copies land
                emit_v(3)

            # ============ per-batch pipeline: logits -> softmax -> wT -> hv
            # (hv of batch b overlaps logits of batch b+1 on PE) ============
            with tc.tile_pool(name="miscB2", bufs=1) as mb:
                # vC2[p, b, :]: rows (re, im); vC2s rows (im, re)
                vC2 = mb.tile([2, BPC, OUT], BF16)
                vC2s = mb.tile([2, BPC, OUT], BF16)
                nc.sync.dma_start(out=vC2[0:1, :, :], in_=vCr_sb[:])
                nc.sync.dma_start(out=vC2[1:2, :, :], in_=vCi_sb[:])
                nc.sync.dma_start(out=vC2s[0:1, :, :], in_=vCi_sb[:])
                nc.sync.dma_start(out=vC2s[1:2, :, :], in_=vCr_sb[:])

                w_b = [mb.tile([8, 2, S], F32, name=f"w_b{b}") for b in range(BPC)]
                wTr = [mb.tile([128, 32], BF16, name=f"wTr{a}") for a in range(2)]
                wTi = [mb.tile([128, 32], BF16, name=f"wTi{a}") for a in range(2)]
                wTin = [mb.tile([128, 32], BF16, name=f"wTin{a}") for a in range(2)]
                wtc_a = mb.tile([2, 32], BF16)   # rows: wTr_c, -wTi_c
                wtc_b = mb.tile([2, 32], BF16)   # rows: wTr_c, wTi_c
                wtc_neg = mb.tile([2, 32], BF16)
                hvm_r = [mb.tile([NH, OUT], BF16, name=f"hvm_r{b}") for b in range(BPC)]
                hvm_i = [mb.tile([NH, OUT], BF16, name=f"hvm_i{b}") for b in range(BPC)]
                hvm_all_r = mb.tile([32, OUT], BF16)
                hvm_all_i = mb.tile([32, OUT], BF16)

                with tc.tile_pool(name="psB2", bufs=2, space="PSUM") as psB2, \
                     tc.tile_pool(name="psB3", bufs=1, space="PSUM") as psB3, \
                     tc.tile_pool(name="psB4", bufs=2, space="PSUM") as psB4:
                    # pw[:, 0:2, :] = wT re s-halves; [:, 2:4, :] = im; ptc sep
                    pw = psB3.tile([128, 4, 32], F32, tag="pw")
                    ptc = psB3.tile([2, 32], F32, tag="ptc")

                    def softmax(b, ri, psum):
                        # logits are O(+-8): exp safe in f32 without max-shift
                        sm = mb.tile([8, 1], F32, tag="ssm", name=f"sm{b}_{ri}")
                        rs = mb.tile([8, 1], F32, tag="srs", name=f"rs{b}_{ri}")
                        nc.scalar.activation(w_b[b][:, ri, :], psum[:, 0:S],
                                             ACTF.Exp, bias=0.0, scale=1.0,
                                             accum_out=sm[:])
                        nc.vector.reciprocal(rs[:], sm[:])
                        nc.vector.tensor_scalar_mul(w_b[b][:, ri, :],
                                                    w_b[b][:, ri, :], rs[:])

                    def emit_logits(b):
                        bcols = slice(b * 8, b * 8 + 8)
                        pr = psB2.tile([8, SP], F32, tag="plg", name=f"plgr{b}")
                        for j, (q, x) in enumerate(
                                [(qkT_r[e][:, bcols], xbr[e][:, b, :])
                                 for e in range(4)]
                                + [(qkT_in[e][:, bcols], xbi[e][:, b, :])
                                   for e in range(4)]):
                            mm(pr[:], q, x, start=(j == 0), stop=(j == 7))
                        softmax(b, 0, pr)
                        pq = psB2.tile([8, SP], F32, tag="plg", name=f"plgi{b}")
                        for j, (q, x) in enumerate(
                                [(qkT_r[e][:, bcols], xbi[e][:, b, :])
                                 for e in range(4)]
                                + [(qkT_i[e][:, bcols], xbr[e][:, b, :])
                                   for e in range(4)]):
                            mm(pq[:], q, x, start=(j == 0), stop=(j == 7))
                        softmax(b, 1, pq)

                    def emit_attn(b):
                        bcols = slice(b * 8, b * 8 + 8)
                        # -- transpose w -> wT columns for this batch --
                        for a in range(2):
                            cs = slice(a * 128, (a + 1) * 128)
                            for ri in range(2):
                                nc.tensor.matmul(pw[:, 2 * ri + a, bcols],
                                                 w_b[b][:, ri, cs],
                                                 ident[0:8, 0:8],
                                                 is_transpose=True,
                                                 skip_group_check=True)
                        nc.tensor.matmul(ptc[:, bcols], w_b[b][:, :, 256],
                                         ident[0:8, 0:8], is_transpose=True,
                                         skip_group_check=True)
                        for a in range(2):
                            nc.scalar.copy(wTr[a][:, bcols], pw[:, a, bcols])
                            nc.scalar.copy(wTi[a][:, bcols], pw[:, 2 + a, bcols])
                            nc.scalar.activation(wTin[a][:, bcols],
                                                 pw[:, 2 + a, bcols],
                                                 ACTF.Copy, bias=0.0, scale=-1.0)
                        nc.scalar.copy(wtc_b[:, bcols], ptc[:, bcols])
                        nc.scalar.activation(wtc_neg[:, bcols], ptc[:, bcols],
                                             ACTF.Copy, bias=0.0, scale=-1.0)
                        nc.sync.dma_start(out=wtc_a[0:1, bcols],
                                          in_=wtc_b[0:1, bcols])
                        nc.sync.dma_start(out=wtc_a[1:2, bcols],
                                          in_=wtc_neg[1:2, bcols])
                        # -- hv --
                        ph_r = psB4.tile([NH, OUT], F32, tag="phr", name=f"phr{b}")
                        ph_i = psB4.tile([NH, OUT], F32, tag="phi", name=f"phi{b}")
                        mm(ph_r[:], wTr[0][:, bcols], vr[b][0][:], start=True, stop=False)
                        mm(ph_r[:], wTr[1][:, bcols], vr[b][1][:], start=False, stop=False)
                        mm(ph_r[:], wTin[0][:, bcols], vi[b][0][:], start=False, stop=False)
                        mm(ph_r[:], wTin[1][:, bcols], vi[b][1][:], start=False, stop=False)
                        mm(ph_r[:], wtc_a[:, bcols], vC2[:, b, :], start=False, stop=True)
                        mm(ph_i[:], wTi[0][:, bcols], vr[b][0][:], start=True, stop=False)
                        mm(ph_i[:], wTi[1][:, bcols], vr[b][1][:], start=False, stop=False)
                        mm(ph_i[:], wTr[0][:, bcols], vi[b][0][:], start=False, stop=False)
                        mm(ph_i[:], wTr[1][:, bcols], vi[b][1][:], start=False, stop=False)
                        mm(ph_i[:], wtc_b[:, bcols], vC2s[:, b, :], start=False, stop=True)
                        nc.vector.tensor_mul(hvm_r[b][:], ph_r[:], mask8[:])
                        nc.vector.tensor_mul(hvm_i[b][:], ph_i[:], mask8[:])
                        nc.sync.dma_start(out=hvm_all_r[b * 8:b * 8 + 8, :],
                                          in_=hvm_r[b][:])
                        nc.sync.dma_start(out=hvm_all_i[b * 8:b * 8 + 8, :],
                                          in_=hvm_i[b][:])

                    # software-pipelined: logits of b+1 issue before attn of b
                    # so PE never waits on softmax
                    emit_logits(0)
                    emit_logits(1)
                    emit_attn(0)
                    emit_logits(2)
                    emit_attn(1)
                    emit_logits(3)
                    emit_attn(2)
                    emit_attn(3)

                # ---- extract attn0^T [128, 4] per f-tile via selection matmul ----
                att_r = [mb.tile([128, 4], BF16, name=f"att_r{u}") for u in range(4)]
                att_i = [mb.tile([128, 4], BF16, name=f"att_i{u}") for u in range(4)]
                att_in = [mb.tile([128, 4], BF16, name=f"att_in{u}") for u in range(4)]
                with tc.tile_pool(name="psB5", bufs=2, space="PSUM") as psB5:
                    for u in range(4):
                        fs = slice(u * 128, (u + 1) * 128)
                        par = psB5.tile([128, 4], F32, tag="par", name=f"par{u}")
                        pai = psB5.tile([128, 4], F32, tag="pai", name=f"pai{u}")
                        mm(par[:], hvm_all_r[:, fs], sel32[:], start=True, stop=True)
                        mm(pai[:], hvm_all_i[:, fs], sel32[:], start=True, stop=True)
                        nc.scalar.copy(att_r[u][:], par[:])
                        nc.scalar.copy(att_i[u][:], pai[:])
                        nc.scalar.activation(att_in[u][:], pai[:], ACTF.Copy,
                                             bias=0.0, scale=-1.0)

                # ---- y = attn0 @ Wc^T + b_c ----
                yr_sb = mb.tile([BPC, OUT], F32)
                yi_sb = mb.tile([BPC, OUT], F32)
                with tc.tile_pool(name="psB6", bufs=1, space="PSUM") as psB6:
                    py_r = psB6.tile([BPC, OUT], F32, tag="pyr")
                    py_i = psB6.tile([BPC, OUT], F32, tag="pyi")
                    for j, u in enumerate(range(4)):
                        mm(py_r[:], att_r[u][:], wcr[u][:], start=(j == 0), stop=False)
                        mm(py_r[:], att_in[u][:], wci[u][:], start=False, stop=(j == 3))
                        mm(py_i[:], att_r[u][:], wci[u][:], start=(j == 0), stop=False)
                        mm(py_i[:], att_i[u][:], wcr[u][:], start=False, stop=(j == 3))
                    nc.vector.tensor_add(yr_sb[:], py_r[:], bcr[:])
                    nc.vector.tensor_add(yi_sb[:], py_i[:], bci[:])
                    nc.sync.dma_start(out=d_yr.ap(), in_=yr_sb[:])
                    nc.sync.dma_start(out=d_yi.ap(), in_=yi_sb[:])

    nc.compile()
    return nc


def _host_prep(inputs):
    """Build per-core in_maps from the full inputs."""
    import ml_dtypes
    f32 = np.float32
    bf16 = ml_dtypes.bfloat16
    xr = np.ascontiguousarray(inputs["x_real"], dtype=f32).reshape(B, E, HW)
    xi = np.ascontiguousarray(inputs["x_imag"], dtype=f32).reshape(B, E, HW)
    pos = np.asarray(inputs["pos_r"], dtype=f32) + 1j * np.asarray(inputs["pos_i"], dtype=f32)
    w_in_r = np.asarray(inputs["w_in_r"], dtype=f32)
    w_in_i = np.asarray(inputs["w_in_i"], dtype=f32)
    b_in_r = np.asarray(inputs["b_in_r"], dtype=f32)
    b_in_i = np.asarray(inputs["b_in_i"], dtype=f32)
    w_out = np.asarray(inputs["w_out_r"], dtype=f32) + 1j * np.asarray(inputs["w_out_i"], dtype=f32)
    b_out = np.asarray(inputs["b_out_r"], dtype=f32) + 1j * np.asarray(inputs["b_out_i"], dtype=f32)
    w_p = np.asarray(inputs["w_p_r"], dtype=f32) + 1j * np.asarray(inputs["w_p_i"], dtype=f32)
    b_p = np.asarray(inputs["b_p_r"], dtype=f32) + 1j * np.asarray(inputs["b_p_i"], dtype=f32)

    w_in = w_in_r + 1j * w_in_i
    wq, wk, wv = w_in[:E], w_in[E:2 * E], w_in[2 * E:]
    qs = f32(1.0 / np.sqrt(HD))

    posb = np.zeros((E, SP), np.complex64)
    posb[:, :S] = pos

    wc = w_p @ w_out                                        # [OUT, E] complex
    bq = qs * (b_in_r[:E] + 1j * b_in_i[:E])                # [E]

    b_v = b_in_r[2 * E:] + 1j * b_in_i[2 * E:]
    b_c = (1 + 1j) * (b_v @ wc.T) + b_out @ w_p.T + b_p     # [OUT] complex

    mask8 = np.zeros((NH, OUT), f32)
    for h in range(NH):
        mask8[h, h * HD:(h + 1) * HD] = 1.0
    sel32 = np.zeros((32, BPC), f32)
    for b in range(BPC):
        sel32[b * 8:(b + 1) * 8, b] = 1.0

    tile4 = lambda a: np.ascontiguousarray(
        np.asarray(a, f32).reshape(4, 128, -1).transpose(1, 0, 2)).astype(bf16)
    shared = dict(
        wqr=tile4(wq.real.T * qs),
        wqi=tile4(wq.imag.T * qs),
        wkr=tile4(wk.real),
        wki=tile4(wk.imag),
        wvr=tile4(wv.real.T),
        wvi=tile4(wv.imag.T),
        wcr=tile4(wc.real.T),
        wci=tile4(wc.imag.T),
        bqr=bq.real.astype(f32).reshape(4, 128).T.copy(),
        bqi=bq.imag.astype(f32).reshape(4, 128).T.copy(),
        bcr=np.broadcast_to(b_c.real.astype(f32), (BPC, OUT)).copy(),
        bci=np.broadcast_to(b_c.imag.astype(f32), (BPC, OUT)).copy(),
        ident=np.eye(128, dtype=f32),
        mask8=mask8,
        sel32=sel32.astype(bf16),
        zbd=np.zeros((128, 32), bf16),
    )
    # x_cat fully prepped on host: col 0 = mean, then + pos; col 257 zero
    xrp = np.zeros((B, E, SP), f32)
    xip = np.zeros((B, E, SP), f32)
    xrp[:, :, 1:1 + HW] = xr
    xip[:, :, 1:1 + HW] = xi
    xrp[:, :, 0] = xr.mean(-1)
    xip[:, :, 0] = xi.mean(-1)
    xrp[:, :, :S] += posb.real[None, :, :S]
    xip[:, :, :S] += posb.imag[None, :, :S]
    in_maps = []
    for c in range(NCORES):
        m = dict(shared)
        m["xr"] = np.ascontiguousarray(
            xrp[c * BPC:(c + 1) * BPC].transpose(1, 0, 2)).astype(bf16)
        m["xi"] = np.ascontiguousarray(
            xip[c * BPC:(c + 1) * BPC].transpose(1, 0, 2)).astype(bf16)
        in_maps.append(m)
    return in_maps


def _run(inputs, trace=False, **kw):
    from concourse.bass_utils import run_bass_kernel_spmd
    if "nc" not in _cached:
        _cached["nc"] = _build()
    nc = _cached["nc"]
    in_maps = _host_prep(inputs)
    res = run_bass_kernel_spmd(nc, in_maps, core_ids=list(range(NCORES)),
                               trace=trace, **kw)
    out = np.empty((B, OUT), np.complex64)
    for c in range(NCORES):
        out[c * BPC:(c + 1) * BPC] = (res.results[c]["yr"]
                                      + 1j * res.results[c]["yi"])
    return out, res


def kernel(**inputs) -> np.ndarray:
    out, _ = _run(inputs)
    return out


# revision 34
# speedup vs baseline: 1.1683x; 1.1418x over previous
"""Complex AttentionPool2d on 8 trn2 NeuronCores, data-parallel over batch.

Contract: kernel(**inputs) takes the FULL inputs from setup_inputs() and
returns the FULL [32, 512] complex64 output.

V2: all matmuls bf16 (fp32 PSUM accum); k^T eliminated algebraically.
Math (per batch):
  x = bf16(complex(x_real, x_imag)).reshape(E, 256)
  x_cat = [mean(x), x] + pos                       # [E, 257]
  q0 = x_cat[:, 0] @ wq^T + bq                     # only query pos 0 matters
  qk[h, e] = sum_d q0[h*64+d] wk[h*64+d, e]        # fold q into k-proj
  logits[h, s] = sum_e qk[h, e] x_cat[e, s]        # == q0 . k[s]
  w = softmax(logits.re) + i*softmax(logits.im)
  v = x_cat^T @ wv^T                               # [257, 512]
  attn0 = (w @ v) per-head masked; y = attn0 @ (w_p @ w_out)^T + b_c

Sharding: batch 32 -> 4 per core.
"""
import numpy as np

B, E, HW, S = 32, 512, 256, 257
SP = 258            # S padded even
NH, HD = 8, 64
OUT = 512
NCORES = 8
BPC = B // NCORES   # batches per core

_cached = {}


def _build():
    import concourse.bacc as bacc
    import concourse.tile as tile
    import concourse.mybir as mybir

    F32 = mybir.dt.float32
    BF16 = mybir.dt.bfloat16
    AX = mybir.AxisListType
    ACTF = mybir.ActivationFunctionType

    nc = bacc.Bacc("TRN2", target_bir_lowering=False, debug=False)

    # ---- DRAM I/O ----
    # x layout: [E, BPC, SP] so one DMA per e-tile covers all 4 batches;
    # col 0 reserved for the mean token, col 257 zero pad
    d_xr = nc.dram_tensor("xr", [E, BPC, SP], BF16, kind="ExternalInput")
    d_xi = nc.dram_tensor("xi", [E, BPC, SP], BF16, kind="ExternalInput")
    d_wqr = nc.dram_tensor("wqr", [E, E], BF16, kind="ExternalInput")
    d_wqi = nc.dram_tensor("wqi", [E, E], BF16, kind="ExternalInput")
    d_wkr = nc.dram_tensor("wkr", [E, E], BF16, kind="ExternalInput")
    d_wki = nc.dram_tensor("wki", [E, E], BF16, kind="ExternalInput")
    d_wvr = nc.dram_tensor("wvr", [E, OUT], BF16, kind="ExternalInput")
    d_wvi = nc.dram_tensor("wvi", [E, OUT], BF16, kind="ExternalInput")
    d_wcr = nc.dram_tensor("wcr", [E, OUT], BF16, kind="ExternalInput")
    d_wci = nc.dram_tensor("wci", [E, OUT], BF16, kind="ExternalInput")
    d_bqr = nc.dram_tensor("bqr", [128, 4], F32, kind="ExternalInput")
    d_bqi = nc.dram_tensor("bqi", [128, 4], F32, kind="ExternalInput")
    d_bcr = nc.dram_tensor("bcr", [BPC, OUT], F32, kind="ExternalInput")
    d_bci = nc.dram_tensor("bci", [BPC, OUT], F32, kind="ExternalInput")
    d_id = nc.dram_tensor("ident", [128, 128], F32, kind="ExternalInput")
    d_mask = nc.dram_tensor("mask8", [NH, OUT], F32, kind="ExternalInput")
    d_sel = nc.dram_tensor("sel32", [32, BPC], BF16, kind="ExternalInput")
    d_zbd = nc.dram_tensor("zbd", [128, 32], BF16, kind="ExternalInput")
    d_yr = nc.dram_tensor("yr", [BPC, OUT], F32, kind="ExternalOutput")
    d_yi = nc.dram_tensor("yi", [BPC, OUT], F32, kind="ExternalOutput")

    with tile.TileContext(nc) as tc:
        with tc.tile_pool(name="consts", bufs=1) as consts, \
             tc.tile_pool(name="vpool", bufs=1) as vpool:
            # ---- persistent weights / constants (bf16) ----
            wvr = [consts.tile([128, OUT], BF16, name=f"wvr{e}") for e in range(4)]
            wvi = [consts.tile([128, OUT], BF16, name=f"wvi{e}") for e in range(4)]
            wvin = [consts.tile([128, OUT], BF16, name=f"wvin{e}") for e in range(4)]
            wqr = [consts.tile([128, E], BF16, name=f"wqr{e}") for e in range(4)]
            wqi = [consts.tile([128, E], BF16, name=f"wqi{e}") for e in range(4)]
            wkr = [consts.tile([128, E], BF16, name=f"wkr{e}") for e in range(4)]
            wki = [consts.tile([128, E], BF16, name=f"wki{e}") for e in range(4)]
            wcr = [consts.tile([128, OUT], BF16, name=f"wcr{e}") for e in range(4)]
            wci = [consts.tile([128, OUT], BF16, name=f"wci{e}") for e in range(4)]
            bqr = consts.tile([128, 4], F32)
            bqi = consts.tile([128, 4], F32)
            bqin = consts.tile([128, 4], F32)
            bcr = consts.tile([BPC, OUT], F32)
            bci = consts.tile([BPC, OUT], F32)
            ident = consts.tile([128, 128], F32)
            mask8 = consts.tile([NH, OUT], F32)
            sel32 = consts.tile([32, BPC], BF16)

            # x tiles: [128e, BPC, SP]
            xbr = [vpool.tile([128, BPC, SP], BF16, name=f"xbr{e}") for e in range(4)]
            xbi = [vpool.tile([128, BPC, SP], BF16, name=f"xbi{e}") for e in range(4)]
            x0in = [vpool.tile([128, BPC], BF16, name=f"x0in{e}") for e in range(4)]
            # v tiles live until hv
            vr = [[vpool.tile([128, OUT], BF16, name=f"vr{b}_{s}")
                   for s in range(2)] for b in range(BPC)]
            vi = [[vpool.tile([128, OUT], BF16, name=f"vi{b}_{s}")
                   for s in range(2)] for b in range(BPC)]
            vCr_sb = vpool.tile([BPC, OUT], BF16)
            vCi_sb = vpool.tile([BPC, OUT], BF16)
            # bd: per-u zero-padded block-diag q0 [128, 32] (cols b*8+2u+p)
            bd_r = [vpool.tile([128, 32], BF16, name=f"bd_r{u}") for u in range(4)]
            bd_i = [vpool.tile([128, 32], BF16, name=f"bd_i{u}") for u in range(4)]
            bd_in = [vpool.tile([128, 32], BF16, name=f"bd_in{u}") for u in range(4)]
            q0r_sb = vpool.tile([BPC, E], F32)
            q0i_sb = vpool.tile([BPC, E], F32)
            qk_sb_r = vpool.tile([32, E], F32)
            qk_sb_i = vpool.tile([32, E], F32)
            qkT_r = [vpool.tile([128, 32], BF16, name=f"qkTr{e}") for e in range(4)]
            qkT_i = [vpool.tile([128, 32], BF16, name=f"qkTi{e}") for e in range(4)]
            qkT_in = [vpool.tile([128, 32], BF16, name=f"qkTin{e}") for e in range(4)]

            # ---- DMA issue order matters per queue ----
            # sync queue: small consts then x real
            nc.sync.dma_start(out=ident, in_=d_id.ap())
            nc.sync.dma_start(out=sel32, in_=d_sel.ap())
            nc.sync.dma_start(out=mask8, in_=d_mask.ap())
            nc.sync.dma_start(out=bqr, in_=d_bqr.ap())
            nc.sync.dma_start(out=bqi, in_=d_bqi.ap())
            nc.sync.dma_start(out=bcr, in_=d_bcr.ap())
            nc.sync.dma_start(out=bci, in_=d_bci.ap())
            for u in range(4):
                nc.sync.dma_start(out=bd_r[u], in_=d_zbd.ap())
                nc.sync.dma_start(out=bd_i[u], in_=d_zbd.ap())
                nc.sync.dma_start(out=bd_in[u], in_=d_zbd.ap())
            # Two hw DMA queues (scalar, gpsimd), interleaved in first-need
            # order; weights are pre-tiled [128, 4, X] on host so every DMA
            # is contiguous per partition.
            for e in range(4):
                sl = slice(e * 128, (e + 1) * 128)
                nc.scalar.dma_start(out=xbr[e][:], in_=d_xr.ap()[sl, :, :])
                nc.scalar.dma_start(out=wvr[e], in_=d_wvr.ap()[sl, :])
                if e == 0:
                    nc.scalar.dma_start(out=wqr[0], in_=d_wqr.ap()[0:128, :])
                nc.gpsimd.dma_start(out=xbi[e][:], in_=d_xi.ap()[sl, :, :])
                nc.gpsimd.dma_start(out=wvi[e], in_=d_wvi.ap()[sl, :])
            for e in range(4):
                sl = slice(e * 128, (e + 1) * 128)
                if e > 0:
                    nc.scalar.dma_start(out=wqr[e], in_=d_wqr.ap()[sl, :])
                nc.scalar.dma_start(out=wqi[e], in_=d_wqi.ap()[sl, :])
            for e in range(4):
                sl = slice(e * 128, (e + 1) * 128)
                nc.gpsimd.dma_start(out=wkr[e], in_=d_wkr.ap()[sl, :])
                nc.gpsimd.dma_start(out=wki[e], in_=d_wki.ap()[sl, :])
            for e in range(4):
                sl = slice(e * 128, (e + 1) * 128)
                nc.scalar.dma_start(out=wcr[e], in_=d_wcr.ap()[sl, :])
                nc.gpsimd.dma_start(out=wci[e], in_=d_wci.ap()[sl, :])

            nc.vector.tensor_scalar_mul(bqin, bqi, -1.0)
            # negate wv imag on device (keeps it off the DMA critical path)
            for e in range(4):
                nc.vector.tensor_scalar_mul(wvin[e][:], wvi[e][:], -1.0)
            # x arrives fully prepped from host (mean in col 0, pos added);
            # only the negated imag of token 0 is built on device
            for e in range(4):
                nc.scalar.activation(x0in[e][:], xbi[e][:, :, 0], ACTF.Copy,
                                     bias=0.0, scale=-1.0)

            mm = nc.tensor.matmul

            with tc.tile_pool(name="psA", bufs=2, space="PSUM") as psA:
                # v rows s in [sb*128, (sb+1)*128) = x cols (col 0 = mean tok)
                def emit_v(b):
                    for sb in range(2):
                        cs = slice(sb * 128, (sb + 1) * 128)
                        p1 = psA.tile([128, OUT], F32, tag="pv1", name=f"pv1_{b}_{sb}")
                        pi = psA.tile([128, OUT], F32, tag="pvi", name=f"pvi_{b}_{sb}")
                        for j, (x, w) in enumerate(
                                [(xbr[e][:, b, cs], wvr[e]) for e in range(4)]
                                + [(xbi[e][:, b, cs], wvin[e]) for e in range(4)]):
                            mm(p1[:], x, w[:], start=(j == 0), stop=(j == 7))
                        for j, (x, w) in enumerate(
                                [(xbr[e][:, b, cs], wvi[e]) for e in range(4)]
                                + [(xbi[e][:, b, cs], wvr[e]) for e in range(4)]):
                            mm(pi[:], x, w[:], start=(j == 0), stop=(j == 7))
                        nc.vector.tensor_copy(vr[b][sb][:], p1[:])
                        nc.scalar.copy(vi[b][sb][:], pi[:])

                emit_v(0)

                # ============ q0 -> bd ============
                with tc.tile_pool(name="psB1", bufs=1, space="PSUM") as psB1:
                    pqr = psB1.tile([BPC, E], F32, tag="pqr")
                    pqi = psB1.tile([BPC, E], F32, tag="pqi")
                    for j, (x, w) in enumerate(
                            [(xbr[e][:, :, 0], wqr[e][:]) for e in range(4)]
                            + [(x0in[e][:], wqi[e][:]) for e in range(4)]):
                        mm(pqr[:], x, w, start=(j == 0), stop=(j == 7))
                    for j, (x, w) in enumerate(
                            [(xbr[e][:, :, 0], wqi[e][:]) for e in range(4)]
                            + [(xbi[e][:, :, 0], wqr[e][:]) for e in range(4)]):
                        mm(pqi[:], x, w, start=(j == 0), stop=(j == 7))
                    nc.scalar.copy(q0r_sb[:], pqr[:])
                    nc.scalar.copy(q0i_sb[:], pqi[:])

                    # transpose q0 -> bd block-diag [128, 4u, 8]
                    # bd[p*64+d, u, 2b+p] = q0[b, u*128+p*64+d] + bq bias
                    for u in range(4):
                        fs = slice(u * 128, (u + 1) * 128)
                        ptr = psB1.tile([128, 4], F32, tag="ptq", bufs=1, name=f"ptq{u}")
                        pti = psB1.tile([128, 4], F32, tag="ptj", bufs=1, name=f"ptj{u}")
                        nc.tensor.transpose(ptr[:], q0r_sb[:, fs], ident[0:BPC, 0:BPC])
                        nc.tensor.transpose(pti[:], q0i_sb[:, fs], ident[0:BPC, 0:BPC])
                        for p in range(2):
                            rows = slice(p * 64, (p + 1) * 64)
                            cols = slice(2 * u + p, 32, 8)
                            nc.scalar.activation(bd_r[u][rows, cols], ptr[rows, :],
                                                 ACTF.Identity,
                                                 bias=bqr[rows, u:u + 1], scale=1.0)
                            nc.scalar.activation(bd_i[u][rows, cols], pti[rows, :],
                                                 ACTF.Identity,
                                                 bias=bqi[rows, u:u + 1], scale=1.0)
                            nc.scalar.activation(bd_in[u][rows, cols], pti[rows, :],
                                                 ACTF.Identity,
                                                 bias=bqin[rows, u:u + 1], scale=-1.0)

                # next v batch fills PE while bd copies run
                emit_v(1)

                # ============ qk = bd^T @ wk  [rows b*8 + 2u+p, 512e] ============
                with tc.tile_pool(name="psQK", bufs=1, space="PSUM") as psQK:
                    pkr = psQK.tile([32, E], F32, tag="pkr")
                    pki = psQK.tile([32, E], F32, tag="pki")
                    for j, (bdt, w) in enumerate(
                            [(bd_r[u], wkr[u]) for u in range(4)]
                            + [(bd_in[u], wki[u]) for u in range(4)]):
                        mm(pkr[:], bdt[:], w[:], start=(j == 0), stop=(j == 7))
                    for j, (bdt, w) in enumerate(
                            [(bd_r[u], wki[u]) for u in range(4)]
                            + [(bd_i[u], wkr[u]) for u in range(4)]):
                        mm(pki[:], bdt[:], w[:], start=(j == 0), stop=(j == 7))
                    nc.vector.tensor_copy(qk_sb_r[:], pkr[:])
                    nc.scalar.copy(qk_sb_i[:], pki[:])

                # v b2 + vC fill PE during qk copies / qkT transposes
                emit_v(2)

                # vC: token-256 v row for all batches
                with tc.tile_pool(name="psVC", bufs=1, space="PSUM") as psVC:
                    p1 = psVC.tile([BPC, OUT], F32, tag="pc1")
                    pi = psVC.tile([BPC, OUT], F32, tag="pci")
                    for j, (x, w) in enumerate(
                            [(xbr[e][:, :, 256], wvr[e]) for e in range(4)]
                            + [(xbi[e][:, :, 256], wvin[e]) for e in range(4)]):
                        mm(p1[:], x, w[:], start=(j == 0), stop=(j == 7))
                    for j, (x, w) in enumerate(
                            [(xbr[e][:, :, 256], wvi[e]) for e in range(4)]
                            + [(xbi[e][:, :, 256], wvr[e]) for e in range(4)]):
                        mm(pi[:], x, w[:], start=(j == 0), stop=(j == 7))
                    nc.vector.tensor_copy(vCr_sb[:], p1[:])
                    nc.scalar.copy(vCi_sb[:], pi[:])

                # transpose qk -> qkT [128e, 4u, 8] (+ negated imag)
                with tc.tile_pool(name="psQT", bufs=2, space="PSUM") as psQT:
                    for e in range(4):
                        es = slice(e * 128, (e + 1) * 128)
                        ptr = psQT.tile([128, 32], F32, tag="qtr", name=f"qtr{e}")
                        pti = psQT.tile([128, 32], F32, tag="qti", name=f"qti{e}")
                        nc.tensor.transpose(ptr[:], qk_sb_r[:, es], ident[0:32, 0:32])
                        nc.tensor.transpose(pti[:], qk_sb_i[:, es], ident[0:32, 0:32])
                        nc.scalar.copy(qkT_r[e][:], ptr[:])
                        nc.scalar.copy(qkT_i[e][:], pti[:])
                        nc.vector.tensor_scalar_mul(qkT_in[e][:], pti[:], -1.0)

                # last v batch fills PE while qkT copies land
                emit_v(3)

            # ============ per-batch pipeline: logits -> softmax -> wT -> hv
            # (hv of batch b overlaps logits of batch b+1 on PE) ============
            with tc.tile_pool(name="miscB2", bufs=1) as mb:
                # vC2[p, b, :]: rows (re, im); vC2s rows (im, re)
                vC2 = mb.tile([2, BPC, OUT], BF16)
                vC2s = mb.tile([2, BPC, OUT], BF16)
                nc.sync.dma_start(out=vC2[0:1, :, :], in_=vCr_sb[:])
                nc.sync.dma_start(out=vC2[1:2, :, :], in_=vCi_sb[:])
                nc.sync.dma_start(out=vC2s[0:1, :, :], in_=vCi_sb[:])
                nc.sync.dma_start(out=vC2s[1:2, :, :], in_=vCr_sb[:])

                w_b = [mb.tile([8, 2, S], F32, name=f"w_b{b}") for b in range(BPC)]
                wTr = [mb.tile([128, 32], BF16, name=f"wTr{a}") for a in range(2)]
                wTi = [mb.tile([128, 32], BF16, name=f"wTi{a}") for a in range(2)]
                wTin = [mb.tile([128, 32], BF16, name=f"wTin{a}") for a in range(2)]
                wtc_a = mb.tile([2, 32], BF16)   # rows: wTr_c, -wTi_c
                wtc_b = mb.tile([2, 32], BF16)   # rows: wTr_c, wTi_c
                wtc_neg = mb.tile([2, 32], BF16)
                hvm_r = [mb.tile([NH, OUT], BF16, name=f"hvm_r{b}") for b in range(BPC)]
                hvm_i = [mb.tile([NH, OUT], BF16, name=f"hvm_i{b}") for b in range(BPC)]
                hvm_all_r = mb.tile([32, OUT], BF16)
                hvm_all_i = mb.tile([32, OUT], BF16)

                with tc.tile_pool(name="psB2", bufs=2, space="PSUM") as psB2, \
                     tc.tile_pool(name="psB3", bufs=1, space="PSUM") as psB3, \
                     tc.tile_pool(name="psB4", bufs=2, space="PSUM") as psB4:
                    # pw[:, 0:2, :] = wT re s-halves; [:, 2:4, :] = im; ptc sep
                    pw = psB3.tile([128, 4, 32], F32, tag="pw")
                    ptc = psB3.tile([2, 32], F32, tag="ptc")

                    def softmax(b, ri, psum):
                        # logits are O(+-8): exp safe in f32 without max-shift
                        sm = mb.tile([8, 1], F32, tag="ssm", name=f"sm{b}_{ri}")
                        rs = mb.tile([8, 1], F32, tag="srs", name=f"rs{b}_{ri}")
                        nc.scalar.activation(w_b[b][:, ri, :], psum[:, 0:S],
                                             ACTF.Exp, bias=0.0, scale=1.0,
                                             accum_out=sm[:])
                        nc.vector.reciprocal(rs[:], sm[:])
                        nc.vector.tensor_scalar_mul(w_b[b][:, ri, :],
                                                    w_b[b][:, ri, :], rs[:])

                    def emit_logits(b):
                        bcols = slice(b * 8, b * 8 + 8)
                        pr = psB2.tile([8, SP], F32, tag="plg", name=f"plgr{b}")
                        for j, (q, x) in enumerate(
                                [(qkT_r[e][:, bcols], xbr[e][:, b, :])
                                 for e in range(4)]
                                + [(qkT_in[e][:, bcols], xbi[e][:, b, :])
                                   for e in range(4)]):
                            mm(pr[:], q, x, start=(j == 0), stop=(j == 7))
                        softmax(b, 0, pr)
                        pq = psB2.tile([8, SP], F32, tag="plg", name=f"plgi{b}")
                        for j, (q, x) in enumerate(
                                [(qkT_r[e][:, bcols], xbi[e][:, b, :])
                                 for e in range(4)]
                                + [(qkT_i[e][:, bcols], xbr[e][:, b, :])
                                   for e in range(4)]):
                            mm(pq[:], q, x, start=(j == 0), stop=(j == 7))
                        softmax(b, 1, pq)

                    def emit_attn(b):
                        bcols = slice(b * 8, b * 8 + 8)
                        # -- transpose w -> wT columns for this batch --
                        for a in range(2):
                            cs = slice(a * 128, (a + 1) * 128)
                            for ri in range(2):
                                nc.tensor.matmul(pw[:, 2 * ri + a, bcols],
                                                 w_b[b][:, ri, cs],
                                                 ident[0:8, 0:8],
                                                 is_transpose=True,
                                                 skip_group_check=True)
                        nc.tensor.matmul(ptc[:, bcols], w_b[b][:, :, 256],
                                         ident[0:8, 0:8], is_transpose=True,
                                         skip_group_check=True)
                        for a in range(2):
                            nc.scalar.copy(wTr[a][:, bcols], pw[:, a, bcols])
                            nc.scalar.copy(wTi[a][:, bcols], pw[:, 2 + a, bcols])
                            nc.scalar.activation(wTin[a][:, bcols],
                                                 pw[:, 2 + a, bcols],
                                                 ACTF.Copy, bias=0.0, scale=-1.0)
                        nc.scalar.copy(wtc_b[:, bcols], ptc[:, bcols])
                        nc.scalar.activation(wtc_neg[:, bcols], ptc[:, bcols],
                                             ACTF.Copy, bias=0.0, scale=-1.0)
                        nc.sync.dma_start(out=wtc_a[0:1, bcols],
                                          in_=wtc_b[0:1, bcols])
                        nc.sync.dma_start(out=wtc_a[1:2, bcols],
                                          in_=wtc_neg[1:2, bcols])
                        # -- hv --
                        ph_r = psB4.tile([NH, OUT], F32, tag="phr", name=f"phr{b}")
                        ph_i = psB4.tile([NH, OUT], F32, tag="phi", name=f"phi{b}")
                        mm(ph_r[:], wTr[0][:, bcols], vr[b][0][:], start=True, stop=False)
                        mm(ph_r[:], wTr[1][:, bcols], vr[b][1][:], start=False, stop=False)
                        mm(ph_r[:], wTin[0][:, bcols], vi[b][0][:], start=False, stop=False)
                        mm(ph_r[:], wTin[1][:, bcols], vi[b][1][:], start=False, stop=False)
                        mm(ph_r[:], wtc_a[:, bcols], vC2[:, b, :], start=False, stop=True)
                        mm(ph_i[:], wTi[0][:, bcols], vr[b][0][:], start=True, stop=False)
                        mm(ph_i[:], wTi[1][:, bcols], vr[b][1][:], start=False, stop=False)
                        mm(ph_i[:], wTr[0][:, bcols], vi[b][0][:], start=False, stop=False)
                        mm(ph_i[:], wTr[1][:, bcols], vi[b][1][:], start=False, stop=False)
                        mm(ph_i[:], wtc_b[:, bcols], vC2s[:, b, :], start=False, stop=True)
                        nc.vector.tensor_mul(hvm_r[b][:], ph_r[:], mask8[:])
                        nc.vector.tensor_mul(hvm_i[b][:], ph_i[:], mask8[:])
                        nc.sync.dma_start(out=hvm_all_r[b * 8:b * 8 + 8, :],
                                          in_=hvm_r[b][:])
                        nc.sync.dma_start(out=hvm_all_i[b * 8:b * 8 + 8, :],
                                          in_=hvm_i[b][:])

                    # software-pipelined: logits of b+1 issue before attn of b
                    # so PE never waits on softmax
                    emit_logits(0)
                    emit_logits(1)
                    emit_attn(0)
                    emit_logits(2)
                    emit_attn(1)
                    emit_logits(3)
                    emit_attn(2)
                    emit_attn(3)

                # ---- extract attn0^T [128, 4] per f-tile via selection matmul ----
                att_r = [mb.tile([128, 4], BF16, name=f"att_r{u}") for u in range(4)]
                att_i = [mb.tile([128, 4], BF16, name=f"att_i{u}") for u in range(4)]
                att_in = [mb.tile([128, 4], BF16, name=f"att_in{u}") for u in range(4)]
                with tc.tile_pool(name="psB5", bufs=2, space="PSUM") as psB5:
                    for u in range(4):
                        fs = slice(u * 128, (u + 1) * 128)
                        par = psB5.tile([128, 4], F32, tag="par", name=f"par{u}")
                        pai = psB5.tile([128, 4], F32, tag="pai", name=f"pai{u}")
                        mm(par[:], hvm_all_r[:, fs], sel32[:], start=True, stop=True)
                        mm(pai[:], hvm_all_i[:, fs], sel32[:], start=True, stop=True)
                        nc.scalar.copy(att_r[u][:], par[:])
                        nc.scalar.copy(att_i[u][:], pai[:])
                        nc.scalar.activation(att_in[u][:], pai[:], ACTF.Copy,
                                             bias=0.0, scale=-1.0)

                # ---- y = attn0 @ Wc^T + b_c ----
                yr_sb = mb.tile([BPC, OUT], F32)
                yi_sb = mb.tile([BPC, OUT], F32)
                with tc.tile_pool(name="psB6", bufs=1, space="PSUM") as psB6:
                    py_r = psB6.tile([BPC, OUT], F32, tag="pyr")
                    py_i = psB6.tile([BPC, OUT], F32, tag="pyi")
                    for j, u in enumerate(range(4)):
                        mm(py_r[:], att_r[u][:], wcr[u][:], start=(j == 0), stop=False)
                        mm(py_r[:], att_in[u][:], wci[u][:], start=False, stop=(j == 3))
                        mm(py_i[:], att_r[u][:], wci[u][:], start=(j == 0), stop=False)
                        mm(py_i[:], att_i[u][:], wcr[u][:], start=False, stop=(j == 3))
                    nc.vector.tensor_add(yr_sb[:], py_r[:], bcr[:])
                    nc.vector.tensor_add(yi_sb[:], py_i[:], bci[:])
                    nc.sync.dma_start(out=d_yr.ap(), in_=yr_sb[:])
                    nc.sync.dma_start(out=d_yi.ap(), in_=yi_sb[:])

    nc.compile()
    return nc


def _host_prep(inputs):
    """Build per-core in_maps from the full inputs."""
    import ml_dtypes
    f32 = np.float32
    bf16 = ml_dtypes.bfloat16
    xr = np.ascontiguousarray(inputs["x_real"], dtype=f32).reshape(B, E, HW)
    xi = np.ascontiguousarray(inputs["x_imag"], dtype=f32).reshape(B, E, HW)
    pos = np.asarray(inputs["pos_r"], dtype=f32) + 1j * np.asarray(inputs["pos_i"], dtype=f32)
    w_in_r = np.asarray(inputs["w_in_r"], dtype=f32)
    w_in_i = np.asarray(inputs["w_in_i"], dtype=f32)
    b_in_r = np.asarray(inputs["b_in_r"], dtype=f32)
    b_in_i = np.asarray(inputs["b_in_i"], dtype=f32)
    w_out = np.asarray(inputs["w_out_r"], dtype=f32) + 1j * np.asarray(inputs["w_out_i"], dtype=f32)
    b_out = np.asarray(inputs["b_out_r"], dtype=f32) + 1j * np.asarray(inputs["b_out_i"], dtype=f32)
    w_p = np.asarray(inputs["w_p_r"], dtype=f32) + 1j * np.asarray(inputs["w_p_i"], dtype=f32)
    b_p = np.asarray(inputs["b_p_r"], dtype=f32) + 1j * np.asarray(inputs["b_p_i"], dtype=f32)

    w_in = w_in_r + 1j * w_in_i
    wq, wk, wv = w_in[:E], w_in[E:2 * E], w_in[2 * E:]
    qs = f32(1.0 / np.sqrt(HD))

    posb = np.zeros((E, SP), np.complex64)
    posb[:, :S] = pos

    wc = w_p @ w_out                                        # [OUT, E] complex
    bq = qs * (b_in_r[:E] + 1j * b_in_i[:E])                # [E]

    b_v = b_in_r[2 * E:] + 1j * b_in_i[2 * E:]
    b_c = (1 + 1j) * (b_v @ wc.T) + b_out @ w_p.T + b_p     # [OUT] complex

    mask8 = np.zeros((NH, OUT), f32)
    for h in range(NH):
        mask8[h, h * HD:(h + 1) * HD] = 1.0
    sel32 = np.zeros((32, BPC), f32)
    for b in range(BPC):
        sel32[b * 8:(b + 1) * 8, b] = 1.0

    shared = dict(
        wqr=np.ascontiguousarray(wq.real.T * qs).astype(bf16),
        wqi=np.ascontiguousarray(wq.imag.T * qs).astype(bf16),
        wkr=np.ascontiguousarray(wk.real).astype(bf16),
        wki=np.ascontiguousarray(wk.imag).astype(bf16),
        wvr=np.ascontiguousarray(wv.real.T).astype(bf16),
        wvi=np.ascontiguousarray(wv.imag.T).astype(bf16),
        wcr=np.ascontiguousarray(wc.real.T.astype(f32)).astype(bf16),
        wci=np.ascontiguousarray(wc.imag.T.astype(f32)).astype(bf16),
        bqr=bq.real.astype(f32).reshape(4, 128).T.copy(),
        bqi=bq.imag.astype(f32).reshape(4, 128).T.copy(),
        bcr=np.broadcast_to(b_c.real.astype(f32), (BPC, OUT)).copy(),
        bci=np.broadcast_to(b_c.imag.astype(f32), (BPC, OUT)).copy(),
        ident=np.eye(128, dtype=f32),
        mask8=mask8,
        sel32=sel32.astype(bf16),
        zbd=np.zeros((128, 32), bf16),
    )
    # x_cat fully prepped on host: col 0 = mean, then + pos; col 257 zero
    xrp = np.zeros((B, E, SP), f32)
    xip = np.zeros((B, E, SP), f32)
    xrp[:, :, 1:1 + HW] = xr
    xip[:, :, 1:1 + HW] = xi
    xrp[:, :, 0] = xr.mean(-1)
    xip[:, :, 0] = xi.mean(-1)
    xrp[:, :, :S] += posb.real[None, :, :S]
    xip[:, :, :S] += posb.imag[None, :, :S]
    in_maps = []
    for c in range(NCORES):
        m = dict(shared)
        m["xr"] = np.ascontiguousarray(
            xrp[c * BPC:(c + 1) * BPC].transpose(1, 0, 2)).astype(bf16)
        m["xi"] = np.ascontiguousarray(
            xip[c * BPC:(c + 1) * BPC].transpose(1, 0, 2)).astype(bf16)
        in_maps.append(m)
    return in_maps


def _run(inputs, trace=False, **kw):
    from concourse.bass_utils import run_bass_kernel_spmd
    if "nc" not in _cached:
        _cached["nc"] = _build()
    nc = _cached["nc"]
    in_maps = _host_prep(inputs)
    res = run_bass_kernel_spmd(nc, in_maps, core_ids=list(range(NCORES)),
                               trace=trace, **kw)
    out = np.empty((B, OUT), np.complex64)
    for c in range(NCORES):
        out[c * BPC:(c + 1) * BPC] = (res.results[c]["yr"]
                                      + 1j * res.results[c]["yi"])
    return out, res


def kernel(**inputs) -> np.ndarray:
    out, _ = _run(inputs)
    return out


# revision 36
# speedup vs baseline: 1.2172x; 1.0419x over previous
"""Complex AttentionPool2d on 8 trn2 NeuronCores, data-parallel over batch.

Contract: kernel(**inputs) takes the FULL inputs from setup_inputs() and
returns the FULL [32, 512] complex64 output.

V2: all matmuls bf16 (fp32 PSUM accum); k^T eliminated algebraically.
Math (per batch):
  x = bf16(complex(x_real, x_imag)).reshape(E, 256)
  x_cat = [mean(x), x] + pos                       # [E, 257]
  q0 = x_cat[:, 0] @ wq^T + bq                     # only query pos 0 matters
  qk[h, e] = sum_d q0[h*64+d] wk[h*64+d, e]        # fold q into k-proj
  logits[h, s] = sum_e qk[h, e] x_cat[e, s]        # == q0 . k[s]
  w = softmax(logits.re) + i*softmax(logits.im)
  v = x_cat^T @ wv^T                               # [257, 512]
  attn0 = (w @ v) per-head masked; y = attn0 @ (w_p @ w_out)^T + b_c

Sharding: batch 32 -> 4 per core.
"""
import numpy as np

B, E, HW, S = 32, 512, 256, 257
SP = 258            # S padded even
NH, HD = 8, 64
OUT = 512
NCORES = 8
BPC = B // NCORES   # batches per core

_cached = {}


def _build():
    import concourse.bacc as bacc
    import concourse.tile as tile
    import concourse.mybir as mybir

    F32 = mybir.dt.float32
    BF16 = mybir.dt.bfloat16
    AX = mybir.AxisListType
    ACTF = mybir.ActivationFunctionType

    nc = bacc.Bacc("TRN2", target_bir_lowering=False, debug=False)

    # ---- DRAM I/O ----
    # x layout: [E, BPC, SP] so one DMA per e-tile covers all 4 batches;
    # col 0 reserved for the mean token, col 257 zero pad
    d_xr = nc.dram_tensor("xr", [E, BPC, SP], BF16, kind="ExternalInput")
    d_xi = nc.dram_tensor("xi", [E, BPC, SP], BF16, kind="ExternalInput")
    d_wqr = nc.dram_tensor("wqr", [E, E], BF16, kind="ExternalInput")
    d_wqi = nc.dram_tensor("wqi", [E, E], BF16, kind="ExternalInput")
    d_wkr = nc.dram_tensor("wkr", [E, E], BF16, kind="ExternalInput")
    d_wki = nc.dram_tensor("wki", [E, E], BF16, kind="ExternalInput")
    d_wvr = nc.dram_tensor("wvr", [E, OUT], BF16, kind="ExternalInput")
    d_wvi = nc.dram_tensor("wvi", [E, OUT], BF16, kind="ExternalInput")
    d_wcr = nc.dram_tensor("wcr", [E, OUT], BF16, kind="ExternalInput")
    d_wci = nc.dram_tensor("wci", [E, OUT], BF16, kind="ExternalInput")
    d_bqr = nc.dram_tensor("bqr", [128, 4], F32, kind="ExternalInput")
    d_bqi = nc.dram_tensor("bqi", [128, 4], F32, kind="ExternalInput")
    d_bcp = nc.dram_tensor("bcp", [8, OUT], F32, kind="ExternalInput")
    d_id = nc.dram_tensor("ident", [128, 128], F32, kind="ExternalInput")
    d_mask = nc.dram_tensor("mask8", [NH, OUT], F32, kind="ExternalInput")
    d_sel = nc.dram_tensor("sel32", [32, BPC], BF16, kind="ExternalInput")
    d_zbd = nc.dram_tensor("zbd", [128, 32], BF16, kind="ExternalInput")
    d_yr = nc.dram_tensor("yr", [BPC, OUT], F32, kind="ExternalOutput")
    d_yi = nc.dram_tensor("yi", [BPC, OUT], F32, kind="ExternalOutput")

    with tile.TileContext(nc) as tc:
        with tc.tile_pool(name="consts", bufs=1) as consts, \
             tc.tile_pool(name="vpool", bufs=1) as vpool:
            # ---- persistent weights / constants (bf16) ----
            wvr = [consts.tile([128, OUT], BF16, name=f"wvr{e}") for e in range(4)]
            wvi = [consts.tile([128, OUT], BF16, name=f"wvi{e}") for e in range(4)]
            wvin = [consts.tile([128, OUT], BF16, name=f"wvin{e}") for e in range(4)]
            wqr = [consts.tile([128, E], BF16, name=f"wqr{e}") for e in range(4)]
            wqi = [consts.tile([128, E], BF16, name=f"wqi{e}") for e in range(4)]
            wkr = [consts.tile([128, E], BF16, name=f"wkr{e}") for e in range(4)]
            wki = [consts.tile([128, E], BF16, name=f"wki{e}") for e in range(4)]
            wcr = [consts.tile([128, OUT], BF16, name=f"wcr{e}") for e in range(4)]
            wci = [consts.tile([128, OUT], BF16, name=f"wci{e}") for e in range(4)]
            bqr = consts.tile([128, 4], F32)
            bqi = consts.tile([128, 4], F32)
            bqin = consts.tile([128, 4], F32)
            bcp = consts.tile([8, OUT], F32)
            ident = consts.tile([128, 128], F32)
            mask8 = consts.tile([NH, OUT], F32)
            sel32 = consts.tile([32, BPC], BF16)

            # x tiles: [128e, BPC, SP]
            xbr = [vpool.tile([128, BPC, SP], BF16, name=f"xbr{e}") for e in range(4)]
            xbi = [vpool.tile([128, BPC, SP], BF16, name=f"xbi{e}") for e in range(4)]
            # paired lhsT tiles: [x0r || x0i] and [-x0i || x0r] etc, so one
            # matmul pass produces both complex parts (rows 0:4 re, 4:8 im)
            x0pa = [vpool.tile([128, 8], BF16, name=f"x0pa{e}") for e in range(4)]
            x0pb = [vpool.tile([128, 8], BF16, name=f"x0pb{e}") for e in range(4)]
            xlpa = [vpool.tile([128, 8], BF16, name=f"xlpa{e}") for e in range(4)]
            xlpb = [vpool.tile([128, 8], BF16, name=f"xlpb{e}") for e in range(4)]
            # v tiles live until hv
            vr = [[vpool.tile([128, OUT], BF16, name=f"vr{b}_{s}")
                   for s in range(2)] for b in range(BPC)]
            vi = [[vpool.tile([128, OUT], BF16, name=f"vi{b}_{s}")
                   for s in range(2)] for b in range(BPC)]
            vC_sb = vpool.tile([8, OUT], BF16)
            # bd: per-u zero-padded block-diag q0 [128, 32] (cols b*8+2u+p)
            bd_r = [vpool.tile([128, 32], BF16, name=f"bd_r{u}") for u in range(4)]
            bd_i = [vpool.tile([128, 32], BF16, name=f"bd_i{u}") for u in range(4)]
            bd_in = [vpool.tile([128, 32], BF16, name=f"bd_in{u}") for u in range(4)]
            q0p_sb = vpool.tile([8, E], F32)
            qk_sb_r = vpool.tile([32, E], F32)
            qk_sb_i = vpool.tile([32, E], F32)
            qkT_r = [vpool.tile([128, 32], BF16, name=f"qkTr{e}") for e in range(4)]
            qkT_i = [vpool.tile([128, 32], BF16, name=f"qkTi{e}") for e in range(4)]
            qkT_in = [vpool.tile([128, 32], BF16, name=f"qkTin{e}") for e in range(4)]

            # ---- DMA issue order matters per queue ----
            # sync queue: small consts then x real
            nc.sync.dma_start(out=ident, in_=d_id.ap())
            nc.sync.dma_start(out=sel32, in_=d_sel.ap())
            nc.sync.dma_start(out=mask8, in_=d_mask.ap())
            nc.sync.dma_start(out=bqr, in_=d_bqr.ap())
            nc.sync.dma_start(out=bqi, in_=d_bqi.ap())
            nc.sync.dma_start(out=bcp, in_=d_bcp.ap())
            for u in range(4):
                nc.sync.dma_start(out=bd_r[u], in_=d_zbd.ap())
                nc.sync.dma_start(out=bd_i[u], in_=d_zbd.ap())
                nc.sync.dma_start(out=bd_in[u], in_=d_zbd.ap())
            # Two hw DMA queues (scalar, gpsimd), interleaved in first-need
            # order; weights are pre-tiled [128, 4, X] on host so every DMA
            # is contiguous per partition.
            for e in range(4):
                sl = slice(e * 128, (e + 1) * 128)
                nc.scalar.dma_start(out=xbr[e][:], in_=d_xr.ap()[sl, :, :])
                nc.scalar.dma_start(out=wvr[e], in_=d_wvr.ap()[sl, :])
                if e == 0:
                    nc.scalar.dma_start(out=wqr[0], in_=d_wqr.ap()[0:128, :])
                nc.gpsimd.dma_start(out=xbi[e][:], in_=d_xi.ap()[sl, :, :])
                nc.gpsimd.dma_start(out=wvi[e], in_=d_wvi.ap()[sl, :])
            for e in range(4):
                sl = slice(e * 128, (e + 1) * 128)
                if e > 0:
                    nc.scalar.dma_start(out=wqr[e], in_=d_wqr.ap()[sl, :])
                nc.scalar.dma_start(out=wqi[e], in_=d_wqi.ap()[sl, :])
            for e in range(4):
                sl = slice(e * 128, (e + 1) * 128)
                nc.gpsimd.dma_start(out=wkr[e], in_=d_wkr.ap()[sl, :])
                nc.gpsimd.dma_start(out=wki[e], in_=d_wki.ap()[sl, :])
            for e in range(4):
                sl = slice(e * 128, (e + 1) * 128)
                nc.scalar.dma_start(out=wcr[e], in_=d_wcr.ap()[sl, :])
                nc.gpsimd.dma_start(out=wci[e], in_=d_wci.ap()[sl, :])

            nc.vector.tensor_scalar_mul(bqin, bqi, -1.0)
            # negate wv imag on device (keeps it off the DMA critical path)
            for e in range(4):
                nc.vector.tensor_scalar_mul(wvin[e][:], wvi[e][:], -1.0)
            # x arrives fully prepped from host (mean in col 0, pos added);
            # build the paired token-0 / token-256 lhsT tiles
            for e in range(4):
                nc.scalar.copy(x0pa[e][:, 0:4], xbr[e][:, :, 0])
                nc.scalar.copy(x0pa[e][:, 4:8], xbi[e][:, :, 0])
                nc.scalar.activation(x0pb[e][:, 0:4], xbi[e][:, :, 0],
                                     ACTF.Copy, bias=0.0, scale=-1.0)
                nc.scalar.copy(x0pb[e][:, 4:8], xbr[e][:, :, 0])
                nc.scalar.copy(xlpa[e][:, 0:4], xbr[e][:, :, 256])
                nc.scalar.copy(xlpa[e][:, 4:8], xbi[e][:, :, 256])
                nc.scalar.activation(xlpb[e][:, 0:4], xbi[e][:, :, 256],
                                     ACTF.Copy, bias=0.0, scale=-1.0)
                nc.scalar.copy(xlpb[e][:, 4:8], xbr[e][:, :, 256])

            mm = nc.tensor.matmul

            with tc.tile_pool(name="psA", bufs=2, space="PSUM") as psA:
                # v rows s in [sb*128, (sb+1)*128) = x cols (col 0 = mean tok)
                def emit_v(b):
                    for sb in range(2):
                        cs = slice(sb * 128, (sb + 1) * 128)
                        p1 = psA.tile([128, OUT], F32, tag="pv1", name=f"pv1_{b}_{sb}")
                        pi = psA.tile([128, OUT], F32, tag="pvi", name=f"pvi_{b}_{sb}")
                        for j, (x, w) in enumerate(
                                [(xbr[e][:, b, cs], wvr[e]) for e in range(4)]
                                + [(xbi[e][:, b, cs], wvin[e]) for e in range(4)]):
                            mm(p1[:], x, w[:], start=(j == 0), stop=(j == 7))
                        for j, (x, w) in enumerate(
                                [(xbr[e][:, b, cs], wvi[e]) for e in range(4)]
                                + [(xbi[e][:, b, cs], wvr[e]) for e in range(4)]):
                            mm(pi[:], x, w[:], start=(j == 0), stop=(j == 7))
                        nc.vector.tensor_copy(vr[b][sb][:], p1[:])
                        nc.scalar.copy(vi[b][sb][:], pi[:])

                emit_v(0)

                # ============ q0 -> bd ============
                with tc.tile_pool(name="psB1", bufs=1, space="PSUM") as psB1:
                    pq = psB1.tile([8, E], F32, tag="pq")
                    for j, (x, w) in enumerate(
                            [(x0pa[e][:], wqr[e][:]) for e in range(4)]
                            + [(x0pb[e][:], wqi[e][:]) for e in range(4)]):
                        mm(pq[:], x, w, start=(j == 0), stop=(j == 7))
                    nc.scalar.copy(q0p_sb[:], pq[:])

                    # transpose q0 pair -> bd block-diag; transpose cols 0:4
                    # are q0r batches, 4:8 are q0i
                    for u in range(4):
                        fs = slice(u * 128, (u + 1) * 128)
                        ptq = psB1.tile([128, 8], F32, tag="ptq", bufs=2, name=f"ptq{u}")
                        nc.tensor.transpose(ptq[:], q0p_sb[:, fs], ident[0:8, 0:8])
                        for p in range(2):
                            rows = slice(p * 64, (p + 1) * 64)
                            cols = slice(2 * u + p, 32, 8)
                            nc.scalar.activation(bd_r[u][rows, cols], ptq[rows, 0:4],
                                                 ACTF.Identity,
                                                 bias=bqr[rows, u:u + 1], scale=1.0)
                            nc.scalar.activation(bd_i[u][rows, cols], ptq[rows, 4:8],
                                                 ACTF.Identity,
                                                 bias=bqi[rows, u:u + 1], scale=1.0)
                            nc.scalar.activation(bd_in[u][rows, cols], ptq[rows, 4:8],
                                                 ACTF.Identity,
                                                 bias=bqin[rows, u:u + 1], scale=-1.0)

                # next v batch fills PE while bd copies run
                emit_v(1)

                # ============ qk = bd^T @ wk  [rows b*8 + 2u+p, 512e] ============
                with tc.tile_pool(name="psQK", bufs=1, space="PSUM") as psQK:
                    pkr = psQK.tile([32, E], F32, tag="pkr")
                    pki = psQK.tile([32, E], F32, tag="pki")
                    for j, (bdt, w) in enumerate(
                            [(bd_r[u], wkr[u]) for u in range(4)]
                            + [(bd_in[u], wki[u]) for u in range(4)]):
                        mm(pkr[:], bdt[:], w[:], start=(j == 0), stop=(j == 7))
                    for j, (bdt, w) in enumerate(
                            [(bd_r[u], wki[u]) for u in range(4)]
                            + [(bd_i[u], wkr[u]) for u in range(4)]):
                        mm(pki[:], bdt[:], w[:], start=(j == 0), stop=(j == 7))
                    nc.vector.tensor_copy(qk_sb_r[:], pkr[:])
                    nc.scalar.copy(qk_sb_i[:], pki[:])

                # v b2 + vC fill PE during qk copies / qkT transposes
                emit_v(2)

                # vC: token-256 v row for all batches
                with tc.tile_pool(name="psVC", bufs=1, space="PSUM") as psVC:
                    pvc = psVC.tile([8, OUT], F32, tag="pvc")
                    for j, (x, w) in enumerate(
                            [(xlpa[e][:], wvr[e]) for e in range(4)]
                            + [(xlpb[e][:], wvi[e]) for e in range(4)]):
                        mm(pvc[:], x, w[:], start=(j == 0), stop=(j == 7))
                    nc.vector.tensor_copy(vC_sb[:], pvc[:])

                # transpose qk -> qkT [128e, 4u, 8] (+ negated imag)
                with tc.tile_pool(name="psQT", bufs=2, space="PSUM") as psQT:
                    for e in range(4):
                        es = slice(e * 128, (e + 1) * 128)
                        ptr = psQT.tile([128, 32], F32, tag="qtr", name=f"qtr{e}")
                        pti = psQT.tile([128, 32], F32, tag="qti", name=f"qti{e}")
                        nc.tensor.transpose(ptr[:], qk_sb_r[:, es], ident[0:32, 0:32])
                        nc.tensor.transpose(pti[:], qk_sb_i[:, es], ident[0:32, 0:32])
                        nc.scalar.copy(qkT_r[e][:], ptr[:])
                        nc.scalar.copy(qkT_i[e][:], pti[:])
                        nc.vector.tensor_scalar_mul(qkT_in[e][:], pti[:], -1.0)

                # last v batch fills PE while qkT copies land
                emit_v(3)

            # ============ per-batch pipeline: logits -> softmax -> wT -> hv
            # (hv of batch b overlaps logits of batch b+1 on PE) ============
            with tc.tile_pool(name="miscB2", bufs=1) as mb:
                # vC2[p, b, :]: rows (re, im); vC2s rows (im, re)
                vC2 = mb.tile([2, BPC, OUT], BF16)
                vC2s = mb.tile([2, BPC, OUT], BF16)
                nc.sync.dma_start(out=vC2[0:1, :, :], in_=vC_sb[0:4, :])
                nc.sync.dma_start(out=vC2[1:2, :, :], in_=vC_sb[4:8, :])
                nc.sync.dma_start(out=vC2s[0:1, :, :], in_=vC_sb[4:8, :])
                nc.sync.dma_start(out=vC2s[1:2, :, :], in_=vC_sb[0:4, :])

                w_b = [mb.tile([8, 2, S], F32, name=f"w_b{b}") for b in range(BPC)]
                wTr = [mb.tile([128, 32], BF16, name=f"wTr{a}") for a in range(2)]
                wTi = [mb.tile([128, 32], BF16, name=f"wTi{a}") for a in range(2)]
                wTin = [mb.tile([128, 32], BF16, name=f"wTin{a}") for a in range(2)]
                wtc_a = mb.tile([2, 32], BF16)   # rows: wTr_c, -wTi_c
                wtc_b = mb.tile([2, 32], BF16)   # rows: wTr_c, wTi_c
                wtc_neg = mb.tile([2, 32], BF16)
                hvm_r = [mb.tile([NH, OUT], BF16, name=f"hvm_r{b}") for b in range(BPC)]
                hvm_i = [mb.tile([NH, OUT], BF16, name=f"hvm_i{b}") for b in range(BPC)]
                hvm_all_r = mb.tile([32, OUT], BF16)
                hvm_all_i = mb.tile([32, OUT], BF16)

                with tc.tile_pool(name="psB2", bufs=2, space="PSUM") as psB2, \
                     tc.tile_pool(name="psB3", bufs=1, space="PSUM") as psB3, \
                     tc.tile_pool(name="psB4", bufs=2, space="PSUM") as psB4:
                    # pw[:, 0:2, :] = wT re s-halves; [:, 2:4, :] = im; ptc sep
                    pw = psB3.tile([128, 4, 32], F32, tag="pw")
                    ptc = psB3.tile([2, 32], F32, tag="ptc")

                    def softmax(b, ri, psum):
                        # logits are O(+-8): exp safe in f32 without max-shift
                        sm = mb.tile([8, 1], F32, tag="ssm", name=f"sm{b}_{ri}")
                        rs = mb.tile([8, 1], F32, tag="srs", name=f"rs{b}_{ri}")
                        nc.scalar.activation(w_b[b][:, ri, :], psum[:, 0:S],
                                             ACTF.Exp, bias=0.0, scale=1.0,
                                             accum_out=sm[:])
                        nc.vector.reciprocal(rs[:], sm[:])
                        nc.vector.tensor_scalar_mul(w_b[b][:, ri, :],
                                                    w_b[b][:, ri, :], rs[:])

                    def emit_logits(b):
                        bcols = slice(b * 8, b * 8 + 8)
                        pr = psB2.tile([8, SP], F32, tag="plg", name=f"plgr{b}")
                        for j, (q, x) in enumerate(
                                [(qkT_r[e][:, bcols], xbr[e][:, b, :])
                                 for e in range(4)]
                                + [(qkT_in[e][:, bcols], xbi[e][:, b, :])
                                   for e in range(4)]):
                            mm(pr[:], q, x, start=(j == 0), stop=(j == 7))
                        softmax(b, 0, pr)
                        pq = psB2.tile([8, SP], F32, tag="plg", name=f"plgi{b}")
                        for j, (q, x) in enumerate(
                                [(qkT_r[e][:, bcols], xbi[e][:, b, :])
                                 for e in range(4)]
                                + [(qkT_i[e][:, bcols], xbr[e][:, b, :])
                                   for e in range(4)]):
                            mm(pq[:], q, x, start=(j == 0), stop=(j == 7))
                        softmax(b, 1, pq)

                    def emit_attn(b):
                        bcols = slice(b * 8, b * 8 + 8)
                        # -- transpose w -> wT columns for this batch --
                        for a in range(2):
                            cs = slice(a * 128, (a + 1) * 128)
                            for ri in range(2):
                                nc.tensor.matmul(pw[:, 2 * ri + a, bcols],
                                                 w_b[b][:, ri, cs],
                                                 ident[0:8, 0:8],
                                                 is_transpose=True,
                                                 skip_group_check=True)
                        nc.tensor.matmul(ptc[:, bcols], w_b[b][:, :, 256],
                                         ident[0:8, 0:8], is_transpose=True,
                                         skip_group_check=True)
                        for a in range(2):
                            nc.scalar.copy(wTr[a][:, bcols], pw[:, a, bcols])
                            nc.scalar.copy(wTi[a][:, bcols], pw[:, 2 + a, bcols])
                            nc.scalar.activation(wTin[a][:, bcols],
                                                 pw[:, 2 + a, bcols],
                                                 ACTF.Copy, bias=0.0, scale=-1.0)
                        nc.scalar.copy(wtc_b[:, bcols], ptc[:, bcols])
                        nc.scalar.activation(wtc_neg[:, bcols], ptc[:, bcols],
                                             ACTF.Copy, bias=0.0, scale=-1.0)
                        nc.sync.dma_start(out=wtc_a[0:1, bcols],
                                          in_=wtc_b[0:1, bcols])
                        nc.sync.dma_start(out=wtc_a[1:2, bcols],
                                          in_=wtc_neg[1:2, bcols])
                        # -- hv --
                        ph_r = psB4.tile([NH, OUT], F32, tag="phr", name=f"phr{b}")
                        ph_i = psB4.tile([NH, OUT], F32, tag="phi", name=f"phi{b}")
                        mm(ph_r[:], wTr[0][:, bcols], vr[b][0][:], start=True, stop=False)
                        mm(ph_r[:], wTr[1][:, bcols], vr[b][1][:], start=False, stop=False)
                        mm(ph_r[:], wTin[0][:, bcols], vi[b][0][:], start=False, stop=False)
                        mm(ph_r[:], wTin[1][:, bcols], vi[b][1][:], start=False, stop=False)
                        mm(ph_r[:], wtc_a[:, bcols], vC2[:, b, :], start=False, stop=True)
                        mm(ph_i[:], wTi[0][:, bcols], vr[b][0][:], start=True, stop=False)
                        mm(ph_i[:], wTi[1][:, bcols], vr[b][1][:], start=False, stop=False)
                        mm(ph_i[:], wTr[0][:, bcols], vi[b][0][:], start=False, stop=False)
                        mm(ph_i[:], wTr[1][:, bcols], vi[b][1][:], start=False, stop=False)
                        mm(ph_i[:], wtc_b[:, bcols], vC2s[:, b, :], start=False, stop=True)
                        nc.vector.tensor_mul(hvm_r[b][:], ph_r[:], mask8[:])
                        nc.vector.tensor_mul(hvm_i[b][:], ph_i[:], mask8[:])
                        nc.sync.dma_start(out=hvm_all_r[b * 8:b * 8 + 8, :],
                                          in_=hvm_r[b][:])
                        nc.sync.dma_start(out=hvm_all_i[b * 8:b * 8 + 8, :],
                                          in_=hvm_i[b][:])

                    # software-pipelined: logits of b+1 issue before attn of b
                    # so PE never waits on softmax
                    emit_logits(0)
                    emit_logits(1)
                    emit_attn(0)
                    emit_logits(2)
                    emit_attn(1)
                    emit_logits(3)
                    emit_attn(2)
                    emit_attn(3)

                # ---- extract attn0^T [128, 4] per f-tile via selection matmul ----
                att_pa = [mb.tile([128, 8], BF16, name=f"att_pa{u}") for u in range(4)]
                att_pb = [mb.tile([128, 8], BF16, name=f"att_pb{u}") for u in range(4)]
                with tc.tile_pool(name="psB5", bufs=2, space="PSUM") as psB5:
                    for u in range(4):
                        fs = slice(u * 128, (u + 1) * 128)
                        par = psB5.tile([128, 4], F32, tag="par", name=f"par{u}")
                        pai = psB5.tile([128, 4], F32, tag="pai", name=f"pai{u}")
                        mm(par[:], hvm_all_r[:, fs], sel32[:], start=True, stop=True)
                        mm(pai[:], hvm_all_i[:, fs], sel32[:], start=True, stop=True)
                        nc.scalar.copy(att_pa[u][:, 0:4], par[:])
                        nc.scalar.copy(att_pa[u][:, 4:8], pai[:])
                        nc.scalar.activation(att_pb[u][:, 0:4], pai[:], ACTF.Copy,
                                             bias=0.0, scale=-1.0)
                        nc.scalar.copy(att_pb[u][:, 4:8], par[:])

                # ---- y = attn0 @ Wc^T + b_c (paired re/im in one psum) ----
                y_sb = mb.tile([8, OUT], F32)
                with tc.tile_pool(name="psB6", bufs=1, space="PSUM") as psB6:
                    py = psB6.tile([8, OUT], F32, tag="py")
                    for j, u in enumerate(range(4)):
                        mm(py[:], att_pa[u][:], wcr[u][:], start=(j == 0), stop=False)
                        mm(py[:], att_pb[u][:], wci[u][:], start=False, stop=(j == 3))
                    nc.vector.tensor_add(y_sb[:], py[:], bcp[:])
                    nc.sync.dma_start(out=d_yr.ap(), in_=y_sb[0:4, :])
                    nc.sync.dma_start(out=d_yi.ap(), in_=y_sb[4:8, :])

    nc.compile()
    return nc


def _host_prep(inputs):
    """Build per-core in_maps from the full inputs."""
    import ml_dtypes
    f32 = np.float32
    bf16 = ml_dtypes.bfloat16
    xr = np.ascontiguousarray(inputs["x_real"], dtype=f32).reshape(B, E, HW)
    xi = np.ascontiguousarray(inputs["x_imag"], dtype=f32).reshape(B, E, HW)
    pos = np.asarray(inputs["pos_r"], dtype=f32) + 1j * np.asarray(inputs["pos_i"], dtype=f32)
    w_in_r = np.asarray(inputs["w_in_r"], dtype=f32)
    w_in_i = np.asarray(inputs["w_in_i"], dtype=f32)
    b_in_r = np.asarray(inputs["b_in_r"], dtype=f32)
    b_in_i = np.asarray(inputs["b_in_i"], dtype=f32)
    w_out = np.asarray(inputs["w_out_r"], dtype=f32) + 1j * np.asarray(inputs["w_out_i"], dtype=f32)
    b_out = np.asarray(inputs["b_out_r"], dtype=f32) + 1j * np.asarray(inputs["b_out_i"], dtype=f32)
    w_p = np.asarray(inputs["w_p_r"], dtype=f32) + 1j * np.asarray(inputs["w_p_i"], dtype=f32)
    b_p = np.asarray(inputs["b_p_r"], dtype=f32) + 1j * np.asarray(inputs["b_p_i"], dtype=f32)

    w_in = w_in_r + 1j * w_in_i
    wq, wk, wv = w_in[:E], w_in[E:2 * E], w_in[2 * E:]
    qs = f32(1.0 / np.sqrt(HD))

    posb = np.zeros((E, SP), np.complex64)
    posb[:, :S] = pos

    wc = w_p @ w_out                                        # [OUT, E] complex
    bq = qs * (b_in_r[:E] + 1j * b_in_i[:E])                # [E]

    b_v = b_in_r[2 * E:] + 1j * b_in_i[2 * E:]
    b_c = (1 + 1j) * (b_v @ wc.T) + b_out @ w_p.T + b_p     # [OUT] complex

    mask8 = np.zeros((NH, OUT), f32)
    for h in range(NH):
        mask8[h, h * HD:(h + 1) * HD] = 1.0
    sel32 = np.zeros((32, BPC), f32)
    for b in range(BPC):
        sel32[b * 8:(b + 1) * 8, b] = 1.0

    shared = dict(
        wqr=np.ascontiguousarray(wq.real.T * qs).astype(bf16),
        wqi=np.ascontiguousarray(wq.imag.T * qs).astype(bf16),
        wkr=np.ascontiguousarray(wk.real).astype(bf16),
        wki=np.ascontiguousarray(wk.imag).astype(bf16),
        wvr=np.ascontiguousarray(wv.real.T).astype(bf16),
        wvi=np.ascontiguousarray(wv.imag.T).astype(bf16),
        wcr=np.ascontiguousarray(wc.real.T.astype(f32)).astype(bf16),
        wci=np.ascontiguousarray(wc.imag.T.astype(f32)).astype(bf16),
        bqr=bq.real.astype(f32).reshape(4, 128).T.copy(),
        bqi=bq.imag.astype(f32).reshape(4, 128).T.copy(),
        bcp=np.concatenate([
            np.broadcast_to(b_c.real.astype(f32), (BPC, OUT)),
            np.broadcast_to(b_c.imag.astype(f32), (BPC, OUT))]).copy(),
        ident=np.eye(128, dtype=f32),
        mask8=mask8,
        sel32=sel32.astype(bf16),
        zbd=np.zeros((128, 32), bf16),
    )
    # x_cat fully prepped on host: col 0 = mean, then + pos; col 257 zero
    xrp = np.zeros((B, E, SP), f32)
    xip = np.zeros((B, E, SP), f32)
    xrp[:, :, 1:1 + HW] = xr
    xip[:, :, 1:1 + HW] = xi
    xrp[:, :, 0] = xr.mean(-1)
    xip[:, :, 0] = xi.mean(-1)
    xrp[:, :, :S] += posb.real[None, :, :S]
    xip[:, :, :S] += posb.imag[None, :, :S]
    in_maps = []
    for c in range(NCORES):
        m = dict(shared)
        m["xr"] = np.ascontiguousarray(
            xrp[c * BPC:(c + 1) * BPC].transpose(1, 0, 2)).astype(bf16)
        m["xi"] = np.ascontiguousarray(
            xip[c * BPC:(c + 1) * BPC].transpose(1, 0, 2)).astype(bf16)
        in_maps.append(m)
    return in_maps


def _run(inputs, trace=False, **kw):
    from concourse.bass_utils import run_bass_kernel_spmd
    if "nc" not in _cached:
        _cached["nc"] = _build()
    nc = _cached["nc"]
    in_maps = _host_prep(inputs)
    res = run_bass_kernel_spmd(nc, in_maps, core_ids=list(range(NCORES)),
                               trace=trace, **kw)
    out = np.empty((B, OUT), np.complex64)
    for c in range(NCORES):
        out[c * BPC:(c + 1) * BPC] = (res.results[c]["yr"]
                                      + 1j * res.results[c]["yi"])
    return out, res


def kernel(**inputs) -> np.ndarray:
    out, _ = _run(inputs)
    return out
